# revision 9
# baseline (speedup 1.0000x reference)
"""Multi-head self-attention (B=2, N=4096, D=768, H=12, dh=64) on 8 trn2 NeuronCores.

Sharding: core c handles batch b=c//4 and heads 3*(c%4)..3*(c%4)+2 (head-parallel
attention); an AllGather per 512-token i-chunk redistributes head-outputs so each
core projects its own 192 output columns for all tokens (column-parallel output
projection), assembled host-side.

Per-core pipeline:
  pass 1: fp32r projection of k and v for all tokens. k is written as fp8e4
    (scale folded host-side into w columns) in the DoubleRow layout
    [32 part, 2 dh-half, tok] with the 3 heads stacked on partitions 0:96;
    v is written fp16, PE-transposed into v_sb as [v_h | ones64] blocks per
    128-key chunk (the ones columns make the PV matmul emit the softmax
    denominator replicated across 64 partitions for free).
  pass 2 (interleaved with attention): q projected per 512-token chunk the
    same way, fp8 DoubleRow layout.
  attention per (i-chunk, key-pair, head): one fp8 DoubleRow QK matmul
    (row-tiled at partition 32h, 256 PE cycles) -> psum scores u = s/8 ->
    exp(8u) split 3:2 between ACT table exp and a 2-op DVE polynomial
    (taylor4(u) then x^8, fp16 intermediate) -> fp16 PV with [v|ones]
    stationary accumulating numerator rows 0:64 and denominator rows 64:128
    -> reciprocal[64,512] + multiply -> fp16 AllGather -> output projection.
"""
import sys

sys.path.insert(0, "/opt/trn_rl_repo")

import numpy as np

import concourse.bass as bass
import concourse.mybir as mybir
import concourse.tile as tile
import concourse.bacc as bacc
from concourse.masks import make_identity

N_CORES = 8
B, N, D, H, DH = 2, 4096, 768, 12, 64
HPC = 3            # heads per core
SCALE = D ** -0.5
ALPHA = float(np.sqrt(SCALE / 8.0))   # folded into q and k host-side
F32 = mybir.dt.float32
F32R = mybir.dt.float32r
F16 = mybir.dt.float16
F8 = mybir.dt.float8e4
AF = mybir.ActivationFunctionType
DR = mybir.MatmulPerfMode.DoubleRow
IC = 512           # query chunk size
NIC = N // IC      # 8 i-chunks
NTAU = N // IC     # token chunks for projection
VW = 384           # v_sb cols per 128-key chunk: [v0|1*64|v1|1*64|v2|1*64]
EXP_GROUP = 5      # exp batches: i%5 < EXP_DVE of each 5 go to DVE
EXP_DVE = 2


# ---------------------------------------------------------------- custom DVE exp
def _register_exp_ops():
    """exp(8u) as two DVE ops: EXP_P4_ANT = taylor4(u); EXP_SQ8_ANT = x^8."""
    import concourse.dve_ops as dve_ops
    from concourse.dve_ops import DveOp, OPS, CUSTOM_DVE_SPECS, _SUB_OPCODE_FOR_NAME
    from concourse.dve_spec import Spec, Src0, C0, C1, C2, One, sq, lower
    from concourse.dve_uop import DveOpSpec

    if "EXP_P4_ANT" in _SUB_OPCODE_FOR_NAME:
        return dve_ops.EXP_P4_ANT, dve_ops.EXP_SQ8_ANT

    u = Src0
    p4 = ((((u * C0) + C1) * u + C2) * u + One) * u + One  # c0=1/24 c1=1/6 c2=1/2
    spec_p4 = Spec(
        body=p4,
        reference=lambda in0, in1, s0, s1, imm2: (
            (((in0 * s0 + s1) * in0 + imm2) * in0 + 1.0) * in0 + 1.0
        ),
    )
    spec_sq8 = Spec(
        body=sq(sq(sq(Src0))),
        reference=lambda in0, in1, s0, s1, imm2: in0 ** 8,
    )

    def _mk(name, spec):
        opcode = max(_SUB_OPCODE_FOR_NAME.values()) + 1
        _SUB_OPCODE_FOR_NAME[name] = opcode
        shas = {}
        for ver in ("v3", "v4"):
            s = DveOpSpec(
                name=name, opcode=opcode, uops=lower(spec, ver=ver), rd1_en=False
            )
            shas[ver] = s.sha(ver)
        op = DveOp(name, spec, subdim=False, uops_sha=shas)
        OPS.append(op)
        CUSTOM_DVE_SPECS[name] = spec
        setattr(dve_ops, name, op)
        return op

    p4_op = _mk("EXP_P4_ANT", spec_p4)
    sq8_op = _mk("EXP_SQ8_ANT", spec_sq8)
    return p4_op, sq8_op


# ---------------------------------------------------------------- program build
_PROG_CACHE = {}


def build_program(use_dve_exp=True):
    key = ("prog", use_dve_exp)
    if key in _PROG_CACHE:
        return _PROG_CACHE[key]
    p4_op, sq8_op = _register_exp_ops()

    nc = bacc.Bacc("TRN2", target_bir_lowering=False, debug=False, num_devices=N_CORES)

    xT = nc.dram_tensor("xT", [D, N], F32, kind="ExternalInput").ap()
    wqkv = nc.dram_tensor("wqkv", [D, 768], F32, kind="ExternalInput").ap()
    bqkv = nc.dram_tensor("bqkv", [6, 128], F32, kind="ExternalInput").ap()
    wout = nc.dram_tensor("wout", [D, 192], F16, kind="ExternalInput").ap()
    bout = nc.dram_tensor("bout", [2, 128], F32, kind="ExternalInput").ap()
    y = nc.dram_tensor("y", [HPC * DH, N], F32, kind="ExternalOutput").ap()

    xT_r = xT.bitcast(F32R)
    wqkv_r = wqkv.bitcast(F32R)

    with tile.TileContext(nc, trace_sim=False) as tc:
        with (
            tc.tile_pool(name="consts", bufs=1) as consts,
            tc.tile_pool(name="persist", bufs=1) as persist,
            tc.tile_pool(name="otp", bufs=2) as otp,
            tc.tile_pool(name="spsum", bufs=2, space="PSUM") as spsum,
            tc.tile_pool(name="opsum", bufs=1, space="PSUM") as opsum,
            tc.tile_pool(name="ppsum", bufs=1, space="PSUM") as ppsum,
            tc.tile_pool(name="dram", bufs=1, space="DRAM") as dram,
        ):
            # ---------------- constants
            ident_f = consts.tile([128, 128], F32)
            make_identity(nc, ident_f[:])
            ident16 = consts.tile([128, 128], F16)
            nc.scalar.copy(ident16[:], ident_f[:])

            wq_sb = consts.tile([128, 6 * 768], F32R)  # 6 f-chunks of [128,768]
            for fc in range(6):
                nc.sync.dma_start(
                    out=wq_sb[:, fc * 768 : (fc + 1) * 768],
                    in_=wqkv_r[fc * 128 : (fc + 1) * 128, :],
                )
            bq_sb = consts.tile([128, 6], F32)  # per-m-chunk bias columns
            for m in range(6):
                nc.sync.dma_start(
                    out=bq_sb[:, m : m + 1],
                    in_=bqkv[m : m + 1, :].rearrange("a p -> p a"),
                )

            # ---------------- persistent activations
            # fp8 DoubleRow layouts: [part 32h..32h+32) = head h, dim1 = dh half
            kT8 = persist.tile([128, 2, N], F8, tag="kT8")
            q8t = [
                persist.tile([128, 2, IC], F8, tag=f"q8_{i}", name=f"q8_{i}")
                for i in range(NIC)
            ]
            # v_sb: per 128-key chunk jc: [v0|ones|v1|ones|v2|ones] fp16
            v_sb = persist.tile([128, 32 * VW], F16)
            v_view = v_sb[:].rearrange("p (j c) -> p j c", c=VW)
            for h in range(HPC):
                nc.vector.memset(v_view[:, :, h * 128 + 64 : h * 128 + 128], 1.0)

            # AllGather eighths: in [192, 512] -> out [768, 512] (rank-major rows)
            oT_q = [dram.tile([HPC * DH, IC], F16, name=f"oT_q{i}") for i in range(8)]
            ag_q = [dram.tile([D, IC], F16, name=f"ag_q{i}") for i in range(8)]

            with tc.tile_pool(name="work", bufs=2) as work, \
                 tc.tile_pool(name="ptp", bufs=3) as ptp:

                def load_xts(tau):
                    ts = slice(tau * IC, (tau + 1) * IC)
                    xts = []
                    for fc in range(6):
                        xt = work.tile([128, IC], F32R, tag="xt", bufs=7)
                        nc.sync.dma_start(
                            out=xt[:], in_=xT_r[fc * 128 : (fc + 1) * 128, ts]
                        )
                        xts.append(xt)
                    return xts

                def proj_chunk(xts, m, out_psum):
                    """fp32r matmul of m-chunk into out_psum [128, 512]."""
                    for fc in range(6):
                        nc.tensor.matmul(
                            out_psum,
                            wq_sb[:, fc * 768 + m * 128 : fc * 768 + (m + 1) * 128],
                            xts[fc][:],
                            start=(fc == 0),
                            stop=(fc == 5),
                        )

                # ---------------- pass 1: k + v projection (q deferred)
                for tau in range(NTAU):
                    ts = slice(tau * IC, (tau + 1) * IC)
                    xts = load_xts(tau)
                    # k halves -> kT8 fp8 (m-chunks 2,3)
                    for half in range(2):
                        sp = spsum.tile([128, 1024], F32, tag="sp", bufs=2)
                        proj_chunk(xts, 2 + half, sp[:, 0:IC])
                        nc.scalar.activation(
                            kT8[0:96, half : half + 1, ts],
                            sp[0:96, 0:IC],
                            AF.Identity,
                            bias=bq_sb[0:96, 2 + half : 3 + half],
                        )
                    # v chunks -> vt fp16 (m-chunks 4,5)
                    vt01 = work.tile([128, IC], F16, tag="vt01")
                    vt2 = work.tile([64, IC], F16, tag="vt2")
                    sp = spsum.tile([128, 1024], F32, tag="sp", bufs=2)
                    proj_chunk(xts, 4, sp[:, 0:IC])
                    nc.scalar.activation(
                        vt01[:], sp[:, 0:IC], AF.Identity, bias=bq_sb[:, 4:5]
                    )
                    sp = spsum.tile([128, 1024], F32, tag="sp", bufs=2)
                    proj_chunk(xts, 5, sp[:, 0:IC])
                    nc.scalar.activation(
                        vt2[:], sp[0:64, 0:IC], AF.Identity, bias=bq_sb[0:64, 5:6]
                    )
                    # transpose v into token-major [v|ones] layout, 2 t-blocks
                    # per psum tile: [t.v01(128) | t+1.v01(128) | t.v2(64) | t+1.v2(64)]
                    for tp in range(2):
                        pv = spsum.tile([128, 2048], F16, tag="sp", bufs=2)
                        for s in range(2):
                            t = 2 * tp + s
                            nc.tensor.transpose(
                                pv[:, s * 128 : (s + 1) * 128],
                                vt01[:, t * 128 : (t + 1) * 128],
                                ident16[:],
                            )
                            nc.tensor.transpose(
                                pv[:, 256 + s * 64 : 256 + (s + 1) * 64],
                                vt2[0:64, t * 128 : (t + 1) * 128],
                                ident16[0:64, 0:64],
                            )
                        jc0 = 4 * tau + 2 * tp
                        pv01 = pv[:, 0:256].rearrange("p (t c) -> p t c", c=128)
                        pv2 = pv[:, 256:384].rearrange("p (t c) -> p t c", c=64)
                        nc.vector.tensor_copy(
                            out=v_view[:, jc0 : jc0 + 2, 0:64], in_=pv01[:, :, 0:64]
                        )
                        nc.vector.tensor_copy(
                            out=v_view[:, jc0 : jc0 + 2, 128:192],
                            in_=pv01[:, :, 64:128],
                        )
                        nc.vector.tensor_copy(
                            out=v_view[:, jc0 : jc0 + 2, 256:320], in_=pv2[:, :, :]
                        )

                # ---------------- pass 2: q projection + attention per i-chunk
                exp_batch_idx = [0]

                def exp_batch(sp):
                    """exp(8u) on a [128,1024] psum batch -> fp16 SBUF tile."""
                    i = exp_batch_idx[0]
                    exp_batch_idx[0] += 1
                    pt = ptp.tile([128, 1024], F16, tag="pt", bufs=4)
                    if use_dve_exp and (i % EXP_GROUP) < EXP_DVE:
                        tmp = ptp.tile([128, 1024], F16, tag="exptmp", bufs=2)
                        nc.vector._custom_dve(
                            p4_op, out=tmp[:], in0=sp[:],
                            s0=1.0 / 24, s1=1.0 / 6, imm2=0.5,
                        )
                        nc.vector._custom_dve(sq8_op, out=pt[:], in0=tmp[:])
                    else:
                        nc.scalar.activation(pt[:], sp[:], AF.Exp, scale=8.0)
                    return pt

                for ic in range(NIC):
                    ts = slice(ic * IC, (ic + 1) * IC)
                    # q projection for this token chunk (m-chunks 0,1)
                    xts = load_xts(ic)
                    for half in range(2):
                        pp = ppsum.tile([128, IC], F32, tag="pp", bufs=1)
                        proj_chunk(xts, half, pp[:])
                        nc.scalar.activation(
                            q8t[ic][0:96, half : half + 1, :],
                            pp[0:96, :],
                            AF.Identity,
                            bias=bq_sb[0:96, half : half + 1],
                        )
                    # attention
                    po = [
                        opsum.tile([128, IC], F32, tag=f"po{h}", bufs=1,
                                   name=f"po{h}_{ic}")
                        for h in range(HPC)
                    ]
                    for jp in range(16):
                        for h in range(HPC):
                            sp = spsum.tile([128, 1024], F32, tag="sp", bufs=2)
                            for s in range(2):
                                jc = 2 * jp + s
                                nc.tensor.matmul(
                                    sp[:, s * IC : (s + 1) * IC],
                                    kT8[32 * h : 32 * h + 32, :,
                                        jc * 128 : (jc + 1) * 128],
                                    q8t[ic][32 * h : 32 * h + 32, :, :],
                                    start=True, stop=True,
                                    perf_mode=DR,
                                    tile_position=(32 * h, 0),
                                )
                            pt = exp_batch(sp)
                            for s in range(2):
                                jc = 2 * jp + s
                                nc.tensor.matmul(
                                    po[h][:],
                                    v_sb[:, jc * VW + h * 128 : jc * VW + (h + 1) * 128],
                                    pt[:, s * IC : (s + 1) * IC],
                                    start=(jc == 0), stop=(jc == 31),
                                )
                    for h in range(HPC):
                        rr = otp.tile([128, IC], F32, tag="rr", bufs=2)
                        nc.vector.reciprocal(rr[0:64, :], po[h][64:128, :])
                        ot = otp.tile([64, IC], F16, tag="ot", bufs=3)
                        nc.vector.tensor_tensor(
                            out=ot[:], in0=po[h][0:64, :], in1=rr[0:64, :],
                            op=mybir.AluOpType.mult,
                        )
                        nc.sync.dma_start(
                            out=oT_q[ic][64 * h : 64 * h + 64, :], in_=ot[:]
                        )
                    nc.gpsimd.collective_compute(
                        "AllGather",
                        mybir.AluOpType.bypass,
                        replica_groups=[[0, 1, 2, 3], [4, 5, 6, 7]],
                        ins=[oT_q[ic][:]],
                        outs=[ag_q[ic][:]],
                    )

            # ---------------- phase 3: output projection (column-parallel)
            with tc.tile_pool(name="ph3", bufs=1) as ph3:
                wo_sb = ph3.tile([128, 6 * 192], F16)  # w_out col-slice, 6 d-chunks
                for dc in range(6):
                    nc.sync.dma_start(
                        out=wo_sb[:, dc * 192 : (dc + 1) * 192],
                        in_=wout[dc * 128 : (dc + 1) * 128, :],
                    )
                bo_sb = ph3.tile([128, 2], F32)
                for m in range(2):
                    nc.sync.dma_start(
                        out=bo_sb[:, m : m + 1],
                        in_=bout[m : m + 1, :].rearrange("a p -> p a"),
                    )
                for qtr in range(8):
                    agr = ag_q[qtr][:]
                    ogs = []
                    for dc in range(6):
                        og = ph3.tile([128, IC], F16, tag="og", bufs=8)
                        nc.sync.dma_start(
                            out=og[:], in_=agr[dc * 128 : (dc + 1) * 128, :]
                        )
                        ogs.append(og)
                    for ec, (elo, ew) in enumerate(((0, 128), (128, 64))):
                        py = spsum.tile([128, 1024], F32, tag="sp", bufs=2)
                        for dc in range(6):
                            nc.tensor.matmul(
                                py[0:ew, 0:IC],
                                wo_sb[:, dc * 192 + elo : dc * 192 + elo + ew],
                                ogs[dc][:],
                                start=(dc == 0), stop=(dc == 5),
                            )
                        ysb = ph3.tile([128, IC], F32, tag="ysb", bufs=3)
                        nc.vector.tensor_scalar(
                            out=ysb[0:ew, :], in0=py[0:ew, 0:IC],
                            scalar1=bo_sb[0:ew, ec : ec + 1], scalar2=None,
                            op0=mybir.AluOpType.add,
                        )
                        nc.sync.dma_start(
                            out=y[elo : elo + ew, qtr * IC : (qtr + 1) * IC],
                            in_=ysb[0:ew, :],
                        )

    nc.compile()
    _PROG_CACHE[key] = nc
    return nc


# ---------------------------------------------------------------- host wrapper
def make_in_maps(x, w_qkv, b_qkv, w_out, b_out):
    """Build the 8 per-core input dicts from full inputs."""
    in_maps = []
    xTb = [np.ascontiguousarray(x[b].T) for b in range(B)]  # [768, 4096]
    for c in range(N_CORES):
        b = c // 4
        hs = HPC * (c % 4)

        def sect(kind, h):  # q=0,k=1,v=2
            lo = kind * (H * DH) + h * DH
            return w_qkv[:, lo : lo + DH], b_qkv[lo : lo + DH]

        z32 = np.zeros((D, 32), np.float32)
        z64 = np.zeros((D, 64), np.float32)
        bz32 = np.zeros(32, np.float32)
        bz64 = np.zeros(64, np.float32)
        qs = [sect(0, hs + i) for i in range(3)]
        ks = [sect(1, hs + i) for i in range(3)]
        vs = [sect(2, hs + i) for i in range(3)]
        a = np.float32(ALPHA)
        # m-chunks: qA, qB, kA, kB (heads stacked, 32-dh halves), v01, v2
        cols = np.concatenate(
            [a * qs[0][0][:, 0:32], a * qs[1][0][:, 0:32], a * qs[2][0][:, 0:32], z32,
             a * qs[0][0][:, 32:64], a * qs[1][0][:, 32:64], a * qs[2][0][:, 32:64], z32,
             a * ks[0][0][:, 0:32], a * ks[1][0][:, 0:32], a * ks[2][0][:, 0:32], z32,
             a * ks[0][0][:, 32:64], a * ks[1][0][:, 32:64], a * ks[2][0][:, 32:64], z32,
             vs[0][0], vs[1][0], vs[2][0], z64], axis=1).astype(np.float32)
        bias = np.concatenate(
            [a * qs[0][1][0:32], a * qs[1][1][0:32], a * qs[2][1][0:32], bz32,
             a * qs[0][1][32:64], a * qs[1][1][32:64], a * qs[2][1][32:64], bz32,
             a * ks[0][1][0:32], a * ks[1][1][0:32], a * ks[2][1][0:32], bz32,
             a * ks[0][1][32:64], a * ks[1][1][32:64], a * ks[2][1][32:64], bz32,
             vs[0][1], vs[1][1], vs[2][1], bz64]).astype(np.float32)
        q = c % 4
        bo = np.zeros((2, 128), np.float32)
        bo[0, :] = b_out[192 * q : 192 * q + 128]
        bo[1, :64] = b_out[192 * q + 128 : 192 * q + 192]
        in_maps.append({
            "xT": xTb[b],
            "wqkv": np.ascontiguousarray(cols),
            "bqkv": np.ascontiguousarray(bias.reshape(6, 128)),
            "wout": np.ascontiguousarray(
                w_out[:, 192 * q : 192 * (q + 1)].astype(np.float16)),
            "bout": bo,
        })
    return in_maps


def assemble_output(results):
    out = np.empty((B, N, D), dtype=np.float32)
    for c in range(N_CORES):
        b = c // 4
        q = c % 4
        out[b, :, 192 * q : 192 * (q + 1)] = results[c]["y"].T
    return out


def kernel(x, w_qkv, b_qkv, w_out, b_out):
    from concourse.bass_utils import run_bass_kernel_spmd

    x = np.asarray(x, dtype=np.float32)
    nc = build_program()
    in_maps = make_in_maps(
        x, np.asarray(w_qkv, np.float32), np.asarray(b_qkv, np.float32),
        np.asarray(w_out, np.float32), np.asarray(b_out, np.float32))
    res = run_bass_kernel_spmd(nc, in_maps, core_ids=list(range(N_CORES)))
    return assemble_output(res.results)


# revision 12
# speedup vs baseline: 1.0855x; 1.0855x over previous
"""Multi-head self-attention (B=2, N=4096, D=768, H=12, dh=64) on 8 trn2 NeuronCores.

Sharding: core c handles batch b=c//4 and heads 3*(c%4)..3*(c%4)+2 (head-parallel
attention); an AllGather per 512-token i-chunk redistributes head-outputs so each
core projects its own 192 output columns for all tokens (column-parallel output
projection), assembled host-side.

Per-core pipeline (all matmuls fp16 except psum accumulate):
  x is shipped fp16 and kept resident in SBUF (48KB/partition), so projections
  read it directly with 1024-wide moving operands and no per-pass DMA.
  pass 1: project k (pre-scaled by softmax_scale/8 host-side) and v for all
    tokens; k kept d-on-partition fp16 in row-tiled head-pair layout, v
    PE-transposed into v_sb as [v_h | ones64] fp16 blocks per 128-key chunk —
    the ones columns make each PV matmul emit the softmax denominator
    replicated across psum partitions 64:128 for free.
  pass 2 (interleaved with attention): q projected per 512-token chunk.
  attention per (i-chunk, key-chunk): row-tiled QK pair (concurrent PE
    row-strips) -> psum u = s/8 -> exp(8u) split ~11:5 between ACT table exp
    and a 2-op DVE polynomial (taylor4(u), x^8), DVE batches interleaved so
    the PE never idles a full HAM window -> fp16 PV with [v|ones] stationary
    -> reciprocal[64,512] + multiply -> fp16 AllGather -> column-parallel
    output projection with DVE bias add.
"""
import sys

sys.path.insert(0, "/opt/trn_rl_repo")

import numpy as np

import concourse.bass as bass
import concourse.mybir as mybir
import concourse.tile as tile
import concourse.bacc as bacc
from concourse.masks import make_identity

N_CORES = 8
B, N, D, H, DH = 2, 4096, 768, 12, 64
HPC = 3            # heads per core
SCALE = D ** -0.5
F32 = mybir.dt.float32
F16 = mybir.dt.float16
AF = mybir.ActivationFunctionType
IC = 512           # query chunk size
NIC = N // IC      # 8 i-chunks
VW = 384           # v_sb cols per 128-key chunk: [v0|1*64|v1|1*64|v2|1*64]
EXP_GROUP = 16     # exp batches: (i*5) % 16 < 5 go to the DVE path (interleaved)
EXP_DVE = 5


# ---------------------------------------------------------------- custom DVE exp
def _register_exp_ops():
    """exp(8u) as two DVE ops: EXP_P4_ANT = taylor4(u); EXP_SQ8_ANT = x^8."""
    import concourse.dve_ops as dve_ops
    from concourse.dve_ops import DveOp, OPS, CUSTOM_DVE_SPECS, _SUB_OPCODE_FOR_NAME
    from concourse.dve_spec import Spec, Src0, C0, C1, C2, One, sq, lower
    from concourse.dve_uop import DveOpSpec

    if "EXP_P4_ANT" in _SUB_OPCODE_FOR_NAME:
        return dve_ops.EXP_P4_ANT, dve_ops.EXP_SQ8_ANT

    u = Src0
    p4 = ((((u * C0) + C1) * u + C2) * u + One) * u + One  # c0=1/24 c1=1/6 c2=1/2
    spec_p4 = Spec(
        body=p4,
        reference=lambda in0, in1, s0, s1, imm2: (
            (((in0 * s0 + s1) * in0 + imm2) * in0 + 1.0) * in0 + 1.0
        ),
    )
    spec_sq8 = Spec(
        body=sq(sq(sq(Src0))),
        reference=lambda in0, in1, s0, s1, imm2: in0 ** 8,
    )

    def _mk(name, spec):
        opcode = max(_SUB_OPCODE_FOR_NAME.values()) + 1
        _SUB_OPCODE_FOR_NAME[name] = opcode
        shas = {}
        for ver in ("v3", "v4"):
            s = DveOpSpec(
                name=name, opcode=opcode, uops=lower(spec, ver=ver), rd1_en=False
            )
            shas[ver] = s.sha(ver)
        op = DveOp(name, spec, subdim=False, uops_sha=shas)
        OPS.append(op)
        CUSTOM_DVE_SPECS[name] = spec
        setattr(dve_ops, name, op)
        return op

    p4_op = _mk("EXP_P4_ANT", spec_p4)
    sq8_op = _mk("EXP_SQ8_ANT", spec_sq8)
    return p4_op, sq8_op


# ---------------------------------------------------------------- program build
_PROG_CACHE = {}


def build_program(use_dve_exp=True):
    key = ("prog", use_dve_exp)
    if key in _PROG_CACHE:
        return _PROG_CACHE[key]
    p4_op, sq8_op = _register_exp_ops()

    nc = bacc.Bacc("TRN2", target_bir_lowering=False, debug=False, num_devices=N_CORES)

    xT = nc.dram_tensor("xT", [D, N], F16, kind="ExternalInput").ap()
    wqkv = nc.dram_tensor("wqkv", [D, 768], F16, kind="ExternalInput").ap()
    bqkv = nc.dram_tensor("bqkv", [6, 128], F32, kind="ExternalInput").ap()
    wout = nc.dram_tensor("wout", [D, 192], F16, kind="ExternalInput").ap()
    bout = nc.dram_tensor("bout", [2, 128], F32, kind="ExternalInput").ap()
    y = nc.dram_tensor("y", [HPC * DH, N], F32, kind="ExternalOutput").ap()

    with tile.TileContext(nc, trace_sim=False) as tc:
        with (
            tc.tile_pool(name="consts", bufs=1) as consts,
            tc.tile_pool(name="persist", bufs=1) as persist,
            tc.tile_pool(name="otp", bufs=2) as otp,
            tc.tile_pool(name="spsum", bufs=2, space="PSUM") as spsum,
            tc.tile_pool(name="opsum", bufs=1, space="PSUM") as opsum,
            tc.tile_pool(name="ppsum", bufs=1, space="PSUM") as ppsum,
            tc.tile_pool(name="dram", bufs=1, space="DRAM") as dram,
        ):
            # ---------------- constants + resident fp16 x
            ident_f = consts.tile([128, 128], F32)
            make_identity(nc, ident_f[:])
            ident16 = consts.tile([128, 128], F16)
            nc.scalar.copy(ident16[:], ident_f[:])

            x16 = persist.tile([128, 6 * N], F16)  # 6 d-chunks of [128, 4096]
            for fc in range(6):
                for nq in range(8):
                    nc.sync.dma_start(
                        out=x16[:, fc * N + nq * IC : fc * N + (nq + 1) * IC],
                        in_=xT[fc * 128 : (fc + 1) * 128,
                               nq * IC : (nq + 1) * IC],
                    )
            wq_sb = consts.tile([128, 6 * 768], F16)  # 6 f-chunks of [128,768]
            for fc in range(6):
                nc.sync.dma_start(
                    out=wq_sb[:, fc * 768 : (fc + 1) * 768],
                    in_=wqkv[fc * 128 : (fc + 1) * 128, :],
                )
            bq_sb = consts.tile([128, 6], F32)  # per-m-chunk bias columns
            for m in range(6):
                nc.sync.dma_start(
                    out=bq_sb[:, m : m + 1],
                    in_=bqkv[m : m + 1, :].rearrange("a p -> p a"),
                )

            # ---------------- persistent activations (all fp16)
            qT01 = persist.tile([128, N], F16)  # rows 0:64 head0 qT, 64:128 head1
            qT2 = persist.tile([128, N], F16)   # head2 qT duplicated in both halves
            kT01 = persist.tile([128, N], F16)  # scaled kT, heads 0/1
            kT2 = persist.tile([128, N], F16)   # scaled kT head2, duplicated
            v_sb = persist.tile([128, 32 * VW], F16)
            v_view = v_sb[:].rearrange("p (j c) -> p j c", c=VW)
            for h in range(HPC):
                nc.vector.memset(v_view[:, :, h * 128 + 64 : h * 128 + 128], 1.0)

            # AllGather eighths: in [192, 512] -> out [768, 512] (rank-major rows)
            oT_q = [dram.tile([HPC * DH, IC], F16, name=f"oT_q{i}") for i in range(8)]
            ag_q = [dram.tile([D, IC], F16, name=f"ag_q{i}") for i in range(8)]

            with tc.tile_pool(name="work", bufs=2) as work, \
                 tc.tile_pool(name="ptp", bufs=3) as ptp:

                def proj_chunk(m, tsl, out_psum):
                    """fp16 matmul of m-chunk over token slice tsl into psum."""
                    for fc in range(6):
                        nc.tensor.matmul(
                            out_psum,
                            wq_sb[:, fc * 768 + m * 128 : fc * 768 + (m + 1) * 128],
                            x16[:, fc * N + tsl.start : fc * N + tsl.stop],
                            start=(fc == 0),
                            stop=(fc == 5),
                        )

                # ---------------- pass 1: k + v projection (q deferred)
                for tau in range(8):
                    ts = slice(tau * IC, (tau + 1) * IC)
                    for m, dst in ((2, kT01), (3, kT2)):
                        sp = spsum.tile([128, 1024], F32, tag="sp", bufs=2)
                        proj_chunk(m, ts, sp[:, 0:IC])
                        nc.scalar.activation(
                            dst[:, ts], sp[:, 0:IC], AF.Identity,
                            bias=bq_sb[:, m : m + 1],
                        )
                    vt01 = work.tile([128, IC], F16, tag="vt01")
                    vt2 = work.tile([64, IC], F16, tag="vt2")
                    sp = spsum.tile([128, 1024], F32, tag="sp", bufs=2)
                    proj_chunk(4, ts, sp[:, 0:IC])
                    nc.scalar.activation(
                        vt01[:], sp[:, 0:IC], AF.Identity, bias=bq_sb[:, 4:5]
                    )
                    sp = spsum.tile([128, 1024], F32, tag="sp", bufs=2)
                    proj_chunk(5, ts, sp[:, 0:IC])
                    nc.scalar.activation(
                        vt2[:], sp[0:64, 0:IC], AF.Identity, bias=bq_sb[0:64, 5:6]
                    )
                    # transpose v into token-major [v|ones] layout, 4 t-blocks
                    # per psum tile: [v01(t0..t3) 512 | v2(t0..t3) 256]
                    pv = spsum.tile([128, 2048], F16, tag="sp", bufs=2)
                    for s in range(4):
                        nc.tensor.transpose(
                            pv[:, s * 128 : (s + 1) * 128],
                            vt01[:, s * 128 : (s + 1) * 128],
                            ident16[:],
                        )
                        nc.tensor.transpose(
                            pv[:, 512 + s * 64 : 512 + (s + 1) * 64],
                            vt2[0:64, s * 128 : (s + 1) * 128],
                            ident16[0:64, 0:64],
                        )
                    jc0 = 4 * tau
                    pv01 = pv[:, 0:512].rearrange("p (t c) -> p t c", c=128)
                    pv2 = pv[:, 512:768].rearrange("p (t c) -> p t c", c=64)
                    nc.vector.tensor_copy(
                        out=v_view[:, jc0 : jc0 + 4, 0:64], in_=pv01[:, :, 0:64]
                    )
                    nc.vector.tensor_copy(
                        out=v_view[:, jc0 : jc0 + 4, 128:192],
                        in_=pv01[:, :, 64:128],
                    )
                    nc.vector.tensor_copy(
                        out=v_view[:, jc0 : jc0 + 4, 256:320], in_=pv2[:, :, :]
                    )

                # ---------------- pass 2: q projection + attention per i-chunk
                exp_batch_idx = [0]

                def exp_batch(sp):
                    """exp(8u) on a [128,1024] psum batch -> fp16 SBUF tile."""
                    i = exp_batch_idx[0]
                    exp_batch_idx[0] += 1
                    pt = ptp.tile([128, 1024], F16, tag="pt", bufs=4)
                    if use_dve_exp and ((i * EXP_DVE) % EXP_GROUP) < EXP_DVE:
                        tmp = ptp.tile([128, 1024], F16, tag="exptmp", bufs=2)
                        nc.vector._custom_dve(
                            p4_op, out=tmp[:], in0=sp[:],
                            s0=1.0 / 24, s1=1.0 / 6, imm2=0.5,
                        )
                        nc.vector._custom_dve(sq8_op, out=pt[:], in0=tmp[:])
                    else:
                        nc.scalar.activation(pt[:], sp[:], AF.Exp, scale=8.0)
                    return pt

                for ic in range(NIC):
                    isl = slice(ic * IC, (ic + 1) * IC)
                    # q projection for this token chunk (m-chunks 0,1)
                    for m, dst in ((0, qT01), (1, qT2)):
                        pp = ppsum.tile([128, IC], F32, tag="pp", bufs=1)
                        proj_chunk(m, isl, pp[:])
                        nc.scalar.activation(
                            dst[:, isl], pp[:], AF.Identity,
                            bias=bq_sb[:, m : m + 1],
                        )
                    # attention
                    po = [
                        opsum.tile([128, IC], F32, tag=f"po{h}", bufs=1,
                                   name=f"po{h}_{ic}")
                        for h in range(HPC)
                    ]
                    # heads 0,1: one 128-key chunk per batch, row-tiled pair
                    for jc in range(32):
                        sp = spsum.tile([128, 1024], F32, tag="sp", bufs=2)
                        nc.tensor.matmul(
                            sp[:, 0:IC],
                            kT01[0:64, jc * 128 : (jc + 1) * 128],
                            qT01[0:64, isl],
                            start=True, stop=True, tile_position=(0, 0),
                        )
                        nc.tensor.matmul(
                            sp[:, IC:1024],
                            kT01[64:128, jc * 128 : (jc + 1) * 128],
                            qT01[64:128, isl],
                            start=True, stop=True, tile_position=(64, 0),
                        )
                        pt = exp_batch(sp)
                        nc.tensor.matmul(
                            po[0][:],
                            v_sb[:, jc * VW : jc * VW + 128],
                            pt[:, 0:IC],
                            start=(jc == 0), stop=(jc == 31),
                        )
                        nc.tensor.matmul(
                            po[1][:],
                            v_sb[:, jc * VW + 128 : jc * VW + 256],
                            pt[:, IC:1024],
                            start=(jc == 0), stop=(jc == 31),
                        )
                    # head 2: row-tiled pair over adjacent key chunks
                    for t in range(16):
                        sp = spsum.tile([128, 1024], F32, tag="sp", bufs=2)
                        nc.tensor.matmul(
                            sp[:, 0:IC],
                            kT2[0:64, (2 * t) * 128 : (2 * t + 1) * 128],
                            qT2[0:64, isl],
                            start=True, stop=True, tile_position=(0, 0),
                        )
                        nc.tensor.matmul(
                            sp[:, IC:1024],
                            kT2[64:128, (2 * t + 1) * 128 : (2 * t + 2) * 128],
                            qT2[64:128, isl],
                            start=True, stop=True, tile_position=(64, 0),
                        )
                        pt = exp_batch(sp)
                        for s in range(2):
                            jc = 2 * t + s
                            nc.tensor.matmul(
                                po[2][:],
                                v_sb[:, jc * VW + 256 : jc * VW + 384],
                                pt[:, s * IC : (s + 1) * IC],
                                start=(jc == 0), stop=(jc == 31),
                            )
                    for h in range(HPC):
                        rr = otp.tile([128, IC], F32, tag="rr", bufs=2)
                        nc.vector.reciprocal(rr[0:64, :], po[h][64:128, :])
                        ot = otp.tile([64, IC], F16, tag="ot", bufs=3)
                        nc.vector.tensor_tensor(
                            out=ot[:], in0=po[h][0:64, :], in1=rr[0:64, :],
                            op=mybir.AluOpType.mult,
                        )
                        nc.sync.dma_start(
                            out=oT_q[ic][64 * h : 64 * h + 64, :], in_=ot[:]
                        )
                    nc.gpsimd.collective_compute(
                        "AllGather",
                        mybir.AluOpType.bypass,
                        replica_groups=[[0, 1, 2, 3], [4, 5, 6, 7]],
                        ins=[oT_q[ic][:]],
                        outs=[ag_q[ic][:]],
                    )

            # ---------------- phase 3: output projection (column-parallel)
            with tc.tile_pool(name="ph3", bufs=1) as ph3:
                wo_sb = ph3.tile([128, 6 * 192], F16)  # w_out col-slice, 6 d-chunks
                for dc in range(6):
                    nc.sync.dma_start(
                        out=wo_sb[:, dc * 192 : (dc + 1) * 192],
                        in_=wout[dc * 128 : (dc + 1) * 128, :],
                    )
                bo_sb = ph3.tile([128, 2], F32)
                for m in range(2):
                    nc.sync.dma_start(
                        out=bo_sb[:, m : m + 1],
                        in_=bout[m : m + 1, :].rearrange("a p -> p a"),
                    )
                for qtr in range(8):
                    agr = ag_q[qtr][:]
                    ogs = []
                    for dc in range(6):
                        og = ph3.tile([128, IC], F16, tag="og", bufs=8)
                        nc.sync.dma_start(
                            out=og[:], in_=agr[dc * 128 : (dc + 1) * 128, :]
                        )
                        ogs.append(og)
                    for ec, (elo, ew) in enumerate(((0, 128), (128, 64))):
                        py = spsum.tile([128, 1024], F32, tag="sp", bufs=2)
                        for dc in range(6):
                            nc.tensor.matmul(
                                py[0:ew, 0:IC],
                                wo_sb[:, dc * 192 + elo : dc * 192 + elo + ew],
                                ogs[dc][:],
                                start=(dc == 0), stop=(dc == 5),
                            )
                        ysb = ph3.tile([128, IC], F32, tag="ysb", bufs=3)
                        nc.vector.tensor_scalar(
                            out=ysb[0:ew, :], in0=py[0:ew, 0:IC],
                            scalar1=bo_sb[0:ew, ec : ec + 1], scalar2=None,
                            op0=mybir.AluOpType.add,
                        )
                        nc.sync.dma_start(
                            out=y[elo : elo + ew, qtr * IC : (qtr + 1) * IC],
                            in_=ysb[0:ew, :],
                        )

    nc.compile()
    _PROG_CACHE[key] = nc
    return nc


# ---------------------------------------------------------------- host wrapper
def make_in_maps(x, w_qkv, b_qkv, w_out, b_out):
    """Build the 8 per-core input dicts from full inputs."""
    in_maps = []
    xTb = [np.ascontiguousarray(x[b].T.astype(np.float16)) for b in range(B)]
    kscale = np.float32(SCALE / 8.0)
    for c in range(N_CORES):
        b = c // 4
        hs = HPC * (c % 4)

        def sect(kind, h):  # q=0,k=1,v=2
            lo = kind * (H * DH) + h * DH
            return w_qkv[:, lo : lo + DH], b_qkv[lo : lo + DH]

        q0, bq0 = sect(0, hs); q1, bq1 = sect(0, hs + 1); q2, bq2 = sect(0, hs + 2)
        k0, bk0 = sect(1, hs); k1, bk1 = sect(1, hs + 1); k2, bk2 = sect(1, hs + 2)
        v0, bv0 = sect(2, hs); v1, bv1 = sect(2, hs + 1); v2, bv2 = sect(2, hs + 2)
        z = np.zeros_like(q2); bz = np.zeros_like(bq2)
        # m-chunks: [q0|q1], [q2|q2], [k0|k1]*s, [k2|k2]*s, [v0|v1], [v2|0]
        cols = np.concatenate(
            [q0, q1, q2, q2, k0 * kscale, k1 * kscale, k2 * kscale, k2 * kscale,
             v0, v1, v2, z], axis=1).astype(np.float16)
        bias = np.concatenate(
            [bq0, bq1, bq2, bq2, bk0 * kscale, bk1 * kscale, bk2 * kscale,
             bk2 * kscale, bv0, bv1, bv2, bz]).astype(np.float32)
        q = c % 4
        bo = np.zeros((2, 128), np.float32)
        bo[0, :] = b_out[192 * q : 192 * q + 128]
        bo[1, :64] = b_out[192 * q + 128 : 192 * q + 192]
        in_maps.append({
            "xT": xTb[b],
            "wqkv": np.ascontiguousarray(cols),
            "bqkv": np.ascontiguousarray(bias.reshape(6, 128)),
            "wout": np.ascontiguousarray(
                w_out[:, 192 * q : 192 * (q + 1)].astype(np.float16)),
            "bout": bo,
        })
    return in_maps


def assemble_output(results):
    out = np.empty((B, N, D), dtype=np.float32)
    for c in range(N_CORES):
        b = c // 4
        q = c % 4
        out[b, :, 192 * q : 192 * (q + 1)] = results[c]["y"].T
    return out


def kernel(x, w_qkv, b_qkv, w_out, b_out):
    from concourse.bass_utils import run_bass_kernel_spmd

    x = np.asarray(x, dtype=np.float32)
    nc = build_program()
    in_maps = make_in_maps(
        x, np.asarray(w_qkv, np.float32), np.asarray(b_qkv, np.float32),
        np.asarray(w_out, np.float32), np.asarray(b_out, np.float32))
    res = run_bass_kernel_spmd(nc, in_maps, core_ids=list(range(N_CORES)))
    return assemble_output(res.results)


# revision 16
# speedup vs baseline: 1.2645x; 1.1649x over previous
"""Multi-head self-attention (B=2, N=4096, D=768, H=12, dh=64) on 8 trn2 NeuronCores.

Sharding: core c handles batch b=c//4 and heads 3*(c%4)..3*(c%4)+2 (head-parallel
attention); an AllGather per 512-token i-chunk redistributes head-outputs so each
core projects its own 192 output columns for all tokens (column-parallel output
projection), assembled host-side.

Per-core pipeline (all matmuls fp16 except psum accumulate):
  x is shipped fp16 and kept resident in SBUF (48KB/partition), so projections
  read it directly with 1024-wide moving operands and no per-pass DMA.
  pass 1: project k (pre-scaled by softmax_scale/8 host-side) and v for all
    tokens; k kept d-on-partition fp16 in row-tiled head-pair layout, v
    PE-transposed into v_sb as [v_h | ones64] fp16 blocks per 128-key chunk —
    the ones columns make each PV matmul emit the softmax denominator
    replicated across psum partitions 64:128 for free.
  pass 2 (interleaved with attention): q projected per 512-token chunk.
  attention per (i-chunk, key-chunk): row-tiled QK pair (concurrent PE
    row-strips) -> psum u = s/8 -> exp(8u) split ~11:5 between ACT table exp
    and a 2-op DVE polynomial (taylor4(u), x^8), DVE batches interleaved so
    the PE never idles a full HAM window -> fp16 PV with [v|ones] stationary
    -> reciprocal[64,512] + multiply -> fp16 AllGather -> column-parallel
    output projection with DVE bias add.
"""
import sys

sys.path.insert(0, "/opt/trn_rl_repo")

import numpy as np

import concourse.bass as bass
import concourse.mybir as mybir
import concourse.tile as tile
import concourse.bacc as bacc
from concourse.masks import make_identity

N_CORES = 8
B, N, D, H, DH = 2, 4096, 768, 12, 64
HPC = 3            # heads per core
SCALE = D ** -0.5
F32 = mybir.dt.float32
F16 = mybir.dt.float16
AF = mybir.ActivationFunctionType
IC = 512           # query chunk size
NIC = N // IC      # 8 i-chunks
VW = 384           # v_sb cols per 128-key chunk: [v0|1*64|v1|1*64|v2|1*64]
EXP_GROUP = 16     # exp batches: (i*5) % 16 < 5 go to the DVE path (interleaved)
EXP_DVE = 5


# ---------------------------------------------------------------- custom DVE exp
def _register_exp_ops():
    """exp(8u) as two DVE ops: EXP_P4_ANT = taylor4(u); EXP_SQ8_ANT = x^8."""
    import concourse.dve_ops as dve_ops
    from concourse.dve_ops import DveOp, OPS, CUSTOM_DVE_SPECS, _SUB_OPCODE_FOR_NAME
    from concourse.dve_spec import Spec, Src0, C0, C1, C2, One, sq, lower
    from concourse.dve_uop import DveOpSpec

    if "EXP_P4_ANT" in _SUB_OPCODE_FOR_NAME:
        return dve_ops.EXP_P4_ANT, dve_ops.EXP_SQ8_ANT

    u = Src0
    p4 = ((((u * C0) + C1) * u + C2) * u + One) * u + One  # c0=1/24 c1=1/6 c2=1/2
    spec_p4 = Spec(
        body=p4,
        reference=lambda in0, in1, s0, s1, imm2: (
            (((in0 * s0 + s1) * in0 + imm2) * in0 + 1.0) * in0 + 1.0
        ),
    )
    spec_sq8 = Spec(
        body=sq(sq(sq(Src0))),
        reference=lambda in0, in1, s0, s1, imm2: in0 ** 8,
    )

    def _mk(name, spec):
        opcode = max(_SUB_OPCODE_FOR_NAME.values()) + 1
        _SUB_OPCODE_FOR_NAME[name] = opcode
        shas = {}
        for ver in ("v3", "v4"):
            s = DveOpSpec(
                name=name, opcode=opcode, uops=lower(spec, ver=ver), rd1_en=False
            )
            shas[ver] = s.sha(ver)
        op = DveOp(name, spec, subdim=False, uops_sha=shas)
        OPS.append(op)
        CUSTOM_DVE_SPECS[name] = spec
        setattr(dve_ops, name, op)
        return op

    p4_op = _mk("EXP_P4_ANT", spec_p4)
    sq8_op = _mk("EXP_SQ8_ANT", spec_sq8)
    return p4_op, sq8_op


# ---------------------------------------------------------------- program build
_PROG_CACHE = {}


def build_program(use_dve_exp=True):
    key = ("prog", use_dve_exp)
    if key in _PROG_CACHE:
        return _PROG_CACHE[key]
    p4_op, sq8_op = _register_exp_ops()

    nc = bacc.Bacc("TRN2", target_bir_lowering=False, debug=False, num_devices=N_CORES)

    xT = nc.dram_tensor("xT", [D, N], F16, kind="ExternalInput").ap()
    wqkv = nc.dram_tensor("wqkv", [D, 768], F16, kind="ExternalInput").ap()
    bqkv = nc.dram_tensor("bqkv", [6, 128], F32, kind="ExternalInput").ap()
    wout = nc.dram_tensor("wout", [D, 192], F16, kind="ExternalInput").ap()
    bout = nc.dram_tensor("bout", [2, 128], F32, kind="ExternalInput").ap()
    y = nc.dram_tensor("y", [HPC * DH, N], F32, kind="ExternalOutput").ap()

    with tile.TileContext(nc, trace_sim=False) as tc:
        with (
            tc.tile_pool(name="consts", bufs=1) as consts,
            tc.tile_pool(name="persist", bufs=1) as persist,
            tc.tile_pool(name="otp", bufs=2) as otp,
            tc.tile_pool(name="spsum", bufs=2, space="PSUM") as spsum,
            tc.tile_pool(name="opsum", bufs=1, space="PSUM") as opsum,
            tc.tile_pool(name="ppsum", bufs=1, space="PSUM") as ppsum,
            tc.tile_pool(name="dram", bufs=1, space="DRAM") as dram,
        ):
            # ---------------- constants + resident fp16 x
            ident_f = consts.tile([128, 128], F32)
            make_identity(nc, ident_f[:])
            ident16 = consts.tile([128, 128], F16)
            nc.scalar.copy(ident16[:], ident_f[:])

            x16 = persist.tile([128, 6 * N], F16)  # 6 d-chunks of [128, 4096]
            for fc in range(6):
                for nq in range(8):
                    nc.sync.dma_start(
                        out=x16[:, fc * N + nq * IC : fc * N + (nq + 1) * IC],
                        in_=xT[fc * 128 : (fc + 1) * 128,
                               nq * IC : (nq + 1) * IC],
                    )
            wq_sb = consts.tile([128, 6 * 768], F16)  # 6 f-chunks of [128,768]
            for fc in range(6):
                nc.sync.dma_start(
                    out=wq_sb[:, fc * 768 : (fc + 1) * 768],
                    in_=wqkv[fc * 128 : (fc + 1) * 128, :],
                )
            bq_sb = consts.tile([128, 6], F32)  # per-m-chunk bias columns
            for m in range(6):
                nc.sync.dma_start(
                    out=bq_sb[:, m : m + 1],
                    in_=bqkv[m : m + 1, :].rearrange("a p -> p a"),
                )

            # ---------------- persistent activations (all fp16)
            qT01 = persist.tile([128, N], F16)  # rows 0:64 head0 qT, 64:128 head1
            qT2 = persist.tile([128, N], F16)   # head2 qT duplicated in both halves
            kT01 = persist.tile([128, N], F16)  # scaled kT, heads 0/1
            kT2 = persist.tile([128, N], F16)   # scaled kT head2, duplicated
            v_sb = persist.tile([128, 32 * VW], F16)
            v_view = v_sb[:].rearrange("p (j c) -> p j c", c=VW)
            for h in range(HPC):
                nc.vector.memset(v_view[:, :, h * 128 + 64 : h * 128 + 128], 1.0)

            # AllGather eighths: in [192, 512] -> out [768, 512] (rank-major rows)
            oT_q = [dram.tile([HPC * DH, IC], F16, name=f"oT_q{i}") for i in range(8)]
            ag_q = [dram.tile([D, IC], F16, name=f"ag_q{i}") for i in range(8)]

            # output projection weights (phase 3 is emitted inline per i-chunk)
            wo_sb = consts.tile([128, 6 * 192], F16)  # w_out col-slice, 6 d-chunks
            for dc in range(6):
                nc.sync.dma_start(
                    out=wo_sb[:, dc * 192 : (dc + 1) * 192],
                    in_=wout[dc * 128 : (dc + 1) * 128, :],
                )
            bo_sb = consts.tile([128, 2], F32)
            for m in range(2):
                nc.sync.dma_start(
                    out=bo_sb[:, m : m + 1],
                    in_=bout[m : m + 1, :].rearrange("a p -> p a"),
                )

            with tc.tile_pool(name="work", bufs=2) as work, \
                 tc.tile_pool(name="ptp", bufs=3) as ptp, \
                 tc.tile_pool(name="ph3", bufs=1) as ph3:

                def proj_chunk(m, tsl, out_psum):
                    """fp16 matmul of m-chunk over token slice tsl into psum."""
                    for fc in range(6):
                        nc.tensor.matmul(
                            out_psum,
                            wq_sb[:, fc * 768 + m * 128 : fc * 768 + (m + 1) * 128],
                            x16[:, fc * N + tsl.start : fc * N + tsl.stop],
                            start=(fc == 0),
                            stop=(fc == 5),
                        )

                # ---------------- pass 1: k + v projection (q deferred)
                for tau in range(8):
                    ts = slice(tau * IC, (tau + 1) * IC)
                    for m, dst in ((2, kT01), (3, kT2)):
                        sp = spsum.tile([128, 1024], F32, tag="sp", bufs=2)
                        proj_chunk(m, ts, sp[:, 0:IC])
                        nc.scalar.activation(
                            dst[:, ts], sp[:, 0:IC], AF.Identity,
                            bias=bq_sb[:, m : m + 1],
                        )
                    vt01 = work.tile([128, IC], F16, tag="vt01")
                    vt2 = work.tile([64, IC], F16, tag="vt2")
                    sp = spsum.tile([128, 1024], F32, tag="sp", bufs=2)
                    proj_chunk(4, ts, sp[:, 0:IC])
                    nc.scalar.activation(
                        vt01[:], sp[:, 0:IC], AF.Identity, bias=bq_sb[:, 4:5]
                    )
                    sp = spsum.tile([128, 1024], F32, tag="sp", bufs=2)
                    proj_chunk(5, ts, sp[:, 0:IC])
                    nc.scalar.activation(
                        vt2[:], sp[0:64, 0:IC], AF.Identity, bias=bq_sb[0:64, 5:6]
                    )
                    # transpose v into token-major [v|ones] layout, 4 t-blocks
                    # per psum tile: [v01(t0..t3) 512 | v2(t0..t3) 256]
                    pv = spsum.tile([128, 2048], F16, tag="sp", bufs=2)
                    for s in range(4):
                        nc.tensor.transpose(
                            pv[:, s * 128 : (s + 1) * 128],
                            vt01[:, s * 128 : (s + 1) * 128],
                            ident16[:],
                        )
                        nc.tensor.transpose(
                            pv[:, 512 + s * 64 : 512 + (s + 1) * 64],
                            vt2[0:64, s * 128 : (s + 1) * 128],
                            ident16[0:64, 0:64],
                        )
                    jc0 = 4 * tau
                    pv01 = pv[:, 0:512].rearrange("p (t c) -> p t c", c=128)
                    pv2 = pv[:, 512:768].rearrange("p (t c) -> p t c", c=64)
                    nc.vector.tensor_copy(
                        out=v_view[:, jc0 : jc0 + 4, 0:64], in_=pv01[:, :, 0:64]
                    )
                    nc.vector.tensor_copy(
                        out=v_view[:, jc0 : jc0 + 4, 128:192],
                        in_=pv01[:, :, 64:128],
                    )
                    nc.vector.tensor_copy(
                        out=v_view[:, jc0 : jc0 + 4, 256:320], in_=pv2[:, :, :]
                    )

                # ---------------- pass 2: q projection + attention per i-chunk
                exp_batch_idx = [0]

                def exp_batch(sp):
                    """exp(8u) on a [128,1024] psum batch -> fp16 SBUF tile."""
                    i = exp_batch_idx[0]
                    exp_batch_idx[0] += 1
                    pt = ptp.tile([128, 1024], F16, tag="pt", bufs=4)
                    if use_dve_exp and ((i * EXP_DVE) % EXP_GROUP) < EXP_DVE:
                        tmp = ptp.tile([128, 1024], F16, tag="exptmp", bufs=2)
                        nc.vector._custom_dve(
                            p4_op, out=tmp[:], in0=sp[:],
                            s0=1.0 / 24, s1=1.0 / 6, imm2=0.5,
                        )
                        nc.vector._custom_dve(sq8_op, out=pt[:], in0=tmp[:])
                    else:
                        nc.scalar.activation(pt[:], sp[:], AF.Exp, scale=8.0)
                    return pt

                def q_proj(ic):
                    isl = slice(ic * IC, (ic + 1) * IC)
                    for m, dst in ((0, qT01), (1, qT2)):
                        pp = ppsum.tile([128, IC], F32, tag="pp", bufs=1,
                                        name=f"pp{m}_{ic}")
                        proj_chunk(m, isl, pp[:])
                        nc.scalar.activation(
                            dst[:, isl], pp[:], AF.Identity,
                            bias=bq_sb[:, m : m + 1],
                        )

                def norm_store(po_h, ic, h):
                    """o_h = num / l via 1/l = exp(-ln l) on ACT (one table set
                    with Exp/Identity), then fp16 multiply on DVE."""
                    lt = otp.tile([64, IC], F32, tag="lt", bufs=2)
                    nc.scalar.activation(lt[:], po_h[64:128, :], AF.Ln)
                    rr = otp.tile([64, IC], F32, tag="rr", bufs=2)
                    nc.scalar.activation(rr[:], lt[:], AF.Exp, scale=-1.0)
                    ot = otp.tile([64, IC], F16, tag="ot", bufs=3)
                    nc.vector.tensor_tensor(
                        out=ot[:], in0=po_h[0:64, :], in1=rr[:],
                        op=mybir.AluOpType.mult,
                    )
                    nc.sync.dma_start(
                        out=oT_q[ic][64 * h : 64 * h + 64, :], in_=ot[:]
                    )

                def phase3_qtr(qtr):
                    agr = ag_q[qtr][:]
                    ogs = []
                    for dc in range(6):
                        og = ph3.tile([128, IC], F16, tag="og", bufs=8,
                                      name=f"og{dc}_{qtr}")
                        nc.sync.dma_start(
                            out=og[:], in_=agr[dc * 128 : (dc + 1) * 128, :]
                        )
                        ogs.append(og)
                    for ec, (elo, ew) in enumerate(((0, 128), (128, 64))):
                        py = spsum.tile([128, 1024], F32, tag="sp", bufs=2,
                                        name=f"py{ec}_{qtr}")
                        for dc in range(6):
                            nc.tensor.matmul(
                                py[0:ew, 0:IC],
                                wo_sb[:, dc * 192 + elo : dc * 192 + elo + ew],
                                ogs[dc][:],
                                start=(dc == 0), stop=(dc == 5),
                            )
                        ysb = ph3.tile([128, IC], F32, tag="ysb", bufs=3,
                                       name=f"ysb{ec}_{qtr}")
                        nc.vector.tensor_scalar(
                            out=ysb[0:ew, :], in0=py[0:ew, 0:IC],
                            scalar1=bo_sb[0:ew, ec : ec + 1], scalar2=None,
                            op0=mybir.AluOpType.add,
                        )
                        nc.sync.dma_start(
                            out=y[elo : elo + ew, qtr * IC : (qtr + 1) * IC],
                            in_=ysb[0:ew, :],
                        )

                q_proj(0)
                for ic in range(NIC):
                    isl = slice(ic * IC, (ic + 1) * IC)
                    # attention
                    po = [
                        opsum.tile([128, IC], F32, tag=f"po{h}", bufs=1,
                                   name=f"po{h}_{ic}")
                        for h in range(HPC)
                    ]
                    # heads 0,1: one 128-key chunk per batch, row-tiled pair
                    for jc in range(32):
                        sp = spsum.tile([128, 1024], F32, tag="sp", bufs=2)
                        nc.tensor.matmul(
                            sp[:, 0:IC],
                            kT01[0:64, jc * 128 : (jc + 1) * 128],
                            qT01[0:64, isl],
                            start=True, stop=True, tile_position=(0, 0),
                        )
                        nc.tensor.matmul(
                            sp[:, IC:1024],
                            kT01[64:128, jc * 128 : (jc + 1) * 128],
                            qT01[64:128, isl],
                            start=True, stop=True, tile_position=(64, 0),
                        )
                        pt = exp_batch(sp)
                        nc.tensor.matmul(
                            po[0][:],
                            v_sb[:, jc * VW : jc * VW + 128],
                            pt[:, 0:IC],
                            start=(jc == 0), stop=(jc == 31),
                        )
                        nc.tensor.matmul(
                            po[1][:],
                            v_sb[:, jc * VW + 128 : jc * VW + 256],
                            pt[:, IC:1024],
                            start=(jc == 0), stop=(jc == 31),
                        )
                    # heads 0/1 done: normalize them while head 2 runs, and
                    # project the next chunk's q to keep the PE queue fed
                    norm_store(po[0], ic, 0)
                    norm_store(po[1], ic, 1)
                    if ic + 1 < NIC:
                        q_proj(ic + 1)
                    # head 2: row-tiled pair over adjacent key chunks
                    for t in range(16):
                        sp = spsum.tile([128, 1024], F32, tag="sp", bufs=2)
                        nc.tensor.matmul(
                            sp[:, 0:IC],
                            kT2[0:64, (2 * t) * 128 : (2 * t + 1) * 128],
                            qT2[0:64, isl],
                            start=True, stop=True, tile_position=(0, 0),
                        )
                        nc.tensor.matmul(
                            sp[:, IC:1024],
                            kT2[64:128, (2 * t + 1) * 128 : (2 * t + 2) * 128],
                            qT2[64:128, isl],
                            start=True, stop=True, tile_position=(64, 0),
                        )
                        pt = exp_batch(sp)
                        for s in range(2):
                            jc = 2 * t + s
                            nc.tensor.matmul(
                                po[2][:],
                                v_sb[:, jc * VW + 256 : jc * VW + 384],
                                pt[:, s * IC : (s + 1) * IC],
                                start=(jc == 0), stop=(jc == 31),
                            )
                    norm_store(po[2], ic, 2)
                    nc.gpsimd.collective_compute(
                        "AllGather",
                        mybir.AluOpType.bypass,
                        replica_groups=[[0, 1, 2, 3], [4, 5, 6, 7]],
                        ins=[oT_q[ic][:]],
                        outs=[ag_q[ic][:]],
                    )
                    # inline output projection of the previous quarter: its
                    # AllGather completed during this chunk's attention
                    if ic >= 1:
                        phase3_qtr(ic - 1)
                phase3_qtr(NIC - 1)

    nc.compile()
    _PROG_CACHE[key] = nc
    return nc


# ---------------------------------------------------------------- host wrapper
def make_in_maps(x, w_qkv, b_qkv, w_out, b_out):
    """Build the 8 per-core input dicts from full inputs."""
    in_maps = []
    xTb = [np.ascontiguousarray(x[b].T.astype(np.float16)) for b in range(B)]
    kscale = np.float32(SCALE / 8.0)
    for c in range(N_CORES):
        b = c // 4
        hs = HPC * (c % 4)

        def sect(kind, h):  # q=0,k=1,v=2
            lo = kind * (H * DH) + h * DH
            return w_qkv[:, lo : lo + DH], b_qkv[lo : lo + DH]

        q0, bq0 = sect(0, hs); q1, bq1 = sect(0, hs + 1); q2, bq2 = sect(0, hs + 2)
        k0, bk0 = sect(1, hs); k1, bk1 = sect(1, hs + 1); k2, bk2 = sect(1, hs + 2)
        v0, bv0 = sect(2, hs); v1, bv1 = sect(2, hs + 1); v2, bv2 = sect(2, hs + 2)
        z = np.zeros_like(q2); bz = np.zeros_like(bq2)
        # m-chunks: [q0|q1], [q2|q2], [k0|k1]*s, [k2|k2]*s, [v0|v1], [v2|0]
        cols = np.concatenate(
            [q0, q1, q2, q2, k0 * kscale, k1 * kscale, k2 * kscale, k2 * kscale,
             v0, v1, v2, z], axis=1).astype(np.float16)
        bias = np.concatenate(
            [bq0, bq1, bq2, bq2, bk0 * kscale, bk1 * kscale, bk2 * kscale,
             bk2 * kscale, bv0, bv1, bv2, bz]).astype(np.float32)
        q = c % 4
        bo = np.zeros((2, 128), np.float32)
        bo[0, :] = b_out[192 * q : 192 * q + 128]
        bo[1, :64] = b_out[192 * q + 128 : 192 * q + 192]
        in_maps.append({
            "xT": xTb[b],
            "wqkv": np.ascontiguousarray(cols),
            "bqkv": np.ascontiguousarray(bias.reshape(6, 128)),
            "wout": np.ascontiguousarray(
                w_out[:, 192 * q : 192 * (q + 1)].astype(np.float16)),
            "bout": bo,
        })
    return in_maps


def assemble_output(results):
    out = np.empty((B, N, D), dtype=np.float32)
    for c in range(N_CORES):
        b = c // 4
        q = c % 4
        out[b, :, 192 * q : 192 * (q + 1)] = results[c]["y"].T
    return out


def kernel(x, w_qkv, b_qkv, w_out, b_out):
    from concourse.bass_utils import run_bass_kernel_spmd

    x = np.asarray(x, dtype=np.float32)
    nc = build_program()
    in_maps = make_in_maps(
        x, np.asarray(w_qkv, np.float32), np.asarray(b_qkv, np.float32),
        np.asarray(w_out, np.float32), np.asarray(b_out, np.float32))
    res = run_bass_kernel_spmd(nc, in_maps, core_ids=list(range(N_CORES)))
    return assemble_output(res.results)


# revision 25
# speedup vs baseline: 1.3113x; 1.0370x over previous
"""Multi-head self-attention (B=2, N=4096, D=768, H=12, dh=64) on 8 trn2 NeuronCores.

Sharding: core c handles batch b=c//4 and heads 3*(c%4)..3*(c%4)+2 (head-parallel
attention); an AllGather per 512-token i-chunk redistributes head-outputs so each
core projects its own 192 output columns for all tokens (column-parallel output
projection), assembled host-side.

Per-core pipeline (all matmuls fp16 except psum accumulate):
  x is shipped fp16 and kept resident in SBUF (48KB/partition), so projections
  read it directly with 1024-wide moving operands and no per-pass DMA.
  pass 1: project k (pre-scaled by softmax_scale/8 host-side) and v for all
    tokens; k kept d-on-partition fp16 in row-tiled head-pair layout, v
    PE-transposed into v_sb as [v_h | ones64] fp16 blocks per 128-key chunk —
    the ones columns make each PV matmul emit the softmax denominator
    replicated across psum partitions 64:128 for free.
  pass 2 (interleaved with attention): q projected per 512-token chunk.
  attention per (i-chunk, key-chunk): row-tiled QK pair (concurrent PE
    row-strips) -> psum u = s/8 -> exp(8u) split ~11:5 between ACT table exp
    and a 2-op DVE polynomial (taylor4(u), x^8), DVE batches interleaved so
    the PE never idles a full HAM window -> fp16 PV with [v|ones] stationary
    -> reciprocal[64,512] + multiply -> fp16 AllGather -> column-parallel
    output projection with DVE bias add.
"""
import sys

sys.path.insert(0, "/opt/trn_rl_repo")

import numpy as np

import concourse.bass as bass
import concourse.mybir as mybir
import concourse.tile as tile
import concourse.bacc as bacc
from concourse.masks import make_identity

N_CORES = 8
B, N, D, H, DH = 2, 4096, 768, 12, 64
HPC = 3            # heads per core
SCALE = D ** -0.5
F32 = mybir.dt.float32
F16 = mybir.dt.float16
AF = mybir.ActivationFunctionType
IC = 512           # query chunk size
NIC = N // IC      # 8 i-chunks
VW = 384           # v_sb cols per 128-key chunk: [v0|1*64|v1|1*64|v2|1*64]
EXP_GROUP = 16     # exp batches: (i*EXP_DVE) % 16 < EXP_DVE go to the DVE path
EXP_DVE = 4        # evenly interleaved: every 4th batch on DVE


# ---------------------------------------------------------------- custom DVE exp
def _register_exp_ops():
    """exp(8u) as two DVE ops: EXP_P4_ANT = taylor4(u); EXP_SQ8_ANT = x^8."""
    import concourse.dve_ops as dve_ops
    from concourse.dve_ops import DveOp, OPS, CUSTOM_DVE_SPECS, _SUB_OPCODE_FOR_NAME
    from concourse.dve_spec import Spec, Src0, C0, C1, C2, One, sq, lower
    from concourse.dve_uop import DveOpSpec

    if "EXP_P4_ANT" in _SUB_OPCODE_FOR_NAME:
        return dve_ops.EXP_P4_ANT, dve_ops.EXP_SQ8_ANT

    u = Src0
    p4 = ((((u * C0) + C1) * u + C2) * u + One) * u + One  # c0=1/24 c1=1/6 c2=1/2
    spec_p4 = Spec(
        body=p4,
        reference=lambda in0, in1, s0, s1, imm2: (
            (((in0 * s0 + s1) * in0 + imm2) * in0 + 1.0) * in0 + 1.0
        ),
    )
    spec_sq8 = Spec(
        body=sq(sq(sq(Src0))),
        reference=lambda in0, in1, s0, s1, imm2: in0 ** 8,
    )

    def _mk(name, spec):
        opcode = max(_SUB_OPCODE_FOR_NAME.values()) + 1
        _SUB_OPCODE_FOR_NAME[name] = opcode
        shas = {}
        for ver in ("v3", "v4"):
            s = DveOpSpec(
                name=name, opcode=opcode, uops=lower(spec, ver=ver), rd1_en=False
            )
            shas[ver] = s.sha(ver)
        op = DveOp(name, spec, subdim=False, uops_sha=shas)
        OPS.append(op)
        CUSTOM_DVE_SPECS[name] = spec
        setattr(dve_ops, name, op)
        return op

    p4_op = _mk("EXP_P4_ANT", spec_p4)
    sq8_op = _mk("EXP_SQ8_ANT", spec_sq8)
    return p4_op, sq8_op


# ---------------------------------------------------------------- program build
_PROG_CACHE = {}


def build_program(use_dve_exp=True):
    key = ("prog", use_dve_exp)
    if key in _PROG_CACHE:
        return _PROG_CACHE[key]
    p4_op, sq8_op = _register_exp_ops()

    nc = bacc.Bacc("TRN2", target_bir_lowering=False, debug=False, num_devices=N_CORES)

    xT = nc.dram_tensor("xT", [D, N], F16, kind="ExternalInput").ap()
    wqkv = nc.dram_tensor("wqkv", [D, 768], F16, kind="ExternalInput").ap()
    bqkv = nc.dram_tensor("bqkv", [6, 128], F32, kind="ExternalInput").ap()
    wout = nc.dram_tensor("wout", [D, 192], F16, kind="ExternalInput").ap()
    bout = nc.dram_tensor("bout", [2, 128], F32, kind="ExternalInput").ap()
    y = nc.dram_tensor("y", [HPC * DH, N], F32, kind="ExternalOutput").ap()

    with tile.TileContext(nc, trace_sim=False) as tc:
        with (
            tc.tile_pool(name="consts", bufs=1) as consts,
            tc.tile_pool(name="persist", bufs=1) as persist,
            tc.tile_pool(name="otp", bufs=2) as otp,
            tc.tile_pool(name="spsum", bufs=2, space="PSUM") as spsum,
            tc.tile_pool(name="opsum", bufs=1, space="PSUM") as opsum,
            tc.tile_pool(name="ppsum", bufs=1, space="PSUM") as ppsum,
            tc.tile_pool(name="dram", bufs=1, space="DRAM") as dram,
        ):
            # ---------------- constants + resident fp16 x
            ident_f = consts.tile([128, 128], F32)
            make_identity(nc, ident_f[:])
            ident16 = consts.tile([128, 128], F16)
            nc.scalar.copy(ident16[:], ident_f[:])

            x16 = persist.tile([128, 6 * N], F16)  # 6 d-chunks of [128, 4096]
            # [64, 1024] chunks: 2KB per partition line for DMA efficiency,
            # emitted token-major so early taus land first
            for nq in range(4):
                for fc in range(6):
                    for pb in range(2):
                        nc.sync.dma_start(
                            out=x16[64 * pb : 64 * (pb + 1),
                                    fc * N + nq * 1024 : fc * N + (nq + 1) * 1024],
                            in_=xT[fc * 128 + 64 * pb : fc * 128 + 64 * (pb + 1),
                                   nq * 1024 : (nq + 1) * 1024],
                        )
            wq_sb = consts.tile([128, 6 * 768], F16)  # 6 f-chunks of [128,768]
            for fc in range(6):
                nc.sync.dma_start(
                    out=wq_sb[:, fc * 768 : (fc + 1) * 768],
                    in_=wqkv[fc * 128 : (fc + 1) * 128, :],
                )
            bq_sb = consts.tile([128, 6], F32)  # per-m-chunk bias columns
            for m in range(6):
                nc.sync.dma_start(
                    out=bq_sb[:, m : m + 1],
                    in_=bqkv[m : m + 1, :].rearrange("a p -> p a"),
                )

            # ---------------- persistent activations (all fp16)
            qT01 = persist.tile([128, N], F16)  # rows 0:64 head0 qT, 64:128 head1
            qT2 = persist.tile([128, N], F16)   # head2 qT duplicated in both halves
            kT01 = persist.tile([128, N], F16)  # scaled kT, heads 0/1
            kT2 = persist.tile([128, N], F16)   # scaled kT head2, duplicated
            v_sb = persist.tile([128, 32 * VW], F16)
            v_view = v_sb[:].rearrange("p (j c) -> p j c", c=VW)
            for h in range(HPC):
                nc.vector.memset(v_view[:, :, h * 128 + 64 : h * 128 + 128], 1.0)

            # AllGather eighths: in [192, 512] -> out [768, 512] (rank-major rows)
            oT_q = [dram.tile([HPC * DH, IC], F16, name=f"oT_q{i}") for i in range(8)]
            ag_q = [dram.tile([D, IC], F16, name=f"ag_q{i}") for i in range(8)]

            # output projection weights (phase 3 is emitted inline per i-chunk)
            wo_sb = consts.tile([128, 6 * 192], F16)  # w_out col-slice, 6 d-chunks
            for dc in range(6):
                nc.sync.dma_start(
                    out=wo_sb[:, dc * 192 : (dc + 1) * 192],
                    in_=wout[dc * 128 : (dc + 1) * 128, :],
                )
            bo_sb = consts.tile([128, 2], F32)
            for m in range(2):
                nc.sync.dma_start(
                    out=bo_sb[:, m : m + 1],
                    in_=bout[m : m + 1, :].rearrange("a p -> p a"),
                )

            with tc.tile_pool(name="work", bufs=2) as work, \
                 tc.tile_pool(name="ptp", bufs=3) as ptp, \
                 tc.tile_pool(name="ph3", bufs=1) as ph3:

                def proj_chunk(m, tsl, out_psum):
                    """fp16 matmul of m-chunk over token slice tsl into psum."""
                    for fc in range(6):
                        nc.tensor.matmul(
                            out_psum,
                            wq_sb[:, fc * 768 + m * 128 : fc * 768 + (m + 1) * 128],
                            x16[:, fc * N + tsl.start : fc * N + tsl.stop],
                            start=(fc == 0),
                            stop=(fc == 5),
                        )

                # ---------------- pass 1: k + v projection (q deferred)
                for tau in range(8):
                    ts = slice(tau * IC, (tau + 1) * IC)
                    for m, dst in ((2, kT01), (3, kT2)):
                        sp = spsum.tile([128, 1024], F32, tag="sp", bufs=2)
                        proj_chunk(m, ts, sp[:, 0:IC])
                        nc.scalar.activation(
                            dst[:, ts], sp[:, 0:IC], AF.Identity,
                            bias=bq_sb[:, m : m + 1],
                        )
                    vt01 = work.tile([128, IC], F16, tag="vt01")
                    vt2 = work.tile([64, IC], F16, tag="vt2")
                    sp = spsum.tile([128, 1024], F32, tag="sp", bufs=2)
                    proj_chunk(4, ts, sp[:, 0:IC])
                    nc.scalar.activation(
                        vt01[:], sp[:, 0:IC], AF.Identity, bias=bq_sb[:, 4:5]
                    )
                    sp = spsum.tile([128, 1024], F32, tag="sp", bufs=2)
                    proj_chunk(5, ts, sp[:, 0:IC])
                    nc.scalar.activation(
                        vt2[:], sp[0:64, 0:IC], AF.Identity, bias=bq_sb[0:64, 5:6]
                    )
                    # transpose v into token-major [v|ones] layout, 4 t-blocks
                    # per psum tile: [v01(t0..t3) 512 | v2(t0..t3) 256]
                    pv = spsum.tile([128, 2048], F16, tag="sp", bufs=2)
                    for s in range(4):
                        nc.tensor.transpose(
                            pv[:, s * 128 : (s + 1) * 128],
                            vt01[:, s * 128 : (s + 1) * 128],
                            ident16[:],
                        )
                        nc.tensor.transpose(
                            pv[:, 512 + s * 64 : 512 + (s + 1) * 64],
                            vt2[0:64, s * 128 : (s + 1) * 128],
                            ident16[0:64, 0:64],
                        )
                    jc0 = 4 * tau
                    pv01 = pv[:, 0:512].rearrange("p (t c) -> p t c", c=128)
                    pv2 = pv[:, 512:768].rearrange("p (t c) -> p t c", c=64)
                    nc.vector.tensor_copy(
                        out=v_view[:, jc0 : jc0 + 4, 0:64], in_=pv01[:, :, 0:64]
                    )
                    nc.vector.tensor_copy(
                        out=v_view[:, jc0 : jc0 + 4, 128:192],
                        in_=pv01[:, :, 64:128],
                    )
                    nc.vector.tensor_copy(
                        out=v_view[:, jc0 : jc0 + 4, 256:320], in_=pv2[:, :, :]
                    )

                # ---------------- pass 2: q projection + attention per i-chunk
                exp_batch_idx = [0]

                def exp_batch(sp):
                    """exp(8u) on a [128,1024] psum batch -> fp16 SBUF tile."""
                    i = exp_batch_idx[0]
                    exp_batch_idx[0] += 1
                    pt = ptp.tile([128, 1024], F16, tag="pt", bufs=4)
                    if use_dve_exp and ((i * EXP_DVE) % EXP_GROUP) < EXP_DVE:
                        tmp = ptp.tile([128, 1024], F16, tag="exptmp", bufs=2)
                        nc.vector._custom_dve(
                            p4_op, out=tmp[:], in0=sp[:],
                            s0=1.0 / 24, s1=1.0 / 6, imm2=0.5,
                        )
                        nc.vector._custom_dve(sq8_op, out=pt[:], in0=tmp[:])
                    else:
                        nc.scalar.activation(pt[:], sp[:], AF.Exp, scale=8.0)
                    return pt

                def q_proj(ic):
                    isl = slice(ic * IC, (ic + 1) * IC)
                    for m, dst in ((0, qT01), (1, qT2)):
                        pp = ppsum.tile([128, IC], F32, tag="pp", bufs=1,
                                        name=f"pp{m}_{ic}")
                        proj_chunk(m, isl, pp[:])
                        nc.scalar.activation(
                            dst[:, isl], pp[:], AF.Identity,
                            bias=bq_sb[:, m : m + 1],
                        )

                def norm_store(po_h, ic, h):
                    """o_h = num / l. Builtin reciprocal handles the cross-base
                    read (custom DVE ops only work at partition base 0)."""
                    rr = otp.tile([128, IC], F32, tag="rr", bufs=2)
                    nc.vector.reciprocal(rr[0:64, :], po_h[64:128, :])
                    ot = otp.tile([64, IC], F16, tag="ot", bufs=3)
                    nc.vector.tensor_tensor(
                        out=ot[:], in0=po_h[0:64, :], in1=rr[0:64, :],
                        op=mybir.AluOpType.mult,
                    )
                    nc.sync.dma_start(
                        out=oT_q[ic][64 * h : 64 * h + 64, :], in_=ot[:]
                    )

                def phase3_qtr(qtr):
                    agr = ag_q[qtr][:]
                    ogs = []
                    for dc in range(6):
                        og = ph3.tile([128, IC], F16, tag="og", bufs=8,
                                      name=f"og{dc}_{qtr}")
                        nc.sync.dma_start(
                            out=og[:], in_=agr[dc * 128 : (dc + 1) * 128, :]
                        )
                        ogs.append(og)
                    for ec, (elo, ew) in enumerate(((0, 128), (128, 64))):
                        py = spsum.tile([128, 1024], F32, tag="sp", bufs=2,
                                        name=f"py{ec}_{qtr}")
                        for dc in range(6):
                            nc.tensor.matmul(
                                py[0:ew, 0:IC],
                                wo_sb[:, dc * 192 + elo : dc * 192 + elo + ew],
                                ogs[dc][:],
                                start=(dc == 0), stop=(dc == 5),
                            )
                        ysb = ph3.tile([128, IC], F32, tag="ysb", bufs=3,
                                       name=f"ysb{ec}_{qtr}")
                        nc.scalar.activation(
                            ysb[0:ew, :], py[0:ew, 0:IC], AF.Identity,
                            bias=bo_sb[0:ew, ec : ec + 1],
                        )
                        nc.sync.dma_start(
                            out=y[elo : elo + ew, qtr * IC : (qtr + 1) * IC],
                            in_=ysb[0:ew, :],
                        )

                q_proj(0)
                for ic in range(NIC):
                    isl = slice(ic * IC, (ic + 1) * IC)
                    # attention
                    po = [
                        opsum.tile([128, IC], F32, tag=f"po{h}", bufs=1,
                                   name=f"po{h}_{ic}")
                        for h in range(HPC)
                    ]
                    # heads 0,1: one 128-key chunk per batch, row-tiled pair.
                    # Software-pipelined: QK(jc)+exp(jc) are emitted BEFORE
                    # PV(jc-1) so the in-order PE queue never blocks a ready
                    # QK behind a PV that waits on exp.
                    def qk01(jc):
                        sp = spsum.tile([128, 1024], F32, tag="sp", bufs=2,
                                        name=f"sp01_{ic}_{jc}")
                        nc.tensor.matmul(
                            sp[:, 0:IC],
                            kT01[0:64, jc * 128 : (jc + 1) * 128],
                            qT01[0:64, isl],
                            start=True, stop=True, tile_position=(0, 0),
                        )
                        nc.tensor.matmul(
                            sp[:, IC:1024],
                            kT01[64:128, jc * 128 : (jc + 1) * 128],
                            qT01[64:128, isl],
                            start=True, stop=True, tile_position=(64, 0),
                        )
                        return exp_batch(sp)

                    def pv01(jc, pt):
                        nc.tensor.matmul(
                            po[0][:],
                            v_sb[:, jc * VW : jc * VW + 128],
                            pt[:, 0:IC],
                            start=(jc == 0), stop=(jc == 31),
                        )
                        nc.tensor.matmul(
                            po[1][:],
                            v_sb[:, jc * VW + 128 : jc * VW + 256],
                            pt[:, IC:1024],
                            start=(jc == 0), stop=(jc == 31),
                        )

                    prev = None
                    for jc in range(32):
                        pt = qk01(jc)
                        if prev is not None:
                            pv01(prev[0], prev[1])
                        prev = (jc, pt)
                    pv01(prev[0], prev[1])
                    # heads 0/1 done: normalize them while head 2 runs, and
                    # project the next chunk's q to keep the PE queue fed
                    norm_store(po[0], ic, 0)
                    norm_store(po[1], ic, 1)
                    if ic + 1 < NIC:
                        q_proj(ic + 1)

                    # head 2: row-tiled pair over adjacent key chunks
                    def qk2(t):
                        sp = spsum.tile([128, 1024], F32, tag="sp", bufs=2,
                                        name=f"sp2_{ic}_{t}")
                        nc.tensor.matmul(
                            sp[:, 0:IC],
                            kT2[0:64, (2 * t) * 128 : (2 * t + 1) * 128],
                            qT2[0:64, isl],
                            start=True, stop=True, tile_position=(0, 0),
                        )
                        nc.tensor.matmul(
                            sp[:, IC:1024],
                            kT2[64:128, (2 * t + 1) * 128 : (2 * t + 2) * 128],
                            qT2[64:128, isl],
                            start=True, stop=True, tile_position=(64, 0),
                        )
                        return exp_batch(sp)

                    def pv2(t, pt):
                        for s in range(2):
                            jc = 2 * t + s
                            nc.tensor.matmul(
                                po[2][:],
                                v_sb[:, jc * VW + 256 : jc * VW + 384],
                                pt[:, s * IC : (s + 1) * IC],
                                start=(jc == 0), stop=(jc == 31),
                            )

                    prev = None
                    for t in range(16):
                        pt = qk2(t)
                        if prev is not None:
                            pv2(prev[0], prev[1])
                        prev = (t, pt)
                    pv2(prev[0], prev[1])
                    norm_store(po[2], ic, 2)
                    nc.gpsimd.collective_compute(
                        "AllGather",
                        mybir.AluOpType.bypass,
                        replica_groups=[[0, 1, 2, 3], [4, 5, 6, 7]],
                        ins=[oT_q[ic][:]],
                        outs=[ag_q[ic][:]],
                    )
                    # inline output projection of the previous quarter: its
                    # AllGather completed during this chunk's attention
                    if ic >= 1:
                        phase3_qtr(ic - 1)
                phase3_qtr(NIC - 1)

    nc.compile()
    _PROG_CACHE[key] = nc
    return nc


# ---------------------------------------------------------------- host wrapper
def make_in_maps(x, w_qkv, b_qkv, w_out, b_out):
    """Build the 8 per-core input dicts from full inputs."""
    in_maps = []
    xTb = [np.ascontiguousarray(x[b].T.astype(np.float16)) for b in range(B)]
    kscale = np.float32(SCALE / 8.0)
    for c in range(N_CORES):
        b = c // 4
        hs = HPC * (c % 4)

        def sect(kind, h):  # q=0,k=1,v=2
            lo = kind * (H * DH) + h * DH
            return w_qkv[:, lo : lo + DH], b_qkv[lo : lo + DH]

        q0, bq0 = sect(0, hs); q1, bq1 = sect(0, hs + 1); q2, bq2 = sect(0, hs + 2)
        k0, bk0 = sect(1, hs); k1, bk1 = sect(1, hs + 1); k2, bk2 = sect(1, hs + 2)
        v0, bv0 = sect(2, hs); v1, bv1 = sect(2, hs + 1); v2, bv2 = sect(2, hs + 2)
        z = np.zeros_like(q2); bz = np.zeros_like(bq2)
        # m-chunks: [q0|q1], [q2|q2], [k0|k1]*s, [k2|k2]*s, [v0|v1], [v2|0]
        cols = np.concatenate(
            [q0, q1, q2, q2, k0 * kscale, k1 * kscale, k2 * kscale, k2 * kscale,
             v0, v1, v2, z], axis=1).astype(np.float16)
        bias = np.concatenate(
            [bq0, bq1, bq2, bq2, bk0 * kscale, bk1 * kscale, bk2 * kscale,
             bk2 * kscale, bv0, bv1, bv2, bz]).astype(np.float32)
        q = c % 4
        bo = np.zeros((2, 128), np.float32)
        bo[0, :] = b_out[192 * q : 192 * q + 128]
        bo[1, :64] = b_out[192 * q + 128 : 192 * q + 192]
        in_maps.append({
            "xT": xTb[b],
            "wqkv": np.ascontiguousarray(cols),
            "bqkv": np.ascontiguousarray(bias.reshape(6, 128)),
            "wout": np.ascontiguousarray(
                w_out[:, 192 * q : 192 * (q + 1)].astype(np.float16)),
            "bout": bo,
        })
    return in_maps


def assemble_output(results):
    out = np.empty((B, N, D), dtype=np.float32)
    for c in range(N_CORES):
        b = c // 4
        q = c % 4
        out[b, :, 192 * q : 192 * (q + 1)] = results[c]["y"].T
    return out


def kernel(x, w_qkv, b_qkv, w_out, b_out):
    from concourse.bass_utils import run_bass_kernel_spmd

    x = np.asarray(x, dtype=np.float32)
    nc = build_program()
    in_maps = make_in_maps(
        x, np.asarray(w_qkv, np.float32), np.asarray(b_qkv, np.float32),
        np.asarray(w_out, np.float32), np.asarray(b_out, np.float32))
    res = run_bass_kernel_spmd(nc, in_maps, core_ids=list(range(N_CORES)))
    return assemble_output(res.results)


# revision 28
# speedup vs baseline: 1.3204x; 1.0070x over previous
"""Multi-head self-attention (B=2, N=4096, D=768, H=12, dh=64) on 8 trn2 NeuronCores.

Sharding: core c handles batch b=c//4 and heads 3*(c%4)..3*(c%4)+2 (head-parallel
attention); an AllGather per 512-token i-chunk redistributes head-outputs so each
core projects its own 192 output columns for all tokens (column-parallel output
projection), assembled host-side.

Per-core pipeline (all matmuls fp16 except psum accumulate):
  x is shipped fp16 and kept resident in SBUF (48KB/partition), so projections
  read it directly with 1024-wide moving operands and no per-pass DMA.
  pass 1: project k (pre-scaled by softmax_scale/8 host-side) and v for all
    tokens; k kept d-on-partition fp16 in row-tiled head-pair layout, v
    PE-transposed into v_sb as [v_h | ones64] fp16 blocks per 128-key chunk —
    the ones columns make each PV matmul emit the softmax denominator
    replicated across psum partitions 64:128 for free.
  pass 2 (interleaved with attention): q projected per 512-token chunk.
  attention per (i-chunk, key-chunk): row-tiled QK pair (concurrent PE
    row-strips) -> psum u = s/8 -> exp(8u) split ~11:5 between ACT table exp
    and a 2-op DVE polynomial (taylor4(u), x^8), DVE batches interleaved so
    the PE never idles a full HAM window -> fp16 PV with [v|ones] stationary
    -> reciprocal[64,512] + multiply -> fp16 AllGather -> column-parallel
    output projection with DVE bias add.
"""
import sys

sys.path.insert(0, "/opt/trn_rl_repo")

import numpy as np

import concourse.bass as bass
import concourse.mybir as mybir
import concourse.tile as tile
import concourse.bacc as bacc
from concourse.masks import make_identity

N_CORES = 8
B, N, D, H, DH = 2, 4096, 768, 12, 64
HPC = 3            # heads per core
SCALE = D ** -0.5
F32 = mybir.dt.float32
F16 = mybir.dt.float16
AF = mybir.ActivationFunctionType
IC = 512           # query chunk size
NIC = N // IC      # 8 i-chunks
VW = 384           # v_sb cols per 128-key chunk: [v0|1*64|v1|1*64|v2|1*64]
EXP_GROUP = 16     # exp batches: (i*EXP_DVE) % 16 < EXP_DVE go to the DVE path
EXP_DVE = 4        # evenly interleaved: every 4th batch on DVE


# ---------------------------------------------------------------- custom DVE exp
def _register_exp_ops():
    """exp(8u) as two DVE ops: EXP_P4_ANT = taylor4(u); EXP_SQ8_ANT = x^8."""
    import concourse.dve_ops as dve_ops
    from concourse.dve_ops import DveOp, OPS, CUSTOM_DVE_SPECS, _SUB_OPCODE_FOR_NAME
    from concourse.dve_spec import Spec, Src0, C0, C1, C2, One, sq, lower
    from concourse.dve_uop import DveOpSpec

    if "EXP_P4_ANT" in _SUB_OPCODE_FOR_NAME:
        return dve_ops.EXP_P4_ANT, dve_ops.EXP_SQ8_ANT

    u = Src0
    p4 = ((((u * C0) + C1) * u + C2) * u + One) * u + One  # c0=1/24 c1=1/6 c2=1/2
    spec_p4 = Spec(
        body=p4,
        reference=lambda in0, in1, s0, s1, imm2: (
            (((in0 * s0 + s1) * in0 + imm2) * in0 + 1.0) * in0 + 1.0
        ),
    )
    spec_sq8 = Spec(
        body=sq(sq(sq(Src0))),
        reference=lambda in0, in1, s0, s1, imm2: in0 ** 8,
    )

    def _mk(name, spec):
        opcode = max(_SUB_OPCODE_FOR_NAME.values()) + 1
        _SUB_OPCODE_FOR_NAME[name] = opcode
        shas = {}
        for ver in ("v3", "v4"):
            s = DveOpSpec(
                name=name, opcode=opcode, uops=lower(spec, ver=ver), rd1_en=False
            )
            shas[ver] = s.sha(ver)
        op = DveOp(name, spec, subdim=False, uops_sha=shas)
        OPS.append(op)
        CUSTOM_DVE_SPECS[name] = spec
        setattr(dve_ops, name, op)
        return op

    p4_op = _mk("EXP_P4_ANT", spec_p4)
    sq8_op = _mk("EXP_SQ8_ANT", spec_sq8)
    return p4_op, sq8_op


# ---------------------------------------------------------------- program build
_PROG_CACHE = {}


def build_program(use_dve_exp=True):
    key = ("prog", use_dve_exp)
    if key in _PROG_CACHE:
        return _PROG_CACHE[key]
    p4_op, sq8_op = _register_exp_ops()

    nc = bacc.Bacc("TRN2", target_bir_lowering=False, debug=False, num_devices=N_CORES)

    xT = nc.dram_tensor("xT", [D, N], F16, kind="ExternalInput").ap()
    wqkv = nc.dram_tensor("wqkv", [D, 768], F16, kind="ExternalInput").ap()
    bqkv = nc.dram_tensor("bqkv", [6, 128], F32, kind="ExternalInput").ap()
    wout = nc.dram_tensor("wout", [D, 192], F16, kind="ExternalInput").ap()
    bout = nc.dram_tensor("bout", [2, 128], F32, kind="ExternalInput").ap()
    y = nc.dram_tensor("y", [HPC * DH, N], F32, kind="ExternalOutput").ap()

    with tile.TileContext(nc, trace_sim=False) as tc:
        with (
            tc.tile_pool(name="consts", bufs=1) as consts,
            tc.tile_pool(name="persist", bufs=1) as persist,
            tc.tile_pool(name="otp", bufs=2) as otp,
            tc.tile_pool(name="spsum", bufs=2, space="PSUM") as spsum,
            tc.tile_pool(name="opsum", bufs=1, space="PSUM") as opsum,
            tc.tile_pool(name="ppsum", bufs=1, space="PSUM") as ppsum,
            tc.tile_pool(name="dram", bufs=1, space="DRAM") as dram,
        ):
            # ---------------- constants + resident fp16 x
            ident_f = consts.tile([128, 128], F32)
            make_identity(nc, ident_f[:])
            ident16 = consts.tile([128, 128], F16)
            nc.scalar.copy(ident16[:], ident_f[:])

            x16 = persist.tile([128, 6 * N], F16)  # 6 d-chunks of [128, 4096]
            # [64, 1024] chunks: 2KB per partition line for DMA efficiency,
            # emitted token-major so early taus land first
            for nq in range(4):
                for fc in range(6):
                    for pb in range(2):
                        nc.sync.dma_start(
                            out=x16[64 * pb : 64 * (pb + 1),
                                    fc * N + nq * 1024 : fc * N + (nq + 1) * 1024],
                            in_=xT[fc * 128 + 64 * pb : fc * 128 + 64 * (pb + 1),
                                   nq * 1024 : (nq + 1) * 1024],
                        )
            wq_sb = consts.tile([128, 6 * 768], F16)  # 6 f-chunks of [128,768]
            for fc in range(6):
                nc.sync.dma_start(
                    out=wq_sb[:, fc * 768 : (fc + 1) * 768],
                    in_=wqkv[fc * 128 : (fc + 1) * 128, :],
                )
            bq_sb = consts.tile([128, 6], F32)  # per-m-chunk bias columns
            for m in range(6):
                nc.sync.dma_start(
                    out=bq_sb[:, m : m + 1],
                    in_=bqkv[m : m + 1, :].rearrange("a p -> p a"),
                )

            # ---------------- persistent activations (all fp16)
            # q tiles are per-i-chunk so projecting chunk ic+1 mid-attention
            # carries no tile-granularity WAR against chunk ic's reads
            qT01_t = [
                persist.tile([128, IC], F16, tag=f"q01_{i}", name=f"q01_{i}")
                for i in range(NIC)
            ]
            qT2_t = [
                persist.tile([128, IC], F16, tag=f"q2_{i}", name=f"q2_{i}")
                for i in range(NIC)
            ]
            kT01 = persist.tile([128, N], F16)  # scaled kT, heads 0/1
            kT2 = persist.tile([128, N], F16)   # scaled kT head2, duplicated
            v_sb = persist.tile([128, 32 * VW], F16)
            v_view = v_sb[:].rearrange("p (j c) -> p j c", c=VW)
            for h in range(HPC):
                nc.vector.memset(v_view[:, :, h * 128 + 64 : h * 128 + 128], 1.0)

            # AllGather eighths: in [192, 512] -> out [768, 512] (rank-major rows)
            oT_q = [dram.tile([HPC * DH, IC], F16, name=f"oT_q{i}") for i in range(8)]
            ag_q = [dram.tile([D, IC], F16, name=f"ag_q{i}") for i in range(8)]

            # output projection weights (phase 3 is emitted inline per i-chunk)
            wo_sb = consts.tile([128, 6 * 192], F16)  # w_out col-slice, 6 d-chunks
            for dc in range(6):
                nc.sync.dma_start(
                    out=wo_sb[:, dc * 192 : (dc + 1) * 192],
                    in_=wout[dc * 128 : (dc + 1) * 128, :],
                )
            bo_sb = consts.tile([128, 2], F32)
            for m in range(2):
                nc.sync.dma_start(
                    out=bo_sb[:, m : m + 1],
                    in_=bout[m : m + 1, :].rearrange("a p -> p a"),
                )

            with tc.tile_pool(name="work", bufs=2) as work, \
                 tc.tile_pool(name="ptp", bufs=3) as ptp, \
                 tc.tile_pool(name="ph3", bufs=1) as ph3:

                def proj_chunk(m, tsl, out_psum):
                    """fp16 matmul of m-chunk over token slice tsl into psum."""
                    for fc in range(6):
                        nc.tensor.matmul(
                            out_psum,
                            wq_sb[:, fc * 768 + m * 128 : fc * 768 + (m + 1) * 128],
                            x16[:, fc * N + tsl.start : fc * N + tsl.stop],
                            start=(fc == 0),
                            stop=(fc == 5),
                        )

                # ---------------- pass 1: k + v projection (q deferred)
                for tau in range(8):
                    ts = slice(tau * IC, (tau + 1) * IC)
                    for m, dst in ((2, kT01), (3, kT2)):
                        sp = spsum.tile([128, 1024], F32, tag="sp", bufs=2)
                        proj_chunk(m, ts, sp[:, 0:IC])
                        nc.scalar.activation(
                            dst[:, ts], sp[:, 0:IC], AF.Identity,
                            bias=bq_sb[:, m : m + 1],
                        )
                    vt01 = work.tile([128, IC], F16, tag="vt01")
                    vt2 = work.tile([64, IC], F16, tag="vt2")
                    sp = spsum.tile([128, 1024], F32, tag="sp", bufs=2)
                    proj_chunk(4, ts, sp[:, 0:IC])
                    nc.scalar.activation(
                        vt01[:], sp[:, 0:IC], AF.Identity, bias=bq_sb[:, 4:5]
                    )
                    sp = spsum.tile([128, 1024], F32, tag="sp", bufs=2)
                    proj_chunk(5, ts, sp[:, 0:IC])
                    nc.scalar.activation(
                        vt2[:], sp[0:64, 0:IC], AF.Identity, bias=bq_sb[0:64, 5:6]
                    )
                    # transpose v into token-major [v|ones] layout, 4 t-blocks
                    # per psum tile: [v01(t0..t3) 512 | v2(t0..t3) 256]
                    pv = spsum.tile([128, 2048], F16, tag="sp", bufs=2)
                    for s in range(4):
                        nc.tensor.transpose(
                            pv[:, s * 128 : (s + 1) * 128],
                            vt01[:, s * 128 : (s + 1) * 128],
                            ident16[:],
                        )
                        nc.tensor.transpose(
                            pv[:, 512 + s * 64 : 512 + (s + 1) * 64],
                            vt2[0:64, s * 128 : (s + 1) * 128],
                            ident16[0:64, 0:64],
                        )
                    jc0 = 4 * tau
                    pv01 = pv[:, 0:512].rearrange("p (t c) -> p t c", c=128)
                    pv2 = pv[:, 512:768].rearrange("p (t c) -> p t c", c=64)
                    nc.vector.tensor_copy(
                        out=v_view[:, jc0 : jc0 + 4, 0:64], in_=pv01[:, :, 0:64]
                    )
                    nc.vector.tensor_copy(
                        out=v_view[:, jc0 : jc0 + 4, 128:192],
                        in_=pv01[:, :, 64:128],
                    )
                    nc.vector.tensor_copy(
                        out=v_view[:, jc0 : jc0 + 4, 256:320], in_=pv2[:, :, :]
                    )

                # ---------------- pass 2: q projection + attention per i-chunk
                exp_batch_idx = [0]

                def exp_batch(sp):
                    """exp(8u) on a [128,1024] psum batch -> fp16 SBUF tile."""
                    i = exp_batch_idx[0]
                    exp_batch_idx[0] += 1
                    pt = ptp.tile([128, 1024], F16, tag="pt", bufs=4)
                    if use_dve_exp and ((i * EXP_DVE) % EXP_GROUP) < EXP_DVE:
                        tmp = ptp.tile([128, 1024], F16, tag="exptmp", bufs=2)
                        nc.vector._custom_dve(
                            p4_op, out=tmp[:], in0=sp[:],
                            s0=1.0 / 24, s1=1.0 / 6, imm2=0.5,
                        )
                        nc.vector._custom_dve(sq8_op, out=pt[:], in0=tmp[:])
                    else:
                        nc.scalar.activation(pt[:], sp[:], AF.Exp, scale=8.0)
                    return pt

                def q_proj(ic):
                    isl = slice(ic * IC, (ic + 1) * IC)
                    for m, dst in ((0, qT01_t[ic]), (1, qT2_t[ic])):
                        pp = ppsum.tile([128, IC], F32, tag="pp", bufs=1,
                                        name=f"pp{m}_{ic}")
                        proj_chunk(m, isl, pp[:])
                        nc.scalar.activation(
                            dst[:], pp[:], AF.Identity,
                            bias=bq_sb[:, m : m + 1],
                        )

                def norm_store(po_h, ic, h):
                    """o_h = num / l. Builtin reciprocal handles the cross-base
                    read (custom DVE ops only work at partition base 0)."""
                    rr = otp.tile([128, IC], F32, tag="rr", bufs=2)
                    nc.vector.reciprocal(rr[0:64, :], po_h[64:128, :])
                    ot = otp.tile([64, IC], F16, tag="ot", bufs=3)
                    nc.vector.tensor_tensor(
                        out=ot[:], in0=po_h[0:64, :], in1=rr[0:64, :],
                        op=mybir.AluOpType.mult,
                    )
                    nc.sync.dma_start(
                        out=oT_q[ic][64 * h : 64 * h + 64, :], in_=ot[:]
                    )

                def phase3_qtr(qtr):
                    agr = ag_q[qtr][:]
                    ogs = []
                    for dc in range(6):
                        og = ph3.tile([128, IC], F16, tag="og", bufs=8,
                                      name=f"og{dc}_{qtr}")
                        nc.sync.dma_start(
                            out=og[:], in_=agr[dc * 128 : (dc + 1) * 128, :]
                        )
                        ogs.append(og)
                    for ec, (elo, ew) in enumerate(((0, 128), (128, 64))):
                        py = spsum.tile([128, 1024], F32, tag="sp", bufs=2,
                                        name=f"py{ec}_{qtr}")
                        for dc in range(6):
                            nc.tensor.matmul(
                                py[0:ew, 0:IC],
                                wo_sb[:, dc * 192 + elo : dc * 192 + elo + ew],
                                ogs[dc][:],
                                start=(dc == 0), stop=(dc == 5),
                            )
                        ysb = ph3.tile([128, IC], F32, tag="ysb", bufs=3,
                                       name=f"ysb{ec}_{qtr}")
                        nc.scalar.activation(
                            ysb[0:ew, :], py[0:ew, 0:IC], AF.Identity,
                            bias=bo_sb[0:ew, ec : ec + 1],
                        )
                        nc.sync.dma_start(
                            out=y[elo : elo + ew, qtr * IC : (qtr + 1) * IC],
                            in_=ysb[0:ew, :],
                        )

                q_proj(0)
                for ic in range(NIC):
                    isl = slice(ic * IC, (ic + 1) * IC)
                    # attention
                    po = [
                        opsum.tile([128, IC], F32, tag=f"po{h}", bufs=1,
                                   name=f"po{h}_{ic}")
                        for h in range(HPC)
                    ]
                    # heads 0,1: one 128-key chunk per batch, row-tiled pair.
                    # Software-pipelined: QK(jc)+exp(jc) are emitted BEFORE
                    # PV(jc-1) so the in-order PE queue never blocks a ready
                    # QK behind a PV that waits on exp.
                    def qk01(jc):
                        sp = spsum.tile([128, 1024], F32, tag="sp", bufs=2,
                                        name=f"sp01_{ic}_{jc}")
                        nc.tensor.matmul(
                            sp[:, 0:IC],
                            kT01[0:64, jc * 128 : (jc + 1) * 128],
                            qT01_t[ic][0:64, :],
                            start=True, stop=True, tile_position=(0, 0),
                        )
                        nc.tensor.matmul(
                            sp[:, IC:1024],
                            kT01[64:128, jc * 128 : (jc + 1) * 128],
                            qT01_t[ic][64:128, :],
                            start=True, stop=True, tile_position=(64, 0),
                        )
                        return exp_batch(sp)

                    def pv01(jc, pt):
                        nc.tensor.matmul(
                            po[0][:],
                            v_sb[:, jc * VW : jc * VW + 128],
                            pt[:, 0:IC],
                            start=(jc == 0), stop=(jc == 31),
                        )
                        nc.tensor.matmul(
                            po[1][:],
                            v_sb[:, jc * VW + 128 : jc * VW + 256],
                            pt[:, IC:1024],
                            start=(jc == 0), stop=(jc == 31),
                        )

                    prev = None
                    for jc in range(32):
                        pt = qk01(jc)
                        if prev is not None:
                            pv01(prev[0], prev[1])
                        prev = (jc, pt)
                    pv01(prev[0], prev[1])
                    # heads 0/1 done: normalize them while head 2 runs, and
                    # project the next chunk's q to keep the PE queue fed
                    norm_store(po[0], ic, 0)
                    norm_store(po[1], ic, 1)
                    if ic + 1 < NIC:
                        q_proj(ic + 1)

                    # head 2: row-tiled pair over adjacent key chunks
                    def qk2(t):
                        sp = spsum.tile([128, 1024], F32, tag="sp", bufs=2,
                                        name=f"sp2_{ic}_{t}")
                        nc.tensor.matmul(
                            sp[:, 0:IC],
                            kT2[0:64, (2 * t) * 128 : (2 * t + 1) * 128],
                            qT2_t[ic][0:64, :],
                            start=True, stop=True, tile_position=(0, 0),
                        )
                        nc.tensor.matmul(
                            sp[:, IC:1024],
                            kT2[64:128, (2 * t + 1) * 128 : (2 * t + 2) * 128],
                            qT2_t[ic][64:128, :],
                            start=True, stop=True, tile_position=(64, 0),
                        )
                        return exp_batch(sp)

                    def pv2(t, pt):
                        for s in range(2):
                            jc = 2 * t + s
                            nc.tensor.matmul(
                                po[2][:],
                                v_sb[:, jc * VW + 256 : jc * VW + 384],
                                pt[:, s * IC : (s + 1) * IC],
                                start=(jc == 0), stop=(jc == 31),
                            )

                    prev = None
                    for t in range(16):
                        pt = qk2(t)
                        if prev is not None:
                            pv2(prev[0], prev[1])
                        prev = (t, pt)
                    pv2(prev[0], prev[1])
                    norm_store(po[2], ic, 2)
                    nc.gpsimd.collective_compute(
                        "AllGather",
                        mybir.AluOpType.bypass,
                        replica_groups=[[0, 1, 2, 3], [4, 5, 6, 7]],
                        ins=[oT_q[ic][:]],
                        outs=[ag_q[ic][:]],
                    )
                    # inline output projection of the previous quarter: its
                    # AllGather completed during this chunk's attention
                    if ic >= 1:
                        phase3_qtr(ic - 1)
                phase3_qtr(NIC - 1)

    nc.compile()
    _PROG_CACHE[key] = nc
    return nc


# ---------------------------------------------------------------- host wrapper
def make_in_maps(x, w_qkv, b_qkv, w_out, b_out):
    """Build the 8 per-core input dicts from full inputs."""
    in_maps = []
    xTb = [np.ascontiguousarray(x[b].T.astype(np.float16)) for b in range(B)]
    kscale = np.float32(SCALE / 8.0)
    for c in range(N_CORES):
        b = c // 4
        hs = HPC * (c % 4)

        def sect(kind, h):  # q=0,k=1,v=2
            lo = kind * (H * DH) + h * DH
            return w_qkv[:, lo : lo + DH], b_qkv[lo : lo + DH]

        q0, bq0 = sect(0, hs); q1, bq1 = sect(0, hs + 1); q2, bq2 = sect(0, hs + 2)
        k0, bk0 = sect(1, hs); k1, bk1 = sect(1, hs + 1); k2, bk2 = sect(1, hs + 2)
        v0, bv0 = sect(2, hs); v1, bv1 = sect(2, hs + 1); v2, bv2 = sect(2, hs + 2)
        z = np.zeros_like(q2); bz = np.zeros_like(bq2)
        # m-chunks: [q0|q1], [q2|q2], [k0|k1]*s, [k2|k2]*s, [v0|v1], [v2|0]
        cols = np.concatenate(
            [q0, q1, q2, q2, k0 * kscale, k1 * kscale, k2 * kscale, k2 * kscale,
             v0, v1, v2, z], axis=1).astype(np.float16)
        bias = np.concatenate(
            [bq0, bq1, bq2, bq2, bk0 * kscale, bk1 * kscale, bk2 * kscale,
             bk2 * kscale, bv0, bv1, bv2, bz]).astype(np.float32)
        q = c % 4
        bo = np.zeros((2, 128), np.float32)
        bo[0, :] = b_out[192 * q : 192 * q + 128]
        bo[1, :64] = b_out[192 * q + 128 : 192 * q + 192]
        in_maps.append({
            "xT": xTb[b],
            "wqkv": np.ascontiguousarray(cols),
            "bqkv": np.ascontiguousarray(bias.reshape(6, 128)),
            "wout": np.ascontiguousarray(
                w_out[:, 192 * q : 192 * (q + 1)].astype(np.float16)),
            "bout": bo,
        })
    return in_maps


def assemble_output(results):
    out = np.empty((B, N, D), dtype=np.float32)
    for c in range(N_CORES):
        b = c // 4
        q = c % 4
        out[b, :, 192 * q : 192 * (q + 1)] = results[c]["y"].T
    return out


def kernel(x, w_qkv, b_qkv, w_out, b_out):
    from concourse.bass_utils import run_bass_kernel_spmd

    x = np.asarray(x, dtype=np.float32)
    nc = build_program()
    in_maps = make_in_maps(
        x, np.asarray(w_qkv, np.float32), np.asarray(b_qkv, np.float32),
        np.asarray(w_out, np.float32), np.asarray(b_out, np.float32))
    res = run_bass_kernel_spmd(nc, in_maps, core_ids=list(range(N_CORES)))
    return assemble_output(res.results)


# revision 29
# speedup vs baseline: 1.4449x; 1.0943x over previous
"""Multi-head self-attention (B=2, N=4096, D=768, H=12, dh=64) on 8 trn2 NeuronCores.

Sharding: core c handles batch b=c//4 and heads 3*(c%4)..3*(c%4)+2 (head-parallel
attention); an AllGather per 512-token i-chunk redistributes head-outputs so each
core projects its own 192 output columns for all tokens (column-parallel output
projection), assembled host-side.

Per-core pipeline (all matmuls fp16 except psum accumulate):
  x is shipped fp16 and kept resident in SBUF (48KB/partition), so projections
  read it directly with 1024-wide moving operands and no per-pass DMA.
  pass 1: project k (pre-scaled by softmax_scale/8 host-side) and v for all
    tokens; k kept d-on-partition fp16 in row-tiled head-pair layout, v
    PE-transposed into v_sb as [v_h | ones64] fp16 blocks per 128-key chunk —
    the ones columns make each PV matmul emit the softmax denominator
    replicated across psum partitions 64:128 for free.
  pass 2 (interleaved with attention): q projected per 512-token chunk.
  attention per (i-chunk, key-chunk): row-tiled QK pair (concurrent PE
    row-strips) -> psum u = s/8 -> exp(8u) split ~11:5 between ACT table exp
    and a 2-op DVE polynomial (taylor4(u), x^8), DVE batches interleaved so
    the PE never idles a full HAM window -> fp16 PV with [v|ones] stationary
    -> reciprocal[64,512] + multiply -> fp16 AllGather -> column-parallel
    output projection with DVE bias add.
"""
import sys

sys.path.insert(0, "/opt/trn_rl_repo")

import numpy as np

import concourse.bass as bass
import concourse.mybir as mybir
import concourse.tile as tile
import concourse.bacc as bacc
from concourse.masks import make_identity

N_CORES = 8
B, N, D, H, DH = 2, 4096, 768, 12, 64
HPC = 3            # heads per core
SCALE = D ** -0.5
F32 = mybir.dt.float32
F16 = mybir.dt.float16
AF = mybir.ActivationFunctionType
IC = 512           # query chunk size
NIC = N // IC      # 8 i-chunks
VW = 384           # v_sb cols per 128-key chunk: [v0|1*64|v1|1*64|v2|1*64]
EXP_GROUP = 16     # exp batches: (i*EXP_DVE) % 16 < EXP_DVE go to the DVE path
EXP_DVE = 4        # evenly interleaved: every 4th batch on DVE


# ---------------------------------------------------------------- custom DVE exp
def _register_exp_ops():
    """exp(8u) as two DVE ops: EXP_P4_ANT = taylor4(u); EXP_SQ8_ANT = x^8."""
    import concourse.dve_ops as dve_ops
    from concourse.dve_ops import DveOp, OPS, CUSTOM_DVE_SPECS, _SUB_OPCODE_FOR_NAME
    from concourse.dve_spec import Spec, Src0, C0, C1, C2, One, sq, lower
    from concourse.dve_uop import DveOpSpec

    if "EXP_P4_ANT" in _SUB_OPCODE_FOR_NAME:
        return dve_ops.EXP_P4_ANT, dve_ops.EXP_SQ8_ANT

    u = Src0
    p4 = ((((u * C0) + C1) * u + C2) * u + One) * u + One  # c0=1/24 c1=1/6 c2=1/2
    spec_p4 = Spec(
        body=p4,
        reference=lambda in0, in1, s0, s1, imm2: (
            (((in0 * s0 + s1) * in0 + imm2) * in0 + 1.0) * in0 + 1.0
        ),
    )
    spec_sq8 = Spec(
        body=sq(sq(sq(Src0))),
        reference=lambda in0, in1, s0, s1, imm2: in0 ** 8,
    )

    def _mk(name, spec):
        opcode = max(_SUB_OPCODE_FOR_NAME.values()) + 1
        _SUB_OPCODE_FOR_NAME[name] = opcode
        shas = {}
        for ver in ("v3", "v4"):
            s = DveOpSpec(
                name=name, opcode=opcode, uops=lower(spec, ver=ver), rd1_en=False
            )
            shas[ver] = s.sha(ver)
        op = DveOp(name, spec, subdim=False, uops_sha=shas)
        OPS.append(op)
        CUSTOM_DVE_SPECS[name] = spec
        setattr(dve_ops, name, op)
        return op

    p4_op = _mk("EXP_P4_ANT", spec_p4)
    sq8_op = _mk("EXP_SQ8_ANT", spec_sq8)
    return p4_op, sq8_op


# ---------------------------------------------------------------- program build
_PROG_CACHE = {}


def build_program(use_dve_exp=True):
    key = ("prog", use_dve_exp)
    if key in _PROG_CACHE:
        return _PROG_CACHE[key]
    p4_op, sq8_op = _register_exp_ops()

    nc = bacc.Bacc("TRN2", target_bir_lowering=False, debug=False, num_devices=N_CORES)

    xT = nc.dram_tensor("xT", [D, N], F16, kind="ExternalInput").ap()
    wqkv = nc.dram_tensor("wqkv", [D, 768], F16, kind="ExternalInput").ap()
    bqkv = nc.dram_tensor("bqkv", [6, 128], F32, kind="ExternalInput").ap()
    wout = nc.dram_tensor("wout", [D, 192], F16, kind="ExternalInput").ap()
    bout = nc.dram_tensor("bout", [2, 128], F32, kind="ExternalInput").ap()
    y = nc.dram_tensor("y", [HPC * DH, N], F32, kind="ExternalOutput").ap()

    with tile.TileContext(nc, trace_sim=False) as tc:
        with (
            tc.tile_pool(name="consts", bufs=1) as consts,
            tc.tile_pool(name="persist", bufs=1) as persist,
            tc.tile_pool(name="otp", bufs=2) as otp,
            tc.tile_pool(name="spsum", bufs=3, space="PSUM") as spsum,
            tc.tile_pool(name="opsum", bufs=1, space="PSUM") as opsum,
            tc.tile_pool(name="dram", bufs=1, space="DRAM") as dram,
        ):
            # ---------------- constants + resident fp16 x
            ident_f = consts.tile([128, 128], F32)
            make_identity(nc, ident_f[:])
            ident16 = consts.tile([128, 128], F16)
            nc.scalar.copy(ident16[:], ident_f[:])

            # resident fp16 x, one tile per 1024-token block so early
            # projections only wait on their own block's DMAs
            x16_t = [
                persist.tile([128, 6 * 1024], F16, tag=f"x16_{nq}",
                             name=f"x16_{nq}")
                for nq in range(4)
            ]
            for nq in range(4):
                for fc in range(6):
                    for pb in range(2):
                        nc.sync.dma_start(
                            out=x16_t[nq][64 * pb : 64 * (pb + 1),
                                          fc * 1024 : (fc + 1) * 1024],
                            in_=xT[fc * 128 + 64 * pb : fc * 128 + 64 * (pb + 1),
                                   nq * 1024 : (nq + 1) * 1024],
                        )
            wq_sb = consts.tile([128, 6 * 768], F16)  # 6 f-chunks of [128,768]
            for fc in range(6):
                nc.sync.dma_start(
                    out=wq_sb[:, fc * 768 : (fc + 1) * 768],
                    in_=wqkv[fc * 128 : (fc + 1) * 128, :],
                )
            bq_sb = consts.tile([128, 6], F32)  # per-m-chunk bias columns
            for m in range(6):
                nc.sync.dma_start(
                    out=bq_sb[:, m : m + 1],
                    in_=bqkv[m : m + 1, :].rearrange("a p -> p a"),
                )

            # ---------------- persistent activations (all fp16)
            # q tiles are per-i-chunk so projecting chunk ic+1 mid-attention
            # carries no tile-granularity WAR against chunk ic's reads
            qT01_t = [
                persist.tile([128, IC], F16, tag=f"q01_{i}", name=f"q01_{i}")
                for i in range(NIC)
            ]
            qT2_t = [
                persist.tile([128, IC], F16, tag=f"q2_{i}", name=f"q2_{i}")
                for i in range(NIC)
            ]
            kT01 = persist.tile([128, N], F16)  # scaled kT, heads 0/1
            kT2 = persist.tile([128, N], F16)   # scaled kT head2, duplicated
            v_sb = persist.tile([128, 32 * VW], F16)
            v_view = v_sb[:].rearrange("p (j c) -> p j c", c=VW)
            for h in range(HPC):
                nc.vector.memset(v_view[:, :, h * 128 + 64 : h * 128 + 128], 1.0)

            # AllGather eighths: in [192, 512] -> out [768, 512] (rank-major rows)
            oT_q = [dram.tile([HPC * DH, IC], F16, name=f"oT_q{i}") for i in range(8)]
            ag_q = [dram.tile([D, IC], F16, name=f"ag_q{i}") for i in range(8)]

            # output projection weights (phase 3 is emitted inline per i-chunk)
            wo_sb = consts.tile([128, 6 * 192], F16)  # w_out col-slice, 6 d-chunks
            for dc in range(6):
                nc.sync.dma_start(
                    out=wo_sb[:, dc * 192 : (dc + 1) * 192],
                    in_=wout[dc * 128 : (dc + 1) * 128, :],
                )
            bo_sb = consts.tile([128, 2], F32)
            for m in range(2):
                nc.sync.dma_start(
                    out=bo_sb[:, m : m + 1],
                    in_=bout[m : m + 1, :].rearrange("a p -> p a"),
                )

            with tc.tile_pool(name="work", bufs=2) as work, \
                 tc.tile_pool(name="ptp", bufs=3) as ptp, \
                 tc.tile_pool(name="ph3", bufs=1) as ph3:

                def proj_chunk(m, tsl, out_psum):
                    """fp16 matmul of m-chunk over token slice tsl into psum."""
                    nq, off = tsl.start // 1024, tsl.start % 1024
                    w = tsl.stop - tsl.start
                    for fc in range(6):
                        nc.tensor.matmul(
                            out_psum,
                            wq_sb[:, fc * 768 + m * 128 : fc * 768 + (m + 1) * 128],
                            x16_t[nq][:, fc * 1024 + off : fc * 1024 + off + w],
                            start=(fc == 0),
                            stop=(fc == 5),
                        )

                # ---------------- pass 1: k + v projection (q deferred)
                for tau in range(8):
                    ts = slice(tau * IC, (tau + 1) * IC)
                    for m, dst in ((2, kT01), (3, kT2)):
                        sp = spsum.tile([128, 1024], F32, tag="sp", bufs=3)
                        proj_chunk(m, ts, sp[:, 0:IC])
                        nc.scalar.activation(
                            dst[:, ts], sp[:, 0:IC], AF.Identity,
                            bias=bq_sb[:, m : m + 1],
                        )
                    vt01 = work.tile([128, IC], F16, tag="vt01")
                    vt2 = work.tile([64, IC], F16, tag="vt2")
                    sp = spsum.tile([128, 1024], F32, tag="sp", bufs=3)
                    proj_chunk(4, ts, sp[:, 0:IC])
                    nc.scalar.activation(
                        vt01[:], sp[:, 0:IC], AF.Identity, bias=bq_sb[:, 4:5]
                    )
                    sp = spsum.tile([128, 1024], F32, tag="sp", bufs=3)
                    proj_chunk(5, ts, sp[:, 0:IC])
                    nc.scalar.activation(
                        vt2[:], sp[0:64, 0:IC], AF.Identity, bias=bq_sb[0:64, 5:6]
                    )
                    # transpose v into token-major [v|ones] layout, 4 t-blocks
                    # per psum tile: [v01(t0..t3) 512 | v2(t0..t3) 256]
                    pv = spsum.tile([128, 2048], F16, tag="sp", bufs=3)
                    for s in range(4):
                        nc.tensor.transpose(
                            pv[:, s * 128 : (s + 1) * 128],
                            vt01[:, s * 128 : (s + 1) * 128],
                            ident16[:],
                        )
                        nc.tensor.transpose(
                            pv[:, 512 + s * 64 : 512 + (s + 1) * 64],
                            vt2[0:64, s * 128 : (s + 1) * 128],
                            ident16[0:64, 0:64],
                        )
                    jc0 = 4 * tau
                    pv01 = pv[:, 0:512].rearrange("p (t c) -> p t c", c=128)
                    pv2 = pv[:, 512:768].rearrange("p (t c) -> p t c", c=64)
                    nc.vector.tensor_copy(
                        out=v_view[:, jc0 : jc0 + 4, 0:64], in_=pv01[:, :, 0:64]
                    )
                    nc.vector.tensor_copy(
                        out=v_view[:, jc0 : jc0 + 4, 128:192],
                        in_=pv01[:, :, 64:128],
                    )
                    nc.vector.tensor_copy(
                        out=v_view[:, jc0 : jc0 + 4, 256:320], in_=pv2[:, :, :]
                    )

                # ---------------- pass 2: q projection + attention per i-chunk
                exp_batch_idx = [0]

                def exp_batch(sp):
                    """exp(8u) on a [128,1024] psum batch -> fp16 SBUF tile."""
                    i = exp_batch_idx[0]
                    exp_batch_idx[0] += 1
                    pt = ptp.tile([128, 1024], F16, tag="pt", bufs=4)
                    if use_dve_exp and ((i * EXP_DVE) % EXP_GROUP) < EXP_DVE:
                        tmp = ptp.tile([128, 1024], F16, tag="exptmp", bufs=2)
                        nc.vector._custom_dve(
                            p4_op, out=tmp[:], in0=sp[:],
                            s0=1.0 / 24, s1=1.0 / 6, imm2=0.5,
                        )
                        nc.vector._custom_dve(sq8_op, out=pt[:], in0=tmp[:])
                    else:
                        nc.scalar.activation(pt[:], sp[:], AF.Exp, scale=8.0)
                    return pt

                def q_proj(ic):
                    isl = slice(ic * IC, (ic + 1) * IC)
                    for m, dst in ((0, qT01_t[ic]), (1, qT2_t[ic])):
                        pp = spsum.tile([128, 1024], F32, tag="sp", bufs=3,
                                        name=f"pp{m}_{ic}")
                        proj_chunk(m, isl, pp[:, 0:IC])
                        nc.scalar.activation(
                            dst[:], pp[:, 0:IC], AF.Identity,
                            bias=bq_sb[:, m : m + 1],
                        )

                def norm_store(po_h, ic, h):
                    """o_h = num / l. Builtin reciprocal handles the cross-base
                    read (custom DVE ops only work at partition base 0)."""
                    rr = otp.tile([128, IC], F32, tag="rr", bufs=2)
                    nc.vector.reciprocal(rr[0:64, :], po_h[64:128, :])
                    ot = otp.tile([64, IC], F16, tag="ot", bufs=3)
                    nc.vector.tensor_tensor(
                        out=ot[:], in0=po_h[0:64, :], in1=rr[0:64, :],
                        op=mybir.AluOpType.mult,
                    )
                    nc.sync.dma_start(
                        out=oT_q[ic][64 * h : 64 * h + 64, :], in_=ot[:]
                    )

                def phase3_qtr(qtr):
                    agr = ag_q[qtr][:]
                    ogs = []
                    for dc in range(6):
                        og = ph3.tile([128, IC], F16, tag="og", bufs=8,
                                      name=f"og{dc}_{qtr}")
                        nc.sync.dma_start(
                            out=og[:], in_=agr[dc * 128 : (dc + 1) * 128, :]
                        )
                        ogs.append(og)
                    for ec, (elo, ew) in enumerate(((0, 128), (128, 64))):
                        py = spsum.tile([128, 1024], F32, tag="sp", bufs=3,
                                        name=f"py{ec}_{qtr}")
                        for dc in range(6):
                            nc.tensor.matmul(
                                py[0:ew, 0:IC],
                                wo_sb[:, dc * 192 + elo : dc * 192 + elo + ew],
                                ogs[dc][:],
                                start=(dc == 0), stop=(dc == 5),
                            )
                        ysb = ph3.tile([128, IC], F32, tag="ysb", bufs=3,
                                       name=f"ysb{ec}_{qtr}")
                        nc.scalar.activation(
                            ysb[0:ew, :], py[0:ew, 0:IC], AF.Identity,
                            bias=bo_sb[0:ew, ec : ec + 1],
                        )
                        nc.sync.dma_start(
                            out=y[elo : elo + ew, qtr * IC : (qtr + 1) * IC],
                            in_=ysb[0:ew, :],
                        )

                q_proj(0)
                for ic in range(NIC):
                    isl = slice(ic * IC, (ic + 1) * IC)
                    # attention
                    po = [
                        opsum.tile([128, IC], F32, tag="po0", bufs=1,
                                   name=f"po0_{ic}"),
                        opsum.tile([128, IC], F32, tag="po1", bufs=1,
                                   name=f"po1_{ic}"),
                    ]
                    # heads 0,1: one 128-key chunk per batch, row-tiled pair.
                    # Software-pipelined: QK(jc)+exp(jc) are emitted BEFORE
                    # PV(jc-1) so the in-order PE queue never blocks a ready
                    # QK behind a PV that waits on exp.
                    def qk01(jc):
                        sp = spsum.tile([128, 1024], F32, tag="sp", bufs=3,
                                        name=f"sp01_{ic}_{jc}")
                        nc.tensor.matmul(
                            sp[:, 0:IC],
                            kT01[0:64, jc * 128 : (jc + 1) * 128],
                            qT01_t[ic][0:64, :],
                            start=True, stop=True, tile_position=(0, 0),
                        )
                        nc.tensor.matmul(
                            sp[:, IC:1024],
                            kT01[64:128, jc * 128 : (jc + 1) * 128],
                            qT01_t[ic][64:128, :],
                            start=True, stop=True, tile_position=(64, 0),
                        )
                        return exp_batch(sp)

                    def pv01(jc, pt):
                        nc.tensor.matmul(
                            po[0][:],
                            v_sb[:, jc * VW : jc * VW + 128],
                            pt[:, 0:IC],
                            start=(jc == 0), stop=(jc == 31),
                        )
                        nc.tensor.matmul(
                            po[1][:],
                            v_sb[:, jc * VW + 128 : jc * VW + 256],
                            pt[:, IC:1024],
                            start=(jc == 0), stop=(jc == 31),
                        )

                    prev = None
                    for jc in range(32):
                        pt = qk01(jc)
                        if prev is not None:
                            pv01(prev[0], prev[1])
                        prev = (jc, pt)
                    pv01(prev[0], prev[1])
                    # heads 0/1 done: normalize them while head 2 runs, and
                    # project the next chunk's q to keep the PE queue fed
                    norm_store(po[0], ic, 0)
                    norm_store(po[1], ic, 1)
                    if ic + 1 < NIC:
                        q_proj(ic + 1)
                    po.append(
                        opsum.tile([128, IC], F32, tag="po0", bufs=1,
                                   name=f"po2_{ic}")
                    )

                    # head 2: row-tiled pair over adjacent key chunks
                    def qk2(t):
                        sp = spsum.tile([128, 1024], F32, tag="sp", bufs=3,
                                        name=f"sp2_{ic}_{t}")
                        nc.tensor.matmul(
                            sp[:, 0:IC],
                            kT2[0:64, (2 * t) * 128 : (2 * t + 1) * 128],
                            qT2_t[ic][0:64, :],
                            start=True, stop=True, tile_position=(0, 0),
                        )
                        nc.tensor.matmul(
                            sp[:, IC:1024],
                            kT2[64:128, (2 * t + 1) * 128 : (2 * t + 2) * 128],
                            qT2_t[ic][64:128, :],
                            start=True, stop=True, tile_position=(64, 0),
                        )
                        return exp_batch(sp)

                    def pv2(t, pt):
                        for s in range(2):
                            jc = 2 * t + s
                            nc.tensor.matmul(
                                po[2][:],
                                v_sb[:, jc * VW + 256 : jc * VW + 384],
                                pt[:, s * IC : (s + 1) * IC],
                                start=(jc == 0), stop=(jc == 31),
                            )

                    prev = None
                    for t in range(16):
                        pt = qk2(t)
                        if prev is not None:
                            pv2(prev[0], prev[1])
                        prev = (t, pt)
                    pv2(prev[0], prev[1])
                    norm_store(po[2], ic, 2)
                    nc.gpsimd.collective_compute(
                        "AllGather",
                        mybir.AluOpType.bypass,
                        replica_groups=[[0, 1, 2, 3], [4, 5, 6, 7]],
                        ins=[oT_q[ic][:]],
                        outs=[ag_q[ic][:]],
                    )
                    # inline output projection of the previous quarter: its
                    # AllGather completed during this chunk's attention
                    if ic >= 1:
                        phase3_qtr(ic - 1)
                phase3_qtr(NIC - 1)

    nc.compile()
    _PROG_CACHE[key] = nc
    return nc


# ---------------------------------------------------------------- host wrapper
def make_in_maps(x, w_qkv, b_qkv, w_out, b_out):
    """Build the 8 per-core input dicts from full inputs."""
    in_maps = []
    xTb = [np.ascontiguousarray(x[b].T.astype(np.float16)) for b in range(B)]
    kscale = np.float32(SCALE / 8.0)
    for c in range(N_CORES):
        b = c // 4
        hs = HPC * (c % 4)

        def sect(kind, h):  # q=0,k=1,v=2
            lo = kind * (H * DH) + h * DH
            return w_qkv[:, lo : lo + DH], b_qkv[lo : lo + DH]

        q0, bq0 = sect(0, hs); q1, bq1 = sect(0, hs + 1); q2, bq2 = sect(0, hs + 2)
        k0, bk0 = sect(1, hs); k1, bk1 = sect(1, hs + 1); k2, bk2 = sect(1, hs + 2)
        v0, bv0 = sect(2, hs); v1, bv1 = sect(2, hs + 1); v2, bv2 = sect(2, hs + 2)
        z = np.zeros_like(q2); bz = np.zeros_like(bq2)
        # m-chunks: [q0|q1], [q2|q2], [k0|k1]*s, [k2|k2]*s, [v0|v1], [v2|0]
        cols = np.concatenate(
            [q0, q1, q2, q2, k0 * kscale, k1 * kscale, k2 * kscale, k2 * kscale,
             v0, v1, v2, z], axis=1).astype(np.float16)
        bias = np.concatenate(
            [bq0, bq1, bq2, bq2, bk0 * kscale, bk1 * kscale, bk2 * kscale,
             bk2 * kscale, bv0, bv1, bv2, bz]).astype(np.float32)
        q = c % 4
        bo = np.zeros((2, 128), np.float32)
        bo[0, :] = b_out[192 * q : 192 * q + 128]
        bo[1, :64] = b_out[192 * q + 128 : 192 * q + 192]
        in_maps.append({
            "xT": xTb[b],
            "wqkv": np.ascontiguousarray(cols),
            "bqkv": np.ascontiguousarray(bias.reshape(6, 128)),
            "wout": np.ascontiguousarray(
                w_out[:, 192 * q : 192 * (q + 1)].astype(np.float16)),
            "bout": bo,
        })
    return in_maps


def assemble_output(results):
    out = np.empty((B, N, D), dtype=np.float32)
    for c in range(N_CORES):
        b = c // 4
        q = c % 4
        out[b, :, 192 * q : 192 * (q + 1)] = results[c]["y"].T
    return out


def kernel(x, w_qkv, b_qkv, w_out, b_out):
    from concourse.bass_utils import run_bass_kernel_spmd

    x = np.asarray(x, dtype=np.float32)
    nc = build_program()
    in_maps = make_in_maps(
        x, np.asarray(w_qkv, np.float32), np.asarray(b_qkv, np.float32),
        np.asarray(w_out, np.float32), np.asarray(b_out, np.float32))
    res = run_bass_kernel_spmd(nc, in_maps, core_ids=list(range(N_CORES)))
    return assemble_output(res.results)


# revision 30
# speedup vs baseline: 1.4836x; 1.0267x over previous
"""Multi-head self-attention (B=2, N=4096, D=768, H=12, dh=64) on 8 trn2 NeuronCores.

Sharding: core c handles batch b=c//4 and heads 3*(c%4)..3*(c%4)+2 (head-parallel
attention); an AllGather per 512-token i-chunk redistributes head-outputs so each
core projects its own 192 output columns for all tokens (column-parallel output
projection), assembled host-side.

Per-core pipeline (all matmuls fp16 except psum accumulate):
  x is shipped fp16 and kept resident in SBUF (48KB/partition), so projections
  read it directly with 1024-wide moving operands and no per-pass DMA.
  pass 1: project k (pre-scaled by softmax_scale/8 host-side) and v for all
    tokens; k kept d-on-partition fp16 in row-tiled head-pair layout, v
    PE-transposed into v_sb as [v_h | ones64] fp16 blocks per 128-key chunk —
    the ones columns make each PV matmul emit the softmax denominator
    replicated across psum partitions 64:128 for free.
  pass 2 (interleaved with attention): q projected per 512-token chunk.
  attention per (i-chunk, key-chunk): row-tiled QK pair (concurrent PE
    row-strips) -> psum u = s/8 -> exp(8u) split ~11:5 between ACT table exp
    and a 2-op DVE polynomial (taylor4(u), x^8), DVE batches interleaved so
    the PE never idles a full HAM window -> fp16 PV with [v|ones] stationary
    -> reciprocal[64,512] + multiply -> fp16 AllGather -> column-parallel
    output projection with DVE bias add.
"""
import sys

sys.path.insert(0, "/opt/trn_rl_repo")

import numpy as np

import concourse.bass as bass
import concourse.mybir as mybir
import concourse.tile as tile
import concourse.bacc as bacc
from concourse.masks import make_identity

N_CORES = 8
B, N, D, H, DH = 2, 4096, 768, 12, 64
HPC = 3            # heads per core
SCALE = D ** -0.5
F32 = mybir.dt.float32
F16 = mybir.dt.float16
AF = mybir.ActivationFunctionType
IC = 512           # query chunk size
NIC = N // IC      # 8 i-chunks
VW = 384           # v_sb cols per 128-key chunk: [v0|1*64|v1|1*64|v2|1*64]
EXP_GROUP = 16     # exp batches: (i*EXP_DVE) % 16 < EXP_DVE go to the DVE path
EXP_DVE = 4        # evenly interleaved: every 4th batch on DVE


# ---------------------------------------------------------------- custom DVE exp
def _register_exp_ops():
    """exp(8u) as two DVE ops: EXP_P4_ANT = taylor4(u); EXP_SQ8_ANT = x^8."""
    import concourse.dve_ops as dve_ops
    from concourse.dve_ops import DveOp, OPS, CUSTOM_DVE_SPECS, _SUB_OPCODE_FOR_NAME
    from concourse.dve_spec import Spec, Src0, C0, C1, C2, One, sq, lower
    from concourse.dve_uop import DveOpSpec

    if "EXP_P4_ANT" in _SUB_OPCODE_FOR_NAME:
        return dve_ops.EXP_P4_ANT, dve_ops.EXP_SQ8_ANT

    u = Src0
    p4 = ((((u * C0) + C1) * u + C2) * u + One) * u + One  # c0=1/24 c1=1/6 c2=1/2
    spec_p4 = Spec(
        body=p4,
        reference=lambda in0, in1, s0, s1, imm2: (
            (((in0 * s0 + s1) * in0 + imm2) * in0 + 1.0) * in0 + 1.0
        ),
    )
    spec_sq8 = Spec(
        body=sq(sq(sq(Src0))),
        reference=lambda in0, in1, s0, s1, imm2: in0 ** 8,
    )

    def _mk(name, spec):
        opcode = max(_SUB_OPCODE_FOR_NAME.values()) + 1
        _SUB_OPCODE_FOR_NAME[name] = opcode
        shas = {}
        for ver in ("v3", "v4"):
            s = DveOpSpec(
                name=name, opcode=opcode, uops=lower(spec, ver=ver), rd1_en=False
            )
            shas[ver] = s.sha(ver)
        op = DveOp(name, spec, subdim=False, uops_sha=shas)
        OPS.append(op)
        CUSTOM_DVE_SPECS[name] = spec
        setattr(dve_ops, name, op)
        return op

    p4_op = _mk("EXP_P4_ANT", spec_p4)
    sq8_op = _mk("EXP_SQ8_ANT", spec_sq8)
    return p4_op, sq8_op


# ---------------------------------------------------------------- program build
_PROG_CACHE = {}


def build_program(use_dve_exp=True):
    key = ("prog", use_dve_exp)
    if key in _PROG_CACHE:
        return _PROG_CACHE[key]
    p4_op, sq8_op = _register_exp_ops()

    nc = bacc.Bacc("TRN2", target_bir_lowering=False, debug=False, num_devices=N_CORES)

    xT = nc.dram_tensor("xT", [D, N], F16, kind="ExternalInput").ap()
    wqkv = nc.dram_tensor("wqkv", [D, 768], F16, kind="ExternalInput").ap()
    bqkv = nc.dram_tensor("bqkv", [6, 128], F32, kind="ExternalInput").ap()
    wout = nc.dram_tensor("wout", [D, 192], F16, kind="ExternalInput").ap()
    bout = nc.dram_tensor("bout", [2, 128], F32, kind="ExternalInput").ap()
    y = nc.dram_tensor("y", [HPC * DH, N], F32, kind="ExternalOutput").ap()

    with tile.TileContext(nc, trace_sim=False) as tc:
        with (
            tc.tile_pool(name="consts", bufs=1) as consts,
            tc.tile_pool(name="persist", bufs=1) as persist,
            tc.tile_pool(name="otp", bufs=2) as otp,
            tc.tile_pool(name="spsum", bufs=3, space="PSUM") as spsum,
            tc.tile_pool(name="opsum", bufs=1, space="PSUM") as opsum,
            tc.tile_pool(name="dram", bufs=1, space="DRAM") as dram,
        ):
            # ---------------- constants + resident fp16 x
            ident_f = consts.tile([128, 128], F32)
            make_identity(nc, ident_f[:])
            ident16 = consts.tile([128, 128], F16)
            nc.scalar.copy(ident16[:], ident_f[:])

            # resident fp16 x, one tile per 1024-token block so early
            # projections only wait on their own block's DMAs
            x16_t = [
                persist.tile([128, 6 * 1024], F16, tag=f"x16_{nq}",
                             name=f"x16_{nq}")
                for nq in range(4)
            ]
            for nq in range(4):
                for fc in range(6):
                    for pb in range(2):
                        nc.sync.dma_start(
                            out=x16_t[nq][64 * pb : 64 * (pb + 1),
                                          fc * 1024 : (fc + 1) * 1024],
                            in_=xT[fc * 128 + 64 * pb : fc * 128 + 64 * (pb + 1),
                                   nq * 1024 : (nq + 1) * 1024],
                        )
            wq_sb = consts.tile([128, 6 * 768], F16)  # 6 f-chunks of [128,768]
            for fc in range(6):
                nc.sync.dma_start(
                    out=wq_sb[:, fc * 768 : (fc + 1) * 768],
                    in_=wqkv[fc * 128 : (fc + 1) * 128, :],
                )
            bq_sb = consts.tile([128, 6], F32)  # per-m-chunk bias columns
            for m in range(6):
                nc.sync.dma_start(
                    out=bq_sb[:, m : m + 1],
                    in_=bqkv[m : m + 1, :].rearrange("a p -> p a"),
                )

            # ---------------- persistent activations (all fp16)
            # q tiles are per-i-chunk so projecting chunk ic+1 mid-attention
            # carries no tile-granularity WAR against chunk ic's reads
            qT01_t = [
                persist.tile([128, IC], F16, tag=f"q01_{i}", name=f"q01_{i}")
                for i in range(NIC)
            ]
            qT2_t = [
                persist.tile([128, IC], F16, tag=f"q2_{i}", name=f"q2_{i}")
                for i in range(NIC)
            ]
            kT01 = persist.tile([128, N], F16)  # scaled kT, heads 0/1
            kT2 = persist.tile([128, N], F16)   # scaled kT head2, duplicated
            v_sb = persist.tile([128, 32 * VW], F16)
            v_view = v_sb[:].rearrange("p (j c) -> p j c", c=VW)
            for h in range(HPC):
                nc.vector.memset(v_view[:, :, h * 128 + 64 : h * 128 + 128], 1.0)

            # AllGather eighths: in [192, 512] -> out [768, 512] (rank-major rows)
            oT_q = [dram.tile([HPC * DH, IC], F16, name=f"oT_q{i}") for i in range(8)]
            ag_q = [dram.tile([D, IC], F16, name=f"ag_q{i}") for i in range(8)]

            # output projection weights (phase 3 is emitted inline per i-chunk)
            wo_sb = consts.tile([128, 6 * 192], F16)  # w_out col-slice, 6 d-chunks
            for dc in range(6):
                nc.sync.dma_start(
                    out=wo_sb[:, dc * 192 : (dc + 1) * 192],
                    in_=wout[dc * 128 : (dc + 1) * 128, :],
                )
            bo_sb = consts.tile([128, 2], F32)
            for m in range(2):
                nc.sync.dma_start(
                    out=bo_sb[:, m : m + 1],
                    in_=bout[m : m + 1, :].rearrange("a p -> p a"),
                )

            with tc.tile_pool(name="work", bufs=2) as work, \
                 tc.tile_pool(name="ptp", bufs=3) as ptp, \
                 tc.tile_pool(name="ph3", bufs=1) as ph3:

                def proj_chunk(m, tsl, out_psum):
                    """fp16 matmul of m-chunk over token slice tsl into psum."""
                    nq, off = tsl.start // 1024, tsl.start % 1024
                    w = tsl.stop - tsl.start
                    for fc in range(6):
                        nc.tensor.matmul(
                            out_psum,
                            wq_sb[:, fc * 768 + m * 128 : fc * 768 + (m + 1) * 128],
                            x16_t[nq][:, fc * 1024 + off : fc * 1024 + off + w],
                            start=(fc == 0),
                            stop=(fc == 5),
                        )

                # ---------------- pass 1: k + v projection (q deferred)
                for tau in range(8):
                    ts = slice(tau * IC, (tau + 1) * IC)
                    for m, dst in ((2, kT01), (3, kT2)):
                        sp = spsum.tile([128, 1024], F32, tag="sp", bufs=3)
                        proj_chunk(m, ts, sp[:, 0:IC])
                        nc.scalar.activation(
                            dst[:, ts], sp[:, 0:IC], AF.Identity,
                            bias=bq_sb[:, m : m + 1],
                        )
                    vt01 = work.tile([128, IC], F16, tag="vt01")
                    vt2 = work.tile([64, IC], F16, tag="vt2")
                    sp = spsum.tile([128, 1024], F32, tag="sp", bufs=3)
                    proj_chunk(4, ts, sp[:, 0:IC])
                    nc.scalar.activation(
                        vt01[:], sp[:, 0:IC], AF.Identity, bias=bq_sb[:, 4:5]
                    )
                    sp = spsum.tile([128, 1024], F32, tag="sp", bufs=3)
                    proj_chunk(5, ts, sp[:, 0:IC])
                    nc.scalar.activation(
                        vt2[:], sp[0:64, 0:IC], AF.Identity, bias=bq_sb[0:64, 5:6]
                    )
                    # transpose v into token-major [v|ones] layout, 4 t-blocks
                    # per psum tile: [v01(t0..t3) 512 | v2(t0..t3) 256]
                    pv = spsum.tile([128, 2048], F16, tag="sp", bufs=3)
                    for s in range(4):
                        nc.tensor.transpose(
                            pv[:, s * 128 : (s + 1) * 128],
                            vt01[:, s * 128 : (s + 1) * 128],
                            ident16[:],
                        )
                        nc.tensor.transpose(
                            pv[:, 512 + s * 64 : 512 + (s + 1) * 64],
                            vt2[0:64, s * 128 : (s + 1) * 128],
                            ident16[0:64, 0:64],
                        )
                    jc0 = 4 * tau
                    pv01 = pv[:, 0:512].rearrange("p (t c) -> p t c", c=128)
                    pv2 = pv[:, 512:768].rearrange("p (t c) -> p t c", c=64)
                    nc.vector.tensor_copy(
                        out=v_view[:, jc0 : jc0 + 4, 0:64], in_=pv01[:, :, 0:64]
                    )
                    nc.vector.tensor_copy(
                        out=v_view[:, jc0 : jc0 + 4, 128:192],
                        in_=pv01[:, :, 64:128],
                    )
                    nc.vector.tensor_copy(
                        out=v_view[:, jc0 : jc0 + 4, 256:320], in_=pv2[:, :, :]
                    )

                # ---------------- pass 2: q projection + attention per i-chunk
                exp_batch_idx = [0]

                def exp_batch(sp):
                    """exp(8u) on a [128,1024] psum batch -> fp16 SBUF tile."""
                    i = exp_batch_idx[0]
                    exp_batch_idx[0] += 1
                    pt = ptp.tile([128, 1024], F16, tag="pt", bufs=4)
                    if use_dve_exp and ((i * EXP_DVE) % EXP_GROUP) < EXP_DVE:
                        tmp = ptp.tile([128, 1024], F16, tag="exptmp", bufs=2)
                        nc.vector._custom_dve(
                            p4_op, out=tmp[:], in0=sp[:],
                            s0=1.0 / 24, s1=1.0 / 6, imm2=0.5,
                        )
                        nc.vector._custom_dve(sq8_op, out=pt[:], in0=tmp[:])
                    else:
                        nc.scalar.activation(pt[:], sp[:], AF.Exp, scale=8.0)
                    return pt

                def q_proj(ic):
                    isl = slice(ic * IC, (ic + 1) * IC)
                    for m, dst in ((0, qT01_t[ic]), (1, qT2_t[ic])):
                        pp = spsum.tile([128, 1024], F32, tag="sp", bufs=3,
                                        name=f"pp{m}_{ic}")
                        proj_chunk(m, isl, pp[:, 0:IC])
                        nc.scalar.activation(
                            dst[:], pp[:, 0:IC], AF.Identity,
                            bias=bq_sb[:, m : m + 1],
                        )

                def norm_store(po_h, ic, h):
                    """o_h = num / l. Builtin reciprocal handles the cross-base
                    read (custom DVE ops only work at partition base 0)."""
                    rr = otp.tile([128, IC], F32, tag="rr", bufs=2)
                    nc.vector.reciprocal(rr[0:64, :], po_h[64:128, :])
                    ot = otp.tile([64, IC], F16, tag="ot", bufs=3)
                    nc.vector.tensor_tensor(
                        out=ot[:], in0=po_h[0:64, :], in1=rr[0:64, :],
                        op=mybir.AluOpType.mult,
                    )
                    nc.sync.dma_start(
                        out=oT_q[ic][64 * h : 64 * h + 64, :], in_=ot[:]
                    )

                def phase3_qtr(qtr):
                    agr = ag_q[qtr][:]
                    ogs = []
                    for dc in range(6):
                        og = ph3.tile([128, IC], F16, tag="og", bufs=8,
                                      name=f"og{dc}_{qtr}")
                        nc.sync.dma_start(
                            out=og[:], in_=agr[dc * 128 : (dc + 1) * 128, :]
                        )
                        ogs.append(og)
                    for ec, (elo, ew) in enumerate(((0, 128), (128, 64))):
                        py = spsum.tile([128, 1024], F32, tag="sp", bufs=3,
                                        name=f"py{ec}_{qtr}")
                        for dc in range(6):
                            nc.tensor.matmul(
                                py[0:ew, 0:IC],
                                wo_sb[:, dc * 192 + elo : dc * 192 + elo + ew],
                                ogs[dc][:],
                                start=(dc == 0), stop=(dc == 5),
                            )
                        ysb = ph3.tile([128, IC], F32, tag="ysb", bufs=3,
                                       name=f"ysb{ec}_{qtr}")
                        nc.scalar.activation(
                            ysb[0:ew, :], py[0:ew, 0:IC], AF.Identity,
                            bias=bo_sb[0:ew, ec : ec + 1],
                        )
                        nc.sync.dma_start(
                            out=y[elo : elo + ew, qtr * IC : (qtr + 1) * IC],
                            in_=ysb[0:ew, :],
                        )

                q_proj(0)

                # ---- flat software-pipelined batch stream across all
                # i-chunks: QK(b)+exp(b) always emitted before PV(b-1), so
                # the in-order PE queue and the sp ring never drain at
                # chunk boundaries. Bookkeeping (norms, q-proj, AllGather,
                # output projection) is emitted as in-stream hooks.
                po_t = {}

                def qk(ic, kind, idx):
                    isl = slice(ic * IC, (ic + 1) * IC)
                    sp = spsum.tile([128, 1024], F32, tag="sp", bufs=3,
                                    name=f"sp{kind}_{ic}_{idx}")
                    if kind == "01":
                        nc.tensor.matmul(
                            sp[:, 0:IC],
                            kT01[0:64, idx * 128 : (idx + 1) * 128],
                            qT01_t[ic][0:64, :],
                            start=True, stop=True, tile_position=(0, 0),
                        )
                        nc.tensor.matmul(
                            sp[:, IC:1024],
                            kT01[64:128, idx * 128 : (idx + 1) * 128],
                            qT01_t[ic][64:128, :],
                            start=True, stop=True, tile_position=(64, 0),
                        )
                    else:
                        nc.tensor.matmul(
                            sp[:, 0:IC],
                            kT2[0:64, (2 * idx) * 128 : (2 * idx + 1) * 128],
                            qT2_t[ic][0:64, :],
                            start=True, stop=True, tile_position=(0, 0),
                        )
                        nc.tensor.matmul(
                            sp[:, IC:1024],
                            kT2[64:128, (2 * idx + 1) * 128 : (2 * idx + 2) * 128],
                            qT2_t[ic][64:128, :],
                            start=True, stop=True, tile_position=(64, 0),
                        )
                    return exp_batch(sp)

                def pv(ic, kind, idx, pt):
                    if kind == "01":
                        if idx == 0:
                            po_t[ic] = [
                                opsum.tile([128, IC], F32, tag="po0", bufs=1,
                                           name=f"po0_{ic}"),
                                opsum.tile([128, IC], F32, tag="po1", bufs=1,
                                           name=f"po1_{ic}"),
                            ]
                        po = po_t[ic]
                        nc.tensor.matmul(
                            po[0][:],
                            v_sb[:, idx * VW : idx * VW + 128],
                            pt[:, 0:IC],
                            start=(idx == 0), stop=(idx == 31),
                        )
                        nc.tensor.matmul(
                            po[1][:],
                            v_sb[:, idx * VW + 128 : idx * VW + 256],
                            pt[:, IC:1024],
                            start=(idx == 0), stop=(idx == 31),
                        )
                    else:
                        if idx == 0:
                            po_t[ic].append(
                                opsum.tile([128, IC], F32, tag="po0", bufs=1,
                                           name=f"po2_{ic}")
                            )
                        po = po_t[ic]
                        for s in range(2):
                            jc = 2 * idx + s
                            nc.tensor.matmul(
                                po[2][:],
                                v_sb[:, jc * VW + 256 : jc * VW + 384],
                                pt[:, s * IC : (s + 1) * IC],
                                start=(jc == 0), stop=(jc == 31),
                            )

                def post_pv_hooks(ic, kind, idx):
                    if kind == "01" and idx == 8 and ic + 1 < NIC:
                        q_proj(ic + 1)
                    elif kind == "01" and idx == 31:
                        norm_store(po_t[ic][0], ic, 0)
                        norm_store(po_t[ic][1], ic, 1)
                    elif kind == "2" and idx == 15:
                        norm_store(po_t[ic][2], ic, 2)
                        nc.gpsimd.collective_compute(
                            "AllGather",
                            mybir.AluOpType.bypass,
                            replica_groups=[[0, 1, 2, 3], [4, 5, 6, 7]],
                            ins=[oT_q[ic][:]],
                            outs=[ag_q[ic][:]],
                        )
                        if ic >= 1:
                            phase3_qtr(ic - 1)

                stream = [
                    (ic, kind, idx)
                    for ic in range(NIC)
                    for kind, count in (("01", 32), ("2", 16))
                    for idx in range(count)
                ]
                pending = None
                for b in stream:
                    pt = qk(*b)
                    if pending is not None:
                        pv(*pending[0], pending[1])
                        post_pv_hooks(*pending[0])
                    pending = (b, pt)
                pv(*pending[0], pending[1])
                post_pv_hooks(*pending[0])
                phase3_qtr(NIC - 1)

    nc.compile()
    _PROG_CACHE[key] = nc
    return nc


# ---------------------------------------------------------------- host wrapper
def make_in_maps(x, w_qkv, b_qkv, w_out, b_out):
    """Build the 8 per-core input dicts from full inputs."""
    in_maps = []
    xTb = [np.ascontiguousarray(x[b].T.astype(np.float16)) for b in range(B)]
    kscale = np.float32(SCALE / 8.0)
    for c in range(N_CORES):
        b = c // 4
        hs = HPC * (c % 4)

        def sect(kind, h):  # q=0,k=1,v=2
            lo = kind * (H * DH) + h * DH
            return w_qkv[:, lo : lo + DH], b_qkv[lo : lo + DH]

        q0, bq0 = sect(0, hs); q1, bq1 = sect(0, hs + 1); q2, bq2 = sect(0, hs + 2)
        k0, bk0 = sect(1, hs); k1, bk1 = sect(1, hs + 1); k2, bk2 = sect(1, hs + 2)
        v0, bv0 = sect(2, hs); v1, bv1 = sect(2, hs + 1); v2, bv2 = sect(2, hs + 2)
        z = np.zeros_like(q2); bz = np.zeros_like(bq2)
        # m-chunks: [q0|q1], [q2|q2], [k0|k1]*s, [k2|k2]*s, [v0|v1], [v2|0]
        cols = np.concatenate(
            [q0, q1, q2, q2, k0 * kscale, k1 * kscale, k2 * kscale, k2 * kscale,
             v0, v1, v2, z], axis=1).astype(np.float16)
        bias = np.concatenate(
            [bq0, bq1, bq2, bq2, bk0 * kscale, bk1 * kscale, bk2 * kscale,
             bk2 * kscale, bv0, bv1, bv2, bz]).astype(np.float32)
        q = c % 4
        bo = np.zeros((2, 128), np.float32)
        bo[0, :] = b_out[192 * q : 192 * q + 128]
        bo[1, :64] = b_out[192 * q + 128 : 192 * q + 192]
        in_maps.append({
            "xT": xTb[b],
            "wqkv": np.ascontiguousarray(cols),
            "bqkv": np.ascontiguousarray(bias.reshape(6, 128)),
            "wout": np.ascontiguousarray(
                w_out[:, 192 * q : 192 * (q + 1)].astype(np.float16)),
            "bout": bo,
        })
    return in_maps


def assemble_output(results):
    out = np.empty((B, N, D), dtype=np.float32)
    for c in range(N_CORES):
        b = c // 4
        q = c % 4
        out[b, :, 192 * q : 192 * (q + 1)] = results[c]["y"].T
    return out


def kernel(x, w_qkv, b_qkv, w_out, b_out):
    from concourse.bass_utils import run_bass_kernel_spmd

    x = np.asarray(x, dtype=np.float32)
    nc = build_program()
    in_maps = make_in_maps(
        x, np.asarray(w_qkv, np.float32), np.asarray(b_qkv, np.float32),
        np.asarray(w_out, np.float32), np.asarray(b_out, np.float32))
    res = run_bass_kernel_spmd(nc, in_maps, core_ids=list(range(N_CORES)))
    return assemble_output(res.results)


# revision 31
# speedup vs baseline: 1.5668x; 1.0561x over previous
"""Multi-head self-attention (B=2, N=4096, D=768, H=12, dh=64) on 8 trn2 NeuronCores.

Sharding: core c handles batch b=c//4 and heads 3*(c%4)..3*(c%4)+2 (head-parallel
attention); an AllGather per 512-token i-chunk redistributes head-outputs so each
core projects its own 192 output columns for all tokens (column-parallel output
projection), assembled host-side.

Per-core pipeline (all matmuls fp16 except psum accumulate):
  x is shipped fp16 and kept resident in SBUF (48KB/partition), so projections
  read it directly with 1024-wide moving operands and no per-pass DMA.
  pass 1: project k (pre-scaled by softmax_scale/8 host-side) and v for all
    tokens; k kept d-on-partition fp16 in row-tiled head-pair layout, v
    PE-transposed into v_sb as [v_h | ones64] fp16 blocks per 128-key chunk —
    the ones columns make each PV matmul emit the softmax denominator
    replicated across psum partitions 64:128 for free.
  pass 2 (interleaved with attention): q projected per 512-token chunk.
  attention per (i-chunk, key-chunk): row-tiled QK pair (concurrent PE
    row-strips) -> psum u = s/8 -> exp(8u) split ~11:5 between ACT table exp
    and a 2-op DVE polynomial (taylor4(u), x^8), DVE batches interleaved so
    the PE never idles a full HAM window -> fp16 PV with [v|ones] stationary
    -> reciprocal[64,512] + multiply -> fp16 AllGather -> column-parallel
    output projection with DVE bias add.
"""
import sys

sys.path.insert(0, "/opt/trn_rl_repo")

import numpy as np

import concourse.bass as bass
import concourse.mybir as mybir
import concourse.tile as tile
import concourse.bacc as bacc
from concourse.masks import make_identity

N_CORES = 8
B, N, D, H, DH = 2, 4096, 768, 12, 64
HPC = 3            # heads per core
SCALE = D ** -0.5
F32 = mybir.dt.float32
F16 = mybir.dt.float16
AF = mybir.ActivationFunctionType
IC = 512           # query chunk size
NIC = N // IC      # 8 i-chunks
VW = 384           # v_sb cols per 128-key chunk: [v0|1*64|v1|1*64|v2|1*64]
EXP_GROUP = 16     # exp batches: (i*EXP_DVE) % 16 < EXP_DVE go to the DVE path
EXP_DVE = 4        # evenly interleaved: every 4th batch on DVE


# ---------------------------------------------------------------- custom DVE exp
def _register_exp_ops():
    """exp(8u) as two DVE ops: EXP_P4_ANT = taylor4(u); EXP_SQ8_ANT = x^8."""
    import concourse.dve_ops as dve_ops
    from concourse.dve_ops import DveOp, OPS, CUSTOM_DVE_SPECS, _SUB_OPCODE_FOR_NAME
    from concourse.dve_spec import Spec, Src0, C0, C1, C2, One, sq, lower
    from concourse.dve_uop import DveOpSpec

    if "EXP_P4_ANT" in _SUB_OPCODE_FOR_NAME:
        return dve_ops.EXP_P4_ANT, dve_ops.EXP_SQ8_ANT

    u = Src0
    p4 = ((((u * C0) + C1) * u + C2) * u + One) * u + One  # c0=1/24 c1=1/6 c2=1/2
    spec_p4 = Spec(
        body=p4,
        reference=lambda in0, in1, s0, s1, imm2: (
            (((in0 * s0 + s1) * in0 + imm2) * in0 + 1.0) * in0 + 1.0
        ),
    )
    spec_sq8 = Spec(
        body=sq(sq(sq(Src0))),
        reference=lambda in0, in1, s0, s1, imm2: in0 ** 8,
    )

    def _mk(name, spec):
        opcode = max(_SUB_OPCODE_FOR_NAME.values()) + 1
        _SUB_OPCODE_FOR_NAME[name] = opcode
        shas = {}
        for ver in ("v3", "v4"):
            s = DveOpSpec(
                name=name, opcode=opcode, uops=lower(spec, ver=ver), rd1_en=False
            )
            shas[ver] = s.sha(ver)
        op = DveOp(name, spec, subdim=False, uops_sha=shas)
        OPS.append(op)
        CUSTOM_DVE_SPECS[name] = spec
        setattr(dve_ops, name, op)
        return op

    p4_op = _mk("EXP_P4_ANT", spec_p4)
    sq8_op = _mk("EXP_SQ8_ANT", spec_sq8)
    return p4_op, sq8_op


# ---------------------------------------------------------------- program build
_PROG_CACHE = {}


def build_program(use_dve_exp=True):
    key = ("prog", use_dve_exp)
    if key in _PROG_CACHE:
        return _PROG_CACHE[key]
    p4_op, sq8_op = _register_exp_ops()

    nc = bacc.Bacc("TRN2", target_bir_lowering=False, debug=False, num_devices=N_CORES)

    xT = nc.dram_tensor("xT", [D, N], F16, kind="ExternalInput").ap()
    wqkv = nc.dram_tensor("wqkv", [D, 768], F16, kind="ExternalInput").ap()
    bqkv = nc.dram_tensor("bqkv", [6, 128], F32, kind="ExternalInput").ap()
    wout = nc.dram_tensor("wout", [D, 192], F16, kind="ExternalInput").ap()
    bout = nc.dram_tensor("bout", [2, 128], F32, kind="ExternalInput").ap()
    y = nc.dram_tensor("y", [HPC * DH, N], F32, kind="ExternalOutput").ap()

    with tile.TileContext(nc, trace_sim=False) as tc:
        with (
            tc.tile_pool(name="consts", bufs=1) as consts,
            tc.tile_pool(name="persist", bufs=1) as persist,
            tc.tile_pool(name="otp", bufs=2) as otp,
            tc.tile_pool(name="spsum", bufs=3, space="PSUM") as spsum,
            tc.tile_pool(name="opsum", bufs=1, space="PSUM") as opsum,
            tc.tile_pool(name="dram", bufs=1, space="DRAM") as dram,
        ):
            # ---------------- constants + resident fp16 x
            ident_f = consts.tile([128, 128], F32)
            make_identity(nc, ident_f[:])
            ident16 = consts.tile([128, 128], F16)
            nc.scalar.copy(ident16[:], ident_f[:])

            # resident fp16 x, one tile per 1024-token block so early
            # projections only wait on their own block's DMAs
            x16_t = [
                persist.tile([128, 6 * 1024], F16, tag=f"x16_{nq}",
                             name=f"x16_{nq}")
                for nq in range(4)
            ]
            for nq in range(4):
                for fc in range(6):
                    for pb in range(2):
                        nc.sync.dma_start(
                            out=x16_t[nq][64 * pb : 64 * (pb + 1),
                                          fc * 1024 : (fc + 1) * 1024],
                            in_=xT[fc * 128 + 64 * pb : fc * 128 + 64 * (pb + 1),
                                   nq * 1024 : (nq + 1) * 1024],
                        )
            wq_sb = consts.tile([128, 6 * 768], F16)  # 6 f-chunks of [128,768]
            for fc in range(6):
                nc.sync.dma_start(
                    out=wq_sb[:, fc * 768 : (fc + 1) * 768],
                    in_=wqkv[fc * 128 : (fc + 1) * 128, :],
                )
            bq_sb = consts.tile([128, 6], F32)  # per-m-chunk bias columns
            for m in range(6):
                nc.sync.dma_start(
                    out=bq_sb[:, m : m + 1],
                    in_=bqkv[m : m + 1, :].rearrange("a p -> p a"),
                )

            # ---------------- persistent activations (all fp16)
            # q tiles are per-i-chunk so projecting chunk ic+1 mid-attention
            # carries no tile-granularity WAR against chunk ic's reads
            qT01_t = [
                persist.tile([128, IC], F16, tag=f"q01_{i}", name=f"q01_{i}")
                for i in range(NIC)
            ]
            qT2_t = [
                persist.tile([128, IC], F16, tag=f"q2_{i}", name=f"q2_{i}")
                for i in range(NIC)
            ]
            kT01 = persist.tile([128, N], F16)  # scaled kT, heads 0/1
            kT2 = persist.tile([128, N], F16)   # scaled kT head2, duplicated
            v_sb = persist.tile([128, 32 * VW], F16)
            v_view = v_sb[:].rearrange("p (j c) -> p j c", c=VW)
            for h in range(HPC):
                nc.vector.memset(v_view[:, :, h * 128 + 64 : h * 128 + 128], 1.0)

            # AllGather eighths: in [192, 512] -> out [768, 512] (rank-major rows)
            oT_q = [dram.tile([HPC * DH, IC], F16, name=f"oT_q{i}") for i in range(8)]
            ag_q = [dram.tile([D, IC], F16, name=f"ag_q{i}") for i in range(8)]

            # output projection weights (phase 3 is emitted inline per i-chunk)
            wo_sb = consts.tile([128, 6 * 192], F16)  # w_out col-slice, 6 d-chunks
            for dc in range(6):
                nc.sync.dma_start(
                    out=wo_sb[:, dc * 192 : (dc + 1) * 192],
                    in_=wout[dc * 128 : (dc + 1) * 128, :],
                )
            bo_sb = consts.tile([128, 2], F32)
            for m in range(2):
                nc.sync.dma_start(
                    out=bo_sb[:, m : m + 1],
                    in_=bout[m : m + 1, :].rearrange("a p -> p a"),
                )

            with tc.tile_pool(name="work", bufs=2) as work, \
                 tc.tile_pool(name="ptp", bufs=3) as ptp, \
                 tc.tile_pool(name="ph3", bufs=1) as ph3:

                def proj_chunk(m, tsl, out_psum):
                    """fp16 matmul of m-chunk over token slice tsl into psum."""
                    nq, off = tsl.start // 1024, tsl.start % 1024
                    w = tsl.stop - tsl.start
                    for fc in range(6):
                        nc.tensor.matmul(
                            out_psum,
                            wq_sb[:, fc * 768 + m * 128 : fc * 768 + (m + 1) * 128],
                            x16_t[nq][:, fc * 1024 + off : fc * 1024 + off + w],
                            start=(fc == 0),
                            stop=(fc == 5),
                        )

                # ---------------- pass 1: k + v projection (q deferred)
                for tau in range(8):
                    ts = slice(tau * IC, (tau + 1) * IC)
                    for m, dst in ((2, kT01), (3, kT2)):
                        sp = spsum.tile([128, 1024], F32, tag="sp", bufs=3)
                        proj_chunk(m, ts, sp[:, 0:IC])
                        nc.scalar.activation(
                            dst[:, ts], sp[:, 0:IC], AF.Identity,
                            bias=bq_sb[:, m : m + 1],
                        )
                    vt01 = work.tile([128, IC], F16, tag="vt01")
                    vt2 = work.tile([64, IC], F16, tag="vt2")
                    sp = spsum.tile([128, 1024], F32, tag="sp", bufs=3)
                    proj_chunk(4, ts, sp[:, 0:IC])
                    nc.scalar.activation(
                        vt01[:], sp[:, 0:IC], AF.Identity, bias=bq_sb[:, 4:5]
                    )
                    sp = spsum.tile([128, 1024], F32, tag="sp", bufs=3)
                    proj_chunk(5, ts, sp[:, 0:IC])
                    nc.scalar.activation(
                        vt2[:], sp[0:64, 0:IC], AF.Identity, bias=bq_sb[0:64, 5:6]
                    )
                    # transpose v into token-major [v|ones] layout, 4 t-blocks
                    # per psum tile: [v01(t0..t3) 512 | v2(t0..t3) 256]
                    pv = spsum.tile([128, 2048], F16, tag="sp", bufs=3)
                    for s in range(4):
                        nc.tensor.transpose(
                            pv[:, s * 128 : (s + 1) * 128],
                            vt01[:, s * 128 : (s + 1) * 128],
                            ident16[:],
                        )
                        nc.tensor.transpose(
                            pv[:, 512 + s * 64 : 512 + (s + 1) * 64],
                            vt2[0:64, s * 128 : (s + 1) * 128],
                            ident16[0:64, 0:64],
                        )
                    jc0 = 4 * tau
                    pv01 = pv[:, 0:512].rearrange("p (t c) -> p t c", c=128)
                    pv2 = pv[:, 512:768].rearrange("p (t c) -> p t c", c=64)
                    nc.vector.tensor_copy(
                        out=v_view[:, jc0 : jc0 + 4, 0:64], in_=pv01[:, :, 0:64]
                    )
                    nc.vector.tensor_copy(
                        out=v_view[:, jc0 : jc0 + 4, 128:192],
                        in_=pv01[:, :, 64:128],
                    )
                    nc.vector.tensor_copy(
                        out=v_view[:, jc0 : jc0 + 4, 256:320], in_=pv2[:, :, :]
                    )

                # ---------------- pass 2: q projection + attention per i-chunk
                exp_batch_idx = [0]

                def exp_batch(sp):
                    """exp(8u) on a [128,1024] psum batch -> fp16 SBUF tile."""
                    i = exp_batch_idx[0]
                    exp_batch_idx[0] += 1
                    pt = ptp.tile([128, 1024], F16, tag="pt", bufs=5)
                    if use_dve_exp and ((i * EXP_DVE) % EXP_GROUP) < EXP_DVE:
                        tmp = ptp.tile([128, 1024], F16, tag="exptmp", bufs=2)
                        nc.vector._custom_dve(
                            p4_op, out=tmp[:], in0=sp[:],
                            s0=1.0 / 24, s1=1.0 / 6, imm2=0.5,
                        )
                        nc.vector._custom_dve(sq8_op, out=pt[:], in0=tmp[:])
                    else:
                        nc.scalar.activation(pt[:], sp[:], AF.Exp, scale=8.0)
                    return pt

                def q_proj(ic):
                    isl = slice(ic * IC, (ic + 1) * IC)
                    for m, dst in ((0, qT01_t[ic]), (1, qT2_t[ic])):
                        pp = spsum.tile([128, 1024], F32, tag="sp", bufs=3,
                                        name=f"pp{m}_{ic}")
                        proj_chunk(m, isl, pp[:, 0:IC])
                        nc.scalar.activation(
                            dst[:], pp[:, 0:IC], AF.Identity,
                            bias=bq_sb[:, m : m + 1],
                        )

                def norm_store(po_h, ic, h):
                    """o_h = num / l. Builtin reciprocal handles the cross-base
                    read (custom DVE ops only work at partition base 0)."""
                    rr = otp.tile([128, IC], F32, tag="rr", bufs=2)
                    nc.vector.reciprocal(rr[0:64, :], po_h[64:128, :])
                    ot = otp.tile([64, IC], F16, tag="ot", bufs=3)
                    nc.vector.tensor_tensor(
                        out=ot[:], in0=po_h[0:64, :], in1=rr[0:64, :],
                        op=mybir.AluOpType.mult,
                    )
                    nc.sync.dma_start(
                        out=oT_q[ic][64 * h : 64 * h + 64, :], in_=ot[:]
                    )

                def phase3_qtr(qtr):
                    agr = ag_q[qtr][:]
                    ogs = []
                    for dc in range(6):
                        og = ph3.tile([128, IC], F16, tag="og", bufs=8,
                                      name=f"og{dc}_{qtr}")
                        nc.sync.dma_start(
                            out=og[:], in_=agr[dc * 128 : (dc + 1) * 128, :]
                        )
                        ogs.append(og)
                    for ec, (elo, ew) in enumerate(((0, 128), (128, 64))):
                        py = spsum.tile([128, 1024], F32, tag="sp", bufs=3,
                                        name=f"py{ec}_{qtr}")
                        for dc in range(6):
                            nc.tensor.matmul(
                                py[0:ew, 0:IC],
                                wo_sb[:, dc * 192 + elo : dc * 192 + elo + ew],
                                ogs[dc][:],
                                start=(dc == 0), stop=(dc == 5),
                            )
                        ysb = ph3.tile([128, IC], F32, tag="ysb", bufs=3,
                                       name=f"ysb{ec}_{qtr}")
                        nc.scalar.activation(
                            ysb[0:ew, :], py[0:ew, 0:IC], AF.Identity,
                            bias=bo_sb[0:ew, ec : ec + 1],
                        )
                        nc.sync.dma_start(
                            out=y[elo : elo + ew, qtr * IC : (qtr + 1) * IC],
                            in_=ysb[0:ew, :],
                        )

                q_proj(0)

                # ---- flat software-pipelined batch stream across all
                # i-chunks: QK(b)+exp(b) always emitted before PV(b-1), so
                # the in-order PE queue and the sp ring never drain at
                # chunk boundaries. Bookkeeping (norms, q-proj, AllGather,
                # output projection) is emitted as in-stream hooks.
                po_t = {}

                def qk(ic, kind, idx):
                    isl = slice(ic * IC, (ic + 1) * IC)
                    sp = spsum.tile([128, 1024], F32, tag="sp", bufs=3,
                                    name=f"sp{kind}_{ic}_{idx}")
                    if kind == "01":
                        nc.tensor.matmul(
                            sp[:, 0:IC],
                            kT01[0:64, idx * 128 : (idx + 1) * 128],
                            qT01_t[ic][0:64, :],
                            start=True, stop=True, tile_position=(0, 0),
                        )
                        nc.tensor.matmul(
                            sp[:, IC:1024],
                            kT01[64:128, idx * 128 : (idx + 1) * 128],
                            qT01_t[ic][64:128, :],
                            start=True, stop=True, tile_position=(64, 0),
                        )
                    else:
                        nc.tensor.matmul(
                            sp[:, 0:IC],
                            kT2[0:64, (2 * idx) * 128 : (2 * idx + 1) * 128],
                            qT2_t[ic][0:64, :],
                            start=True, stop=True, tile_position=(0, 0),
                        )
                        nc.tensor.matmul(
                            sp[:, IC:1024],
                            kT2[64:128, (2 * idx + 1) * 128 : (2 * idx + 2) * 128],
                            qT2_t[ic][64:128, :],
                            start=True, stop=True, tile_position=(64, 0),
                        )
                    return exp_batch(sp)

                def pv(ic, kind, idx, pt):
                    if kind == "01":
                        if idx == 0:
                            po_t[ic] = [
                                opsum.tile([128, IC], F32, tag="po0", bufs=1,
                                           name=f"po0_{ic}"),
                                opsum.tile([128, IC], F32, tag="po1", bufs=1,
                                           name=f"po1_{ic}"),
                            ]
                        po = po_t[ic]
                        nc.tensor.matmul(
                            po[0][:],
                            v_sb[:, idx * VW : idx * VW + 128],
                            pt[:, 0:IC],
                            start=(idx == 0), stop=(idx == 31),
                        )
                        nc.tensor.matmul(
                            po[1][:],
                            v_sb[:, idx * VW + 128 : idx * VW + 256],
                            pt[:, IC:1024],
                            start=(idx == 0), stop=(idx == 31),
                        )
                    else:
                        if idx == 0:
                            po_t[ic].append(
                                opsum.tile([128, IC], F32, tag="po0", bufs=1,
                                           name=f"po2_{ic}")
                            )
                        po = po_t[ic]
                        for s in range(2):
                            jc = 2 * idx + s
                            nc.tensor.matmul(
                                po[2][:],
                                v_sb[:, jc * VW + 256 : jc * VW + 384],
                                pt[:, s * IC : (s + 1) * IC],
                                start=(jc == 0), stop=(jc == 31),
                            )

                def post_pv_hooks(ic, kind, idx):
                    if kind == "01" and idx == 8 and ic + 1 < NIC:
                        q_proj(ic + 1)
                    elif kind == "01" and idx == 31:
                        norm_store(po_t[ic][0], ic, 0)
                        norm_store(po_t[ic][1], ic, 1)
                    elif kind == "2" and idx == 15:
                        norm_store(po_t[ic][2], ic, 2)
                        nc.gpsimd.collective_compute(
                            "AllGather",
                            mybir.AluOpType.bypass,
                            replica_groups=[[0, 1, 2, 3], [4, 5, 6, 7]],
                            ins=[oT_q[ic][:]],
                            outs=[ag_q[ic][:]],
                        )
                        if ic >= 1:
                            phase3_qtr(ic - 1)

                stream = [
                    (ic, kind, idx)
                    for ic in range(NIC)
                    for kind, count in (("01", 32), ("2", 16))
                    for idx in range(count)
                ]
                from collections import deque
                pending = deque()
                for b in stream:
                    pt = qk(*b)
                    pending.append((b, pt))
                    if len(pending) > 2:
                        done = pending.popleft()
                        pv(*done[0], done[1])
                        post_pv_hooks(*done[0])
                while pending:
                    done = pending.popleft()
                    pv(*done[0], done[1])
                    post_pv_hooks(*done[0])
                phase3_qtr(NIC - 1)

    nc.compile()
    _PROG_CACHE[key] = nc
    return nc


# ---------------------------------------------------------------- host wrapper
def make_in_maps(x, w_qkv, b_qkv, w_out, b_out):
    """Build the 8 per-core input dicts from full inputs."""
    in_maps = []
    xTb = [np.ascontiguousarray(x[b].T.astype(np.float16)) for b in range(B)]
    kscale = np.float32(SCALE / 8.0)
    for c in range(N_CORES):
        b = c // 4
        hs = HPC * (c % 4)

        def sect(kind, h):  # q=0,k=1,v=2
            lo = kind * (H * DH) + h * DH
            return w_qkv[:, lo : lo + DH], b_qkv[lo : lo + DH]

        q0, bq0 = sect(0, hs); q1, bq1 = sect(0, hs + 1); q2, bq2 = sect(0, hs + 2)
        k0, bk0 = sect(1, hs); k1, bk1 = sect(1, hs + 1); k2, bk2 = sect(1, hs + 2)
        v0, bv0 = sect(2, hs); v1, bv1 = sect(2, hs + 1); v2, bv2 = sect(2, hs + 2)
        z = np.zeros_like(q2); bz = np.zeros_like(bq2)
        # m-chunks: [q0|q1], [q2|q2], [k0|k1]*s, [k2|k2]*s, [v0|v1], [v2|0]
        cols = np.concatenate(
            [q0, q1, q2, q2, k0 * kscale, k1 * kscale, k2 * kscale, k2 * kscale,
             v0, v1, v2, z], axis=1).astype(np.float16)
        bias = np.concatenate(
            [bq0, bq1, bq2, bq2, bk0 * kscale, bk1 * kscale, bk2 * kscale,
             bk2 * kscale, bv0, bv1, bv2, bz]).astype(np.float32)
        q = c % 4
        bo = np.zeros((2, 128), np.float32)
        bo[0, :] = b_out[192 * q : 192 * q + 128]
        bo[1, :64] = b_out[192 * q + 128 : 192 * q + 192]
        in_maps.append({
            "xT": xTb[b],
            "wqkv": np.ascontiguousarray(cols),
            "bqkv": np.ascontiguousarray(bias.reshape(6, 128)),
            "wout": np.ascontiguousarray(
                w_out[:, 192 * q : 192 * (q + 1)].astype(np.float16)),
            "bout": bo,
        })
    return in_maps


def assemble_output(results):
    out = np.empty((B, N, D), dtype=np.float32)
    for c in range(N_CORES):
        b = c // 4
        q = c % 4
        out[b, :, 192 * q : 192 * (q + 1)] = results[c]["y"].T
    return out


def kernel(x, w_qkv, b_qkv, w_out, b_out):
    from concourse.bass_utils import run_bass_kernel_spmd

    x = np.asarray(x, dtype=np.float32)
    nc = build_program()
    in_maps = make_in_maps(
        x, np.asarray(w_qkv, np.float32), np.asarray(b_qkv, np.float32),
        np.asarray(w_out, np.float32), np.asarray(b_out, np.float32))
    res = run_bass_kernel_spmd(nc, in_maps, core_ids=list(range(N_CORES)))
    return assemble_output(res.results)


# revision 32
# speedup vs baseline: 1.6396x; 1.0465x over previous
"""Multi-head self-attention (B=2, N=4096, D=768, H=12, dh=64) on 8 trn2 NeuronCores.

Sharding: core c handles batch b=c//4 and heads 3*(c%4)..3*(c%4)+2 (head-parallel
attention); an AllGather per 512-token i-chunk redistributes head-outputs so each
core projects its own 192 output columns for all tokens (column-parallel output
projection), assembled host-side.

Per-core pipeline (all matmuls fp16 except psum accumulate):
  x is shipped fp16 and kept resident in SBUF (48KB/partition), so projections
  read it directly with 1024-wide moving operands and no per-pass DMA.
  pass 1: project k (pre-scaled by softmax_scale/8 host-side) and v for all
    tokens; k kept d-on-partition fp16 in row-tiled head-pair layout, v
    PE-transposed into v_sb as [v_h | ones64] fp16 blocks per 128-key chunk —
    the ones columns make each PV matmul emit the softmax denominator
    replicated across psum partitions 64:128 for free.
  pass 2 (interleaved with attention): q projected per 512-token chunk.
  attention per (i-chunk, key-chunk): row-tiled QK pair (concurrent PE
    row-strips) -> psum u = s/8 -> exp(8u) split ~11:5 between ACT table exp
    and a 2-op DVE polynomial (taylor4(u), x^8), DVE batches interleaved so
    the PE never idles a full HAM window -> fp16 PV with [v|ones] stationary
    -> reciprocal[64,512] + multiply -> fp16 AllGather -> column-parallel
    output projection with DVE bias add.
"""
import sys

sys.path.insert(0, "/opt/trn_rl_repo")

import numpy as np

import concourse.bass as bass
import concourse.mybir as mybir
import concourse.tile as tile
import concourse.bacc as bacc
from concourse.masks import make_identity

N_CORES = 8
B, N, D, H, DH = 2, 4096, 768, 12, 64
HPC = 3            # heads per core
SCALE = D ** -0.5
F32 = mybir.dt.float32
F16 = mybir.dt.float16
AF = mybir.ActivationFunctionType
IC = 512           # query chunk size
NIC = N // IC      # 8 i-chunks
VW = 384           # v_sb cols per 128-key chunk: [v0|1*64|v1|1*64|v2|1*64]
EXP_GROUP = 32     # exp batches: (i*EXP_DVE) % EXP_GROUP < EXP_DVE -> DVE path
EXP_DVE = 9        # ~28% of batches on DVE, evenly interleaved


# ---------------------------------------------------------------- custom DVE exp
def _register_exp_ops():
    """exp(8u) as two DVE ops: EXP_P4_ANT = taylor4(u); EXP_SQ8_ANT = x^8."""
    import concourse.dve_ops as dve_ops
    from concourse.dve_ops import DveOp, OPS, CUSTOM_DVE_SPECS, _SUB_OPCODE_FOR_NAME
    from concourse.dve_spec import Spec, Src0, C0, C1, C2, One, sq, lower
    from concourse.dve_uop import DveOpSpec

    if "EXP_P4_ANT" in _SUB_OPCODE_FOR_NAME:
        return dve_ops.EXP_P4_ANT, dve_ops.EXP_SQ8_ANT

    u = Src0
    p4 = ((((u * C0) + C1) * u + C2) * u + One) * u + One  # c0=1/24 c1=1/6 c2=1/2
    spec_p4 = Spec(
        body=p4,
        reference=lambda in0, in1, s0, s1, imm2: (
            (((in0 * s0 + s1) * in0 + imm2) * in0 + 1.0) * in0 + 1.0
        ),
    )
    spec_sq8 = Spec(
        body=sq(sq(sq(Src0))),
        reference=lambda in0, in1, s0, s1, imm2: in0 ** 8,
    )

    def _mk(name, spec):
        opcode = max(_SUB_OPCODE_FOR_NAME.values()) + 1
        _SUB_OPCODE_FOR_NAME[name] = opcode
        shas = {}
        for ver in ("v3", "v4"):
            s = DveOpSpec(
                name=name, opcode=opcode, uops=lower(spec, ver=ver), rd1_en=False
            )
            shas[ver] = s.sha(ver)
        op = DveOp(name, spec, subdim=False, uops_sha=shas)
        OPS.append(op)
        CUSTOM_DVE_SPECS[name] = spec
        setattr(dve_ops, name, op)
        return op

    p4_op = _mk("EXP_P4_ANT", spec_p4)
    sq8_op = _mk("EXP_SQ8_ANT", spec_sq8)
    return p4_op, sq8_op


# ---------------------------------------------------------------- program build
_PROG_CACHE = {}


def build_program(use_dve_exp=True):
    key = ("prog", use_dve_exp)
    if key in _PROG_CACHE:
        return _PROG_CACHE[key]
    p4_op, sq8_op = _register_exp_ops()

    nc = bacc.Bacc("TRN2", target_bir_lowering=False, debug=False, num_devices=N_CORES)

    xT = nc.dram_tensor("xT", [D, N], F16, kind="ExternalInput").ap()
    wqkv = nc.dram_tensor("wqkv", [D, 768], F16, kind="ExternalInput").ap()
    bqkv = nc.dram_tensor("bqkv", [6, 128], F32, kind="ExternalInput").ap()
    wout = nc.dram_tensor("wout", [D, 192], F16, kind="ExternalInput").ap()
    bout = nc.dram_tensor("bout", [2, 128], F32, kind="ExternalInput").ap()
    y = nc.dram_tensor("y", [HPC * DH, N], F32, kind="ExternalOutput").ap()

    with tile.TileContext(nc, trace_sim=False) as tc:
        with (
            tc.tile_pool(name="consts", bufs=1) as consts,
            tc.tile_pool(name="persist", bufs=1) as persist,
            tc.tile_pool(name="otp", bufs=2) as otp,
            tc.tile_pool(name="spsum", bufs=3, space="PSUM") as spsum,
            tc.tile_pool(name="opsum", bufs=1, space="PSUM") as opsum,
            tc.tile_pool(name="dram", bufs=1, space="DRAM") as dram,
        ):
            # ---------------- constants + resident fp16 x
            ident_f = consts.tile([128, 128], F32)
            make_identity(nc, ident_f[:])
            ident16 = consts.tile([128, 128], F16)
            nc.scalar.copy(ident16[:], ident_f[:])

            # resident fp16 x, one tile per 1024-token block so early
            # projections only wait on their own block's DMAs
            x16_t = [
                persist.tile([128, 6 * 1024], F16, tag=f"x16_{nq}",
                             name=f"x16_{nq}")
                for nq in range(4)
            ]
            for nq in range(4):
                for fc in range(6):
                    for pb in range(2):
                        nc.sync.dma_start(
                            out=x16_t[nq][64 * pb : 64 * (pb + 1),
                                          fc * 1024 : (fc + 1) * 1024],
                            in_=xT[fc * 128 + 64 * pb : fc * 128 + 64 * (pb + 1),
                                   nq * 1024 : (nq + 1) * 1024],
                        )
            wq_sb = consts.tile([128, 6 * 768], F16)  # 6 f-chunks of [128,768]
            for fc in range(6):
                nc.sync.dma_start(
                    out=wq_sb[:, fc * 768 : (fc + 1) * 768],
                    in_=wqkv[fc * 128 : (fc + 1) * 128, :],
                )
            bq_sb = consts.tile([128, 6], F32)  # per-m-chunk bias columns
            for m in range(6):
                nc.sync.dma_start(
                    out=bq_sb[:, m : m + 1],
                    in_=bqkv[m : m + 1, :].rearrange("a p -> p a"),
                )

            # ---------------- persistent activations (all fp16)
            # q tiles are per-i-chunk so projecting chunk ic+1 mid-attention
            # carries no tile-granularity WAR against chunk ic's reads
            qT01_t = [
                persist.tile([128, IC], F16, tag=f"q01_{i}", name=f"q01_{i}")
                for i in range(NIC)
            ]
            qT2_t = [
                persist.tile([128, IC], F16, tag=f"q2_{i}", name=f"q2_{i}")
                for i in range(NIC)
            ]
            kT01 = persist.tile([128, N], F16)  # scaled kT, heads 0/1
            kT2 = persist.tile([128, N], F16)   # scaled kT head2, duplicated
            v_sb = persist.tile([128, 32 * VW], F16)
            v_view = v_sb[:].rearrange("p (j c) -> p j c", c=VW)
            for h in range(HPC):
                nc.vector.memset(v_view[:, :, h * 128 + 64 : h * 128 + 128], 1.0)

            # AllGather eighths: in [192, 512] -> out [768, 512] (rank-major rows)
            oT_q = [dram.tile([HPC * DH, IC], F16, name=f"oT_q{i}") for i in range(8)]
            ag_q = [dram.tile([D, IC], F16, name=f"ag_q{i}") for i in range(8)]

            # output projection weights (phase 3 is emitted inline per i-chunk)
            wo_sb = consts.tile([128, 6 * 192], F16)  # w_out col-slice, 6 d-chunks
            for dc in range(6):
                nc.sync.dma_start(
                    out=wo_sb[:, dc * 192 : (dc + 1) * 192],
                    in_=wout[dc * 128 : (dc + 1) * 128, :],
                )
            bo_sb = consts.tile([128, 2], F32)
            for m in range(2):
                nc.sync.dma_start(
                    out=bo_sb[:, m : m + 1],
                    in_=bout[m : m + 1, :].rearrange("a p -> p a"),
                )

            with tc.tile_pool(name="work", bufs=2) as work, \
                 tc.tile_pool(name="ptp", bufs=3) as ptp, \
                 tc.tile_pool(name="ph3", bufs=1) as ph3:

                def proj_chunk(m, tsl, out_psum):
                    """fp16 matmul of m-chunk over token slice tsl into psum."""
                    nq, off = tsl.start // 1024, tsl.start % 1024
                    w = tsl.stop - tsl.start
                    for fc in range(6):
                        nc.tensor.matmul(
                            out_psum,
                            wq_sb[:, fc * 768 + m * 128 : fc * 768 + (m + 1) * 128],
                            x16_t[nq][:, fc * 1024 + off : fc * 1024 + off + w],
                            start=(fc == 0),
                            stop=(fc == 5),
                        )

                # ---------------- pass 1: k + v projection (q deferred)
                for tau in range(8):
                    ts = slice(tau * IC, (tau + 1) * IC)
                    for m, dst in ((2, kT01), (3, kT2)):
                        sp = spsum.tile([128, 1024], F32, tag="sp", bufs=3)
                        proj_chunk(m, ts, sp[:, 0:IC])
                        nc.scalar.activation(
                            dst[:, ts], sp[:, 0:IC], AF.Identity,
                            bias=bq_sb[:, m : m + 1],
                        )
                    vt01 = work.tile([128, IC], F16, tag="vt01")
                    vt2 = work.tile([64, IC], F16, tag="vt2")
                    sp = spsum.tile([128, 1024], F32, tag="sp", bufs=3)
                    proj_chunk(4, ts, sp[:, 0:IC])
                    nc.scalar.activation(
                        vt01[:], sp[:, 0:IC], AF.Identity, bias=bq_sb[:, 4:5]
                    )
                    sp = spsum.tile([128, 1024], F32, tag="sp", bufs=3)
                    proj_chunk(5, ts, sp[:, 0:IC])
                    nc.scalar.activation(
                        vt2[:], sp[0:64, 0:IC], AF.Identity, bias=bq_sb[0:64, 5:6]
                    )
                    # transpose v into token-major [v|ones] layout, 4 t-blocks
                    # per psum tile: [v01(t0..t3) 512 | v2(t0..t3) 256]
                    pv = spsum.tile([128, 2048], F16, tag="sp", bufs=3)
                    for s in range(4):
                        nc.tensor.transpose(
                            pv[:, s * 128 : (s + 1) * 128],
                            vt01[:, s * 128 : (s + 1) * 128],
                            ident16[:],
                        )
                        nc.tensor.transpose(
                            pv[:, 512 + s * 64 : 512 + (s + 1) * 64],
                            vt2[0:64, s * 128 : (s + 1) * 128],
                            ident16[0:64, 0:64],
                        )
                    jc0 = 4 * tau
                    pv01 = pv[:, 0:512].rearrange("p (t c) -> p t c", c=128)
                    pv2 = pv[:, 512:768].rearrange("p (t c) -> p t c", c=64)
                    nc.vector.tensor_copy(
                        out=v_view[:, jc0 : jc0 + 4, 0:64], in_=pv01[:, :, 0:64]
                    )
                    nc.vector.tensor_copy(
                        out=v_view[:, jc0 : jc0 + 4, 128:192],
                        in_=pv01[:, :, 64:128],
                    )
                    nc.vector.tensor_copy(
                        out=v_view[:, jc0 : jc0 + 4, 256:320], in_=pv2[:, :, :]
                    )

                # ---------------- pass 2: q projection + attention per i-chunk
                exp_batch_idx = [0]

                def exp_batch(sp):
                    """exp(8u) on a [128,1024] psum batch -> fp16 SBUF tile."""
                    i = exp_batch_idx[0]
                    exp_batch_idx[0] += 1
                    pt = ptp.tile([128, 1024], F16, tag="pt", bufs=5)
                    if use_dve_exp and ((i * EXP_DVE) % EXP_GROUP) < EXP_DVE:
                        tmp = ptp.tile([128, 1024], F16, tag="exptmp", bufs=2)
                        nc.vector._custom_dve(
                            p4_op, out=tmp[:], in0=sp[:],
                            s0=1.0 / 24, s1=1.0 / 6, imm2=0.5,
                        )
                        nc.vector._custom_dve(sq8_op, out=pt[:], in0=tmp[:])
                    else:
                        nc.scalar.activation(pt[:], sp[:], AF.Exp, scale=8.0)
                    return pt

                def q_proj(ic):
                    isl = slice(ic * IC, (ic + 1) * IC)
                    for m, dst in ((0, qT01_t[ic]), (1, qT2_t[ic])):
                        pp = spsum.tile([128, 1024], F32, tag="sp", bufs=3,
                                        name=f"pp{m}_{ic}")
                        proj_chunk(m, isl, pp[:, 0:IC])
                        nc.scalar.activation(
                            dst[:], pp[:, 0:IC], AF.Identity,
                            bias=bq_sb[:, m : m + 1],
                        )

                def norm_store(po_h, ic, h):
                    """o_h = num / l. ACT copies l down to partition base 0
                    (ACT handles cross-base; custom DVE ops do not), then the
                    fast approx reciprocal and the multiply run base-aligned."""
                    l0 = otp.tile([64, IC], F32, tag="l0", bufs=2)
                    nc.scalar.copy(l0[:], po_h[64:128, :])
                    rr = otp.tile([64, IC], F32, tag="rr", bufs=2)
                    nc.vector.reciprocal_approx_fast(rr[:], l0[:])
                    ot = otp.tile([64, IC], F16, tag="ot", bufs=3)
                    nc.vector.tensor_tensor(
                        out=ot[:], in0=po_h[0:64, :], in1=rr[:],
                        op=mybir.AluOpType.mult,
                    )
                    nc.sync.dma_start(
                        out=oT_q[ic][64 * h : 64 * h + 64, :], in_=ot[:]
                    )

                def phase3_qtr(qtr):
                    agr = ag_q[qtr][:]
                    ogs = []
                    for dc in range(6):
                        og = ph3.tile([128, IC], F16, tag="og", bufs=8,
                                      name=f"og{dc}_{qtr}")
                        nc.sync.dma_start(
                            out=og[:], in_=agr[dc * 128 : (dc + 1) * 128, :]
                        )
                        ogs.append(og)
                    for ec, (elo, ew) in enumerate(((0, 128), (128, 64))):
                        py = spsum.tile([128, 1024], F32, tag="sp", bufs=3,
                                        name=f"py{ec}_{qtr}")
                        for dc in range(6):
                            nc.tensor.matmul(
                                py[0:ew, 0:IC],
                                wo_sb[:, dc * 192 + elo : dc * 192 + elo + ew],
                                ogs[dc][:],
                                start=(dc == 0), stop=(dc == 5),
                            )
                        ysb = ph3.tile([128, IC], F32, tag="ysb", bufs=3,
                                       name=f"ysb{ec}_{qtr}")
                        nc.scalar.activation(
                            ysb[0:ew, :], py[0:ew, 0:IC], AF.Identity,
                            bias=bo_sb[0:ew, ec : ec + 1],
                        )
                        nc.sync.dma_start(
                            out=y[elo : elo + ew, qtr * IC : (qtr + 1) * IC],
                            in_=ysb[0:ew, :],
                        )

                q_proj(0)

                # ---- flat software-pipelined batch stream across all
                # i-chunks: QK(b)+exp(b) always emitted before PV(b-1), so
                # the in-order PE queue and the sp ring never drain at
                # chunk boundaries. Bookkeeping (norms, q-proj, AllGather,
                # output projection) is emitted as in-stream hooks.
                po_t = {}

                def qk(ic, kind, idx):
                    isl = slice(ic * IC, (ic + 1) * IC)
                    sp = spsum.tile([128, 1024], F32, tag="sp", bufs=3,
                                    name=f"sp{kind}_{ic}_{idx}")
                    if kind == "01":
                        nc.tensor.matmul(
                            sp[:, 0:IC],
                            kT01[0:64, idx * 128 : (idx + 1) * 128],
                            qT01_t[ic][0:64, :],
                            start=True, stop=True, tile_position=(0, 0),
                        )
                        nc.tensor.matmul(
                            sp[:, IC:1024],
                            kT01[64:128, idx * 128 : (idx + 1) * 128],
                            qT01_t[ic][64:128, :],
                            start=True, stop=True, tile_position=(64, 0),
                        )
                    else:
                        nc.tensor.matmul(
                            sp[:, 0:IC],
                            kT2[0:64, (2 * idx) * 128 : (2 * idx + 1) * 128],
                            qT2_t[ic][0:64, :],
                            start=True, stop=True, tile_position=(0, 0),
                        )
                        nc.tensor.matmul(
                            sp[:, IC:1024],
                            kT2[64:128, (2 * idx + 1) * 128 : (2 * idx + 2) * 128],
                            qT2_t[ic][64:128, :],
                            start=True, stop=True, tile_position=(64, 0),
                        )
                    return exp_batch(sp)

                def pv(ic, kind, idx, pt):
                    if kind == "01":
                        if idx == 0:
                            po_t[ic] = [
                                opsum.tile([128, IC], F32, tag="po0", bufs=1,
                                           name=f"po0_{ic}"),
                                opsum.tile([128, IC], F32, tag="po1", bufs=1,
                                           name=f"po1_{ic}"),
                            ]
                        po = po_t[ic]
                        nc.tensor.matmul(
                            po[0][:],
                            v_sb[:, idx * VW : idx * VW + 128],
                            pt[:, 0:IC],
                            start=(idx == 0), stop=(idx == 31),
                        )
                        nc.tensor.matmul(
                            po[1][:],
                            v_sb[:, idx * VW + 128 : idx * VW + 256],
                            pt[:, IC:1024],
                            start=(idx == 0), stop=(idx == 31),
                        )
                    else:
                        if idx == 0:
                            po_t[ic].append(
                                opsum.tile([128, IC], F32, tag="po0", bufs=1,
                                           name=f"po2_{ic}")
                            )
                        po = po_t[ic]
                        for s in range(2):
                            jc = 2 * idx + s
                            nc.tensor.matmul(
                                po[2][:],
                                v_sb[:, jc * VW + 256 : jc * VW + 384],
                                pt[:, s * IC : (s + 1) * IC],
                                start=(jc == 0), stop=(jc == 31),
                            )

                def post_pv_hooks(ic, kind, idx):
                    if kind == "01" and idx == 8 and ic + 1 < NIC:
                        q_proj(ic + 1)
                    elif kind == "01" and idx == 31:
                        norm_store(po_t[ic][0], ic, 0)
                        norm_store(po_t[ic][1], ic, 1)
                    elif kind == "2" and idx == 15:
                        norm_store(po_t[ic][2], ic, 2)
                        nc.gpsimd.collective_compute(
                            "AllGather",
                            mybir.AluOpType.bypass,
                            replica_groups=[[0, 1, 2, 3], [4, 5, 6, 7]],
                            ins=[oT_q[ic][:]],
                            outs=[ag_q[ic][:]],
                        )
                        if ic >= 1:
                            phase3_qtr(ic - 1)

                stream = [
                    (ic, kind, idx)
                    for ic in range(NIC)
                    for kind, count in (("01", 32), ("2", 16))
                    for idx in range(count)
                ]
                from collections import deque
                pending = deque()
                for b in stream:
                    pt = qk(*b)
                    pending.append((b, pt))
                    if len(pending) > 2:
                        done = pending.popleft()
                        pv(*done[0], done[1])
                        post_pv_hooks(*done[0])
                while pending:
                    done = pending.popleft()
                    pv(*done[0], done[1])
                    post_pv_hooks(*done[0])
                phase3_qtr(NIC - 1)

    nc.compile()
    _PROG_CACHE[key] = nc
    return nc


# ---------------------------------------------------------------- host wrapper
def make_in_maps(x, w_qkv, b_qkv, w_out, b_out):
    """Build the 8 per-core input dicts from full inputs."""
    in_maps = []
    xTb = [np.ascontiguousarray(x[b].T.astype(np.float16)) for b in range(B)]
    kscale = np.float32(SCALE / 8.0)
    for c in range(N_CORES):
        b = c // 4
        hs = HPC * (c % 4)

        def sect(kind, h):  # q=0,k=1,v=2
            lo = kind * (H * DH) + h * DH
            return w_qkv[:, lo : lo + DH], b_qkv[lo : lo + DH]

        q0, bq0 = sect(0, hs); q1, bq1 = sect(0, hs + 1); q2, bq2 = sect(0, hs + 2)
        k0, bk0 = sect(1, hs); k1, bk1 = sect(1, hs + 1); k2, bk2 = sect(1, hs + 2)
        v0, bv0 = sect(2, hs); v1, bv1 = sect(2, hs + 1); v2, bv2 = sect(2, hs + 2)
        z = np.zeros_like(q2); bz = np.zeros_like(bq2)
        # m-chunks: [q0|q1], [q2|q2], [k0|k1]*s, [k2|k2]*s, [v0|v1], [v2|0]
        cols = np.concatenate(
            [q0, q1, q2, q2, k0 * kscale, k1 * kscale, k2 * kscale, k2 * kscale,
             v0, v1, v2, z], axis=1).astype(np.float16)
        bias = np.concatenate(
            [bq0, bq1, bq2, bq2, bk0 * kscale, bk1 * kscale, bk2 * kscale,
             bk2 * kscale, bv0, bv1, bv2, bz]).astype(np.float32)
        q = c % 4
        bo = np.zeros((2, 128), np.float32)
        bo[0, :] = b_out[192 * q : 192 * q + 128]
        bo[1, :64] = b_out[192 * q + 128 : 192 * q + 192]
        in_maps.append({
            "xT": xTb[b],
            "wqkv": np.ascontiguousarray(cols),
            "bqkv": np.ascontiguousarray(bias.reshape(6, 128)),
            "wout": np.ascontiguousarray(
                w_out[:, 192 * q : 192 * (q + 1)].astype(np.float16)),
            "bout": bo,
        })
    return in_maps


def assemble_output(results):
    out = np.empty((B, N, D), dtype=np.float32)
    for c in range(N_CORES):
        b = c // 4
        q = c % 4
        out[b, :, 192 * q : 192 * (q + 1)] = results[c]["y"].T
    return out


def kernel(x, w_qkv, b_qkv, w_out, b_out):
    from concourse.bass_utils import run_bass_kernel_spmd

    x = np.asarray(x, dtype=np.float32)
    nc = build_program()
    in_maps = make_in_maps(
        x, np.asarray(w_qkv, np.float32), np.asarray(b_qkv, np.float32),
        np.asarray(w_out, np.float32), np.asarray(b_out, np.float32))
    res = run_bass_kernel_spmd(nc, in_maps, core_ids=list(range(N_CORES)))
    return assemble_output(res.results)
